# revision 24
# baseline (speedup 1.0000x reference)
# DeepSeek block (MLA attention + top-2-of-8 MoE + shared expert) on 8 TRN2
# NeuronCores, zero-collective sharding.
#
# Core c in [0..8): sequence b = c//4, q = c%4; owns token chunks
# hi = 7-q (slot 0) and lo = q (slot 1), 256 tokens each (causally balanced).
# SPMD: identical program on all cores; per-core data (x columns, rope
# tables, causal masks) arrives as inputs.
#
# v2 layout/perf notes:
# - activations feature-on-partition; matmuls bf16 except MoE which runs
#   fp8e4 DoubleRow (both operands packed [128,2,*], contract 256/matmul).
#   MoE weights pre-scaled x64 on host (fp8e4 min normal 2^-6), down input
#   h carries x8; output rescaled by 1/512 at the final accumulate.
# - one DMA per logical matrix (DRAM tensors pre-arranged (128, blk, cols)).
# - wide ACT ops ([128,1024] exp/silu) to amortize the fixed ACT overhead;
#   causal masks applied only to key blocks 8..15 of slot0 and 0..7 of
#   slot1 (interior blocks are mask-free on every core).
# - expert accumulation on the gpsimd (Pool) engine into SBUF, down
#   projections transient in PSUM.
import os
import numpy as np
import ml_dtypes

import concourse.bacc as bacc
import concourse.mybir as mybir
import concourse.tile as tile
from concourse import bass_utils

F32 = mybir.dt.float32
BF16 = mybir.dt.bfloat16
F8 = mybir.dt.float8e4
AF = mybir.ActivationFunctionType
ALU = mybir.AluOpType
DR = mybir.MatmulPerfMode.DoubleRow

B, T, C, H, D = 2, 2048, 1024, 16, 64
R, ROPE, NOPE = 128, 32, 32
E, I = 8, 512
THETA, EPS = 100000.0, 1e-5
P = 128
NCB = C // P             # 8 C blocks
NIB = I // P             # 4 I blocks
TLOC, CHUNK = 512, 256
KB_SLOT = (16, 8)        # key blocks (of 128) attended per chunk slot

WSC = 64.0               # fp8 weight prescale
HSC = 8.0                # fp8 hidden prescale
OSC = 1.0 / (WSC * HSC)  # down-psum rescale

_CACHE = {}


# =============================================================== device IR
def _emit(nc, tc):
    import contextlib

    def din(name, shape, dt):
        return nc.dram_tensor(name, shape, dt, kind="ExternalInput")

    xt_d = din("xt", (P, NCB, T), BF16)
    xloc_d = din("xloc", (P, NCB, TLOC), F32)
    xlbf_d = din("xlbf", (P, NCB, TLOC), BF16)
    wq_d = din("wq", (P, NCB, H * D), BF16)
    wkva_d = din("wkva", (P, NCB, R + ROPE), BF16)
    wkvb_d = din("wkvb", (R, H * NOPE), BF16)
    wo_d = din("wo", (P, 4, C), BF16)
    cosk_d = din("cosk", (ROPE, T), F32)
    ssink_d = din("ssink", (ROPE, T), F32)
    cosq_d = din("cosq", (2 * ROPE, TLOC), F32)
    ssinq_d = din("ssinq", (2 * ROPE, TLOC), F32)
    perm64_d = din("perm64", (2 * ROPE, 2 * ROPE), BF16)
    perm32_d = din("perm32", (ROPE, ROPE), BF16)
    ident_d = din("ident", (P, P), F32)
    kmask_d = din("kmask", (P, 16 * CHUNK), BF16)
    wgate_d = din("wgate", (P, NCB, E), F32)
    biasg_d = din("biasg", (P, E), F32)
    wg_d = din("wg8", (E + 1, P, NCB * I), F8)   # index 0 = shared expert
    wu_d = din("wu8", (E + 1, P, NCB * I), F8)
    wd_d = din("wd8", (E + 1, P, NIB * C), F8)
    out_d = nc.dram_tensor("outT", (P, NCB, TLOC), F32, kind="ExternalOutput")
    DEBUG = bool(int(os.environ.get("BASSK_DEBUG", "0")))
    if DEBUG:
        dxa_d = nc.dram_tensor("d_xa", (P, NCB, TLOC), F32,
                               kind="ExternalOutput")
        dcm_d = nc.dram_tensor("d_comb", (E, TLOC), BF16,
                               kind="ExternalOutput")
        dem_d = nc.dram_tensor("d_em", (P, 16 * CHUNK), BF16,
                               kind="ExternalOutput")
        dac_d = nc.dram_tensor("d_acc", (P, 2, NIB, TLOC), F32,
                               kind="ExternalOutput")

    whole = contextlib.ExitStack()
    early = contextlib.ExitStack()
    attn = contextlib.ExitStack()

    pc = whole.enter_context(tc.tile_pool(name="pc", bufs=1))
    pw = whole.enter_context(tc.tile_pool(name="pw", bufs=1, side="right"))
    pmid = whole.enter_context(tc.tile_pool(name="pmid", bufs=1))

    # pa: tiles written during the early phase but read by attention
    pa = attn.enter_context(tc.tile_pool(name="pa", bufs=1))
    pt2 = attn.enter_context(tc.tile_pool(name="pt2", bufs=2))

    px = early.enter_context(tc.tile_pool(name="px", bufs=1))
    pt1 = early.enter_context(tc.tile_pool(name="pt1", bufs=2))
    pse = early.enter_context(tc.tile_pool(name="pse", bufs=2, space="PSUM"))

    # ---- constants / tables
    ones_128x1 = pc.tile([P, 1], BF16)
    nc.any.memset(ones_128x1[:], 1.0)
    ones1f = pc.tile([1, 1], F32)
    nc.any.memset(ones1f[:], 1.0)
    eps_sb = pc.tile([1, 1], F32)
    nc.any.memset(eps_sb[:], EPS)
    ident_sb = pc.tile([P, P], F32)
    nc.sync.dma_start(ident_sb[:], ident_d.ap())
    perm64_sb = pc.tile([2 * ROPE, 2 * ROPE], BF16)
    nc.sync.dma_start(perm64_sb[:], perm64_d.ap())
    perm32_sb = pc.tile([ROPE, ROPE], BF16)
    nc.sync.dma_start(perm32_sb[:], perm32_d.ap())
    biasg_sb = pc.tile([P, E], F32)
    nc.sync.dma_start(biasg_sb[:], biasg_d.ap())
    wgate_sb = pc.tile([P, NCB, E], F32)
    nc.sync.dma_start(wgate_sb[:], wgate_d.ap())

    cosk_t = px.tile([ROPE, T], F32)
    nc.sync.dma_start(cosk_t[:], cosk_d.ap())
    ssink_t = px.tile([ROPE, T], F32)
    nc.sync.dma_start(ssink_t[:], ssink_d.ap())
    cosq_t = px.tile([2 * ROPE, TLOC], F32)
    nc.sync.dma_start(cosq_t[:], cosq_d.ap())
    ssinq_t = px.tile([2 * ROPE, TLOC], F32)
    nc.sync.dma_start(ssinq_t[:], ssinq_d.ap())

    # ---- bulk loads
    xt = px.tile([P, NCB, T], BF16)
    nc.sync.dma_start(xt[:], xt_d.ap())
    xlbf = px.tile([P, NCB, TLOC], BF16)
    nc.sync.dma_start(xlbf[:], xlbf_d.ap())
    wq_sb = px.tile([P, NCB, H * D], BF16)
    nc.sync.dma_start(wq_sb[:], wq_d.ap())
    wkva_sb = px.tile([P, NCB, R + ROPE], BF16)
    nc.sync.dma_start(wkva_sb[:], wkva_d.ap())

    # ---- rmsnorm1 stats: global (keys) then local (queries)
    bc1 = px.tile([P, T], F32)
    for nt in range(T // 512):
        sl = slice(nt * 512, (nt + 1) * 512)
        sps = pse.tile([1, 512], F32, name="sps", tag="accA")
        for cb in range(NCB):
            xq = pt1.tile([P, 512], BF16, name="xq", tag="xq")
            nc.vector.tensor_tensor(xq[:], xt[:, cb, sl], xt[:, cb, sl],
                                    ALU.mult)
            nc.tensor.matmul(sps[:], ones_128x1[:], xq[:],
                             start=(cb == 0), stop=(cb == NCB - 1))
        rr = pt1.tile([1, 512], F32, name="rr", tag="rr", bufs=1)
        nc.scalar.activation(rr[:], sps[:], AF.Sqrt, bias=eps_sb[:],
                             scale=1.0 / C)
        iv = pt1.tile([1, 512], F32, name="iv", tag="iv", bufs=1)
        nc.vector.reciprocal(iv[:], rr[:])
        nc.gpsimd.partition_broadcast(bc1[:, sl], iv[:])

    bc1l = px.tile([P, TLOC], F32)
    spsl = pse.tile([1, TLOC], F32, name="spsl", tag="accA")
    for cb in range(NCB):
        xql = pt1.tile([P, TLOC], BF16, name="xql", tag="xq")
        nc.vector.tensor_tensor(xql[:], xlbf[:, cb, :], xlbf[:, cb, :],
                                ALU.mult)
        nc.tensor.matmul(spsl[:], ones_128x1[:], xql[:],
                         start=(cb == 0), stop=(cb == NCB - 1))
    rrl = pt1.tile([1, TLOC], F32, name="rrl", tag="rr", bufs=1)
    nc.scalar.activation(rrl[:], spsl[:], AF.Sqrt, bias=eps_sb[:],
                         scale=1.0 / C)
    ivl = pt1.tile([1, TLOC], F32, name="ivl", tag="iv", bufs=1)
    nc.vector.reciprocal(ivl[:], rrl[:])
    nc.gpsimd.partition_broadcast(bc1l[:], ivl[:])

    # ---- ckv: kv latent (scaled) + scaled k_rope
    kvlat = pa.tile([R, T], BF16)
    krsc = pa.tile([ROPE, T], BF16)     # scaled raw k_rope
    for nt in range(T // 512):
        sl = slice(nt * 512, (nt + 1) * 512)
        lat_ps = pse.tile([P, 512], F32, name="lat_ps", tag="pA")
        for cb in range(NCB):
            nc.tensor.matmul(lat_ps[:], wkva_sb[:, cb, 0:R], xt[:, cb, sl],
                             start=(cb == 0), stop=(cb == NCB - 1))
        rop_ps = pse.tile([ROPE, 512], F32, name="rop_ps", tag="par")
        for cb in range(NCB):
            nc.tensor.matmul(rop_ps[:], wkva_sb[:, cb, R:R + ROPE],
                             xt[:, cb, sl],
                             start=(cb == 0), stop=(cb == NCB - 1))
        nc.vector.tensor_tensor(kvlat[:, sl], lat_ps[:], bc1[:, sl], ALU.mult)
        nc.vector.tensor_tensor(krsc[:, sl], rop_ps[:], bc1[0:ROPE, sl],
                                ALU.mult)

    # ---- rope K -> kropebf [32, T]
    kropebf = pa.tile([ROPE, T], BF16)
    for nt in range(T // 512):
        sl = slice(nt * 512, (nt + 1) * 512)
        park = pse.tile([ROPE, 512], F32, name="park", tag="par")
        nc.tensor.matmul(park[:], perm32_sb[:], krsc[:, sl])
        t1k = pt1.tile([ROPE, 512], F32, name="t1k", tag="t1q")
        nc.gpsimd.tensor_tensor(t1k[:], krsc[:, sl], cosk_t[:, sl], ALU.mult)
        t2k = pt1.tile([ROPE, 512], F32, name="t2k", tag="t2q")
        nc.vector.tensor_tensor(t2k[:], park[:], ssink_t[:, sl], ALU.mult)
        nc.vector.tensor_tensor(kropebf[:, sl], t1k[:], t2k[:], ALU.add)

    # ---- Q projection + rope (whole TLOC per head-pair)
    qbf = []
    for mb in range(8):
        tl = pa.tile([P, TLOC], BF16, name=f"qbf{mb}")
        qps = pse.tile([P, TLOC], F32, name="qps", tag="pA")
        for cb in range(NCB):
            nc.tensor.matmul(qps[:], wq_sb[:, cb, mb * P:(mb + 1) * P],
                             xlbf[:, cb, :],
                             start=(cb == 0), stop=(cb == NCB - 1))
        nc.vector.tensor_tensor(tl[:], qps[:], bc1l[:], ALU.mult)
        qr = pt1.tile([2 * ROPE, TLOC], BF16, name="qr", tag="qr")
        nc.vector.tensor_tensor(qr[0:ROPE, :], qps[32:64, :], bc1l[32:64, :],
                                ALU.mult)
        nc.vector.tensor_tensor(qr[ROPE:2 * ROPE, :], qps[96:128, :],
                                bc1l[96:128, :], ALU.mult)
        parq = pse.tile([2 * ROPE, TLOC], F32, name="parq", tag="par")
        nc.tensor.matmul(parq[:], perm64_sb[:], qr[:])
        t1q = pt1.tile([2 * ROPE, TLOC], F32, name="t1q", tag="t1q")
        nc.gpsimd.tensor_tensor(t1q[:], qr[:], cosq_t[:], ALU.mult)
        t2q = pt1.tile([2 * ROPE, TLOC], F32, name="t2q", tag="t2q")
        nc.vector.tensor_tensor(t2q[:], parq[:], ssinq_t[:], ALU.mult)
        nc.vector.tensor_tensor(tl[32:64, :], t1q[0:ROPE, :], t2q[0:ROPE, :],
                                ALU.add)
        nc.vector.tensor_tensor(tl[96:128, :], t1q[ROPE:2 * ROPE, :],
                                t2q[ROPE:2 * ROPE, :], ALU.add)
        qbf.append(tl)

    early.close()

    # ---- deferred loads (space freed by the early pools)
    pk = attn.enter_context(tc.tile_pool(name="pk", bufs=1))
    xloc = pk.tile([P, NCB, TLOC], F32)
    nc.sync.dma_start(xloc[:], xloc_d.ap())
    wkvb_sb = pk.tile([R, H * NOPE], BF16)
    nc.sync.dma_start(wkvb_sb[:], wkvb_d.ap())
    kmask_sb = pk.tile([P, 16 * CHUNK], BF16)
    nc.sync.dma_start(kmask_sb[:], kmask_d.ap())
    wo_sb = pk.tile([P, 4, C], BF16)
    nc.sync.dma_start(wo_sb[:], wo_d.ap())

    # ---- k_nope -> kfull assembly; V extended with ones row
    sub = contextlib.ExitStack()
    pkx = sub.enter_context(tc.tile_pool(name="pkx", bufs=1))
    psk = sub.enter_context(tc.tile_pool(name="psk", bufs=2, space="PSUM"))
    knope = []
    for mb in range(4):
        tl = pkx.tile([P, T], BF16, name=f"knope{mb}")
        for nt in range(T // 512):
            sl = slice(nt * 512, (nt + 1) * 512)
            kps = psk.tile([P, 512], F32, name="kps", tag="pA")
            nc.tensor.matmul(kps[:], wkvb_sb[:, mb * P:(mb + 1) * P],
                             kvlat[:, sl])
            nc.scalar.copy(tl[:, sl], kps[:])
        knope.append(tl)
    vext = []
    for tb in range(16):
        tl = pk.tile([P, H, 34], BF16, name=f"vext{tb}")
        vps = psk.tile([P, H * NOPE], F32, name="vps", tag="pA")
        nc.tensor.matmul(vps[:], kvlat[:, tb * P:(tb + 1) * P], wkvb_sb[:])
        nc.scalar.copy(tl[:, :, 0:NOPE],
                       vps[:].rearrange("p (h d) -> p h d", h=H))
        nc.any.memset(tl[:, :, NOPE:NOPE + 1], 1.0)
        vext.append(tl)
    kfull = []
    for mb in range(H // 2):
        tl = pk.tile([P, T], BF16, name=f"kfull{mb}")
        h0, h1 = 2 * mb, 2 * mb + 1
        nc.sync.dma_start(tl[0:32, :],
                          knope[h0 // 4][(h0 % 4) * 32:(h0 % 4) * 32 + 32, :])
        nc.sync.dma_start(tl[64:96, :],
                          knope[h1 // 4][(h1 % 4) * 32:(h1 % 4) * 32 + 32, :])
        nc.sync.dma_start(tl[32:64, :], kropebf[:])
        nc.sync.dma_start(tl[96:128, :], kropebf[:])
        kfull.append(tl)
    sub.close()

    # ---- attention core
    core = contextlib.ExitStack()
    psc = core.enter_context(tc.tile_pool(name="psc", bufs=2, space="PSUM"))
    pE = core.enter_context(tc.tile_pool(name="pE", bufs=2))
    yall = []
    for yb in range(4):
        yall.append(pk.tile([P, TLOC], BF16, name=f"yall{yb}"))
    for ch in range(2):
        csl = slice(ch * CHUNK, (ch + 1) * CHUNK)
        nkb = KB_SLOT[ch]
        for h in range(H):
            mb, po = h // 2, (h % 2) * 64
            em = pE.tile([P, nkb * CHUNK], BF16, name="em", tag="em")
            for g in range(nkb // 4):
                sp = psc.tile([P, 4 * CHUNK], F32, name="sp", tag="sc")
                for k4 in range(4):
                    kb = 4 * g + k4
                    nc.tensor.matmul(
                        sp[:, k4 * CHUNK:(k4 + 1) * CHUNK],
                        kfull[mb][po:po + 64, kb * P:(kb + 1) * P],
                        qbf[mb][po:po + 64, csl])
                nc.scalar.activation(em[:, g * 4 * CHUNK:(g + 1) * 4 * CHUNK],
                                     sp[:], AF.Exp, scale=0.125)
            if ch == 0:
                nc.vector.tensor_tensor(em[:, 8 * CHUNK:16 * CHUNK],
                                        em[:, 8 * CHUNK:16 * CHUNK],
                                        kmask_sb[:, 0:8 * CHUNK], ALU.mult)
            else:
                nc.vector.tensor_tensor(em[:], em[:],
                                        kmask_sb[:, 8 * CHUNK:16 * CHUNK],
                                        ALU.mult)
            if DEBUG and ch == 0 and h == 0:
                nc.sync.dma_start(dem_d.ap(), em[:])
            y_ps = psc.tile([NOPE + 1, CHUNK], F32, name="y_ps", tag="yv")
            for kb in range(nkb):
                nc.tensor.matmul(y_ps[:], vext[kb][:, h, 0:NOPE + 1],
                                 em[:, kb * CHUNK:(kb + 1) * CHUNK],
                                 start=(kb == 0), stop=(kb == nkb - 1))
            rr2 = pt2.tile([1, CHUNK], F32, name="rr2", tag="rrA")
            nc.vector.reciprocal(rr2[:], y_ps[NOPE:NOPE + 1, :])
            rb = pt2.tile([NOPE, CHUNK], F32, name="rb", tag="rb")
            nc.gpsimd.partition_broadcast(rb[:], rr2[:])
            yt = yall[h // 4]
            ro = (h % 4) * NOPE
            nc.vector.tensor_tensor(yt[ro:ro + NOPE, csl], y_ps[0:NOPE, :],
                                    rb[:], ALU.mult)
    core.close()

    # ---- Wo + residual -> xa (f32) ; rmsnorm2 ; gate ; comb broadcast
    fin = contextlib.ExitStack()
    psg = fin.enter_context(tc.tile_pool(name="psg", bufs=2, space="PSUM"))
    psh = fin.enter_context(tc.tile_pool(name="psh", bufs=1, space="PSUM"))

    xa = []
    for cb in range(NCB):
        xa.append(pmid.tile([P, TLOC], F32, name=f"xa{cb}"))
    for cb in range(NCB):
        ops = psg.tile([P, TLOC], F32, name="ops", tag="wo")
        for kb in range(4):
            nc.tensor.matmul(ops[:], wo_sb[:, kb, cb * P:(cb + 1) * P],
                             yall[kb][:], start=(kb == 0), stop=(kb == 3))
        nc.vector.scalar_tensor_tensor(xa[cb][:], ops[:], 1.0,
                                       xloc[:, cb, :],
                                       op0=ALU.mult, op1=ALU.add)

    invr2 = pmid.tile([1, TLOC], F32)
    sps2 = psh.tile([1, TLOC], F32, name="sps2", tag="acc2")
    for cb in range(NCB):
        xq2 = pt2.tile([P, TLOC], BF16, name="xq2", tag="xq2")
        nc.vector.tensor_tensor(xq2[:], xa[cb][:], xa[cb][:], ALU.mult)
        nc.tensor.matmul(sps2[:], ones_128x1[:], xq2[:],
                         start=(cb == 0), stop=(cb == NCB - 1))
    rr3 = pt2.tile([1, TLOC], F32, name="rr3", tag="rrA")
    nc.scalar.activation(rr3[:], sps2[:], AF.Sqrt, bias=eps_sb[:],
                         scale=1.0 / C)
    nc.vector.reciprocal(invr2[:], rr3[:])
    bc2 = pt2.tile([P, TLOC], F32, name="bc2", tag="bc2")
    nc.gpsimd.partition_broadcast(bc2[:], invr2[:])

    xmf8 = pmid.tile([P, NCB, TLOC], F8)
    for cb in range(NCB):
        nc.vector.tensor_tensor(xmf8[:, cb, :], xa[cb][:], bc2[:], ALU.mult)
    if DEBUG:
        for cb in range(NCB):
            nc.sync.dma_start(dxa_d.ap()[:, cb, :], xa[cb][:])

    # gate (fp32, from xa scaled by invr2 via transposed column)
    ct_all = pmid.tile([E, TLOC], BF16)
    for tb in range(4):
        tsl = slice(tb * P, (tb + 1) * P)
        g_ps = psh.tile([P, E], F32, name="g_ps", tag="gps")
        for cb in range(NCB):
            nc.tensor.matmul(g_ps[:], xa[cb][:, tsl], wgate_sb[:, cb, :],
                             start=(cb == 0), stop=(cb == NCB - 1))
        ir_ps = psh.tile([P, 1], F32, name="ir_ps", tag="irp")
        nc.tensor.transpose(ir_ps[:], invr2[:, tsl], ones1f[:])
        ir_col = pt2.tile([P, 1], F32, name="ir_col", tag="irc")
        nc.scalar.copy(ir_col[:], ir_ps[:])
        lg = pt2.tile([P, E], F32, name="lg", tag="lg")
        nc.vector.scalar_tensor_tensor(lg[:], g_ps[:], ir_col[:], biasg_sb[:],
                                       op0=ALU.mult, op1=ALU.add)
        m1 = pt2.tile([P, 1], F32, name="m1", tag="m1")
        nc.vector.reduce_max(m1[:], lg[:], axis=mybir.AxisListType.X)
        eq1 = pt2.tile([P, E], F32, name="eq1", tag="eq1")
        nc.vector.tensor_scalar(eq1[:], lg[:], m1[:], None, op0=ALU.is_equal)
        lm = pt2.tile([P, E], F32, name="lm", tag="lm")
        nc.vector.scalar_tensor_tensor(lm[:], eq1[:], -1e9, lg[:],
                                       op0=ALU.mult, op1=ALU.add)
        m2 = pt2.tile([P, 1], F32, name="m2", tag="m2")
        nc.vector.reduce_max(m2[:], lm[:], axis=mybir.AxisListType.X)
        eq2 = pt2.tile([P, E], F32, name="eq2", tag="eq2")
        nc.vector.tensor_scalar(eq2[:], lm[:], m2[:], None, op0=ALU.is_equal)
        dm = pt2.tile([P, 1], F32, name="dm", tag="dm")
        nc.vector.tensor_scalar(dm[:], m1[:], m2[:], None, op0=ALU.subtract)
        w1 = pt2.tile([P, 1], F32, name="w1", tag="w1")
        nc.scalar.activation(w1[:], dm[:], AF.Sigmoid)
        w2 = pt2.tile([P, 1], F32, name="w2", tag="w2")
        nc.vector.tensor_scalar(w2[:], w1[:], -1.0, 1.0, op0=ALU.mult,
                                op1=ALU.add)
        cmb = pt2.tile([P, E], F32, name="cmb", tag="cmb")
        nc.vector.tensor_scalar(cmb[:], eq1[:], w1[:], None, op0=ALU.mult)
        cm2 = pt2.tile([P, E], F32, name="cm2", tag="cm2")
        nc.vector.tensor_scalar(cm2[:], eq2[:], w2[:], None, op0=ALU.mult)
        cmf = pt2.tile([P, E], F32, name="cmf", tag="cmf")
        nc.vector.tensor_tensor(cmf[:], cmb[:], cm2[:], ALU.add)
        ct_ps = psh.tile([E, P], F32, name="ct_ps", tag="ctp")
        nc.tensor.transpose(ct_ps[:], cmf[:], ident_sb[:])
        nc.scalar.copy(ct_all[:, tsl], ct_ps[:])
    if DEBUG:
        nc.sync.dma_start(dcm_d.ap(), ct_all[:])
    bcomb = []
    for e in range(E):
        cte = pt2.tile([1, TLOC], BF16, name="cte", tag="cte")
        nc.sync.dma_start(cte[:], ct_all[e:e + 1, :])
        tl = pmid.tile([P, TLOC], BF16, name=f"bcomb{e}")
        nc.gpsimd.partition_broadcast(tl[:], cte[:])
        bcomb.append(tl)

    fin.close()
    attn.close()

    # ---- MoE: fp8 DoubleRow, quad-of-experts accumulation in PSUM.
    # Quads: [shared], [e1..e4], [e5..e8]. Within a quad the token-half loop
    # is outer so the down psum (4 banks) accumulates all its experts; one
    # flush to SBUF per (quad, half, group). Weights pool holds up to 4
    # experts + 1 prefetch (pw bufs=5).
    moe = contextlib.ExitStack()
    pgu = moe.enter_context(tc.tile_pool(name="pgu", bufs=1, space="PSUM"))
    pwd = moe.enter_context(tc.tile_pool(name="pwd", bufs=1, space="PSUM"))
    pmoe = moe.enter_context(tc.tile_pool(name="pmoe", bufs=2))
    pac2 = moe.enter_context(tc.tile_pool(name="pac2", bufs=1))

    accs = [pac2.tile([P, NIB, TLOC], F32, name=f"acc{g}") for g in range(2)]

    quads = [[0], [1, 2], [3, 4], [5, 6], [7, 8]]
    wq_tiles = {}
    for qi, quad in enumerate(quads):
        for e in quad:
            wgt = pw.tile([P, NCB, I], F8, name="wgt", tag=f"wgt{e % 3}")
            nc.sync.dma_start(wgt[:], wg_d.ap()[e])
            wut = pw.tile([P, NCB, I], F8, name="wut", tag=f"wut{e % 3}")
            nc.sync.dma_start(wut[:], wu_d.ap()[e])
            wdt = pw.tile([P, NIB, C], F8, name="wdt", tag=f"wdt{e % 3}")
            nc.sync.dma_start(wdt[:], wd_d.ap()[e])
            wq_tiles[e] = (wgt, wut, wdt)
        for th in range(2):
            hsl = slice(th * CHUNK, (th + 1) * CHUNK)
            dps = [pwd.tile([P, NIB, CHUNK], F32, name=f"dps{g}", tag=f"d{g}")
                   for g in range(2)]
            hps = []
            for ei, e in enumerate(quad):
                wgt, wut, wdt = wq_tiles[e]
                gp = pgu.tile([P, NIB, CHUNK], F32, name="gp", tag="gp")
                for ib in range(NIB):
                    isl = slice(ib * P, (ib + 1) * P)
                    for j in range(4):
                        nc.tensor.matmul(gp[:, ib, :],
                                         wgt[:, 2 * j:2 * j + 2, isl],
                                         xmf8[:, 2 * j:2 * j + 2, hsl],
                                         start=(j == 0), stop=(j == 3),
                                         perf_mode=DR)
                sg = pmoe.tile([P, NIB, CHUNK], BF16, name="sg", tag="sg")
                nc.scalar.activation(sg[:], gp[:], AF.Silu, scale=1.0 / WSC)
                if e > 0:
                    sgc = pmoe.tile([P, NIB, CHUNK], BF16, name="sgc",
                                    tag="sgc")
                    bce = bcomb[e - 1]
                    for ib in range(NIB):
                        nc.vector.tensor_tensor(sgc[:, ib, :], sg[:, ib, :],
                                                bce[:, hsl], ALU.mult)
                else:
                    sgc = sg
                up = pgu.tile([P, NIB, CHUNK], F32, name="up", tag="up")
                for ib in range(NIB):
                    isl = slice(ib * P, (ib + 1) * P)
                    for j in range(4):
                        nc.tensor.matmul(up[:, ib, :],
                                         wut[:, 2 * j:2 * j + 2, isl],
                                         xmf8[:, 2 * j:2 * j + 2, hsl],
                                         start=(j == 0), stop=(j == 3),
                                         perf_mode=DR)
                hp = pmoe.tile([P, NIB, CHUNK], F8, name="hp", tag="hp",
                               bufs=3)
                nc.vector.scalar_tensor_tensor(hp[:], up[:], 1.0 / HSC,
                                               sgc[:],
                                               op0=ALU.mult, op1=ALU.mult)
                hps.append(hp)
            # region-major down: a PSUM region's accumulation group must
            # finish before the next group in the same bank starts
            # (start_tensor_calc clears has_written bank-wide).
            for cb in range(NCB):
                dst = dps[cb // 4][:, cb % 4, :]
                for ei, e in enumerate(quad):
                    wdt = wq_tiles[e][2]
                    for j in range(2):
                        nc.tensor.matmul(dst, wdt[:, 2 * j:2 * j + 2,
                                                  cb * P:(cb + 1) * P],
                                         hps[ei][:, 2 * j:2 * j + 2, :],
                                         start=(ei == 0 and j == 0),
                                         stop=(ei == len(quad) - 1 and j == 1),
                                         perf_mode=DR)
            for g in range(2):
                if qi == 0:
                    nc.scalar.copy(accs[g][:, :, hsl], dps[g][:])
                else:
                    nc.vector.scalar_tensor_tensor(accs[g][:, :, hsl],
                                                   dps[g][:], 1.0,
                                                   accs[g][:, :, hsl],
                                                   op0=ALU.mult, op1=ALU.add)

    if DEBUG:
        for g in range(2):
            nc.sync.dma_start(dac_d.ap()[:, g], accs[g][:])
    # ---- out = acc/512 + xa
    fo = pac2.tile([P, NCB, TLOC], F32)
    for cb in range(NCB):
        nc.vector.scalar_tensor_tensor(fo[:, cb, :],
                                       accs[cb // 4][:, cb % 4, :], OSC,
                                       xa[cb][:], op0=ALU.mult, op1=ALU.add)
    nc.sync.dma_start(out_d.ap(), fo[:])

    moe.close()
    whole.close()


# =============================================================== host side
def _build():
    if "nc" in _CACHE:
        return _CACHE["nc"]
    nc = bacc.Bacc("TRN2", target_bir_lowering=False, debug=False,
                   num_devices=8)
    with tile.TileContext(nc) as tc:
        _emit(nc, tc)
    nc.compile()
    _CACHE["nc"] = nc
    return nc


def _rope_tables(pos):
    # pos: (N,) positions; returns cos,ssin of shape (ROPE, N) in the
    # row-pair layout (rows 2i/2i+1 both carry angle pos*freq_i; ssin row 2i
    # is -sin, row 2i+1 is +sin).
    freqs = 1.0 / (THETA ** (np.arange(0, ROPE, 2, dtype=np.float32) / ROPE))
    ang = np.outer(freqs, pos.astype(np.float32))          # (16, N)
    cos = np.repeat(np.cos(ang), 2, axis=0).astype(np.float32)
    sin = np.sin(ang).astype(np.float32)
    ssin = np.empty((ROPE, len(pos)), np.float32)
    ssin[0::2] = -sin
    ssin[1::2] = sin
    return cos, ssin


def _blk(a):
    # (C_like, X) -> (128, nb, X): row cb*128+p -> [p, cb, :]
    nb = a.shape[0] // P
    return np.ascontiguousarray(
        a.reshape(nb, P, -1).transpose(1, 0, 2))


def _f8(a):
    return np.clip(np.asarray(a, np.float32), -240.0, 240.0).astype(
        ml_dtypes.float8_e4m3)


def _host_inputs(inputs, core):
    bf = lambda a: np.ascontiguousarray(a).astype(ml_dtypes.bfloat16)
    f32 = lambda a: np.ascontiguousarray(a, dtype=np.float32)
    b, q = core // 4, core % 4
    hi, lo = 7 - q, q           # slot0 = chunk hi, slot1 = chunk lo
    x = np.asarray(inputs["x"], np.float32)
    w_ln1 = np.asarray(inputs["w_ln1"], np.float32)
    w_ln2 = np.asarray(inputs["w_ln2"], np.float32)
    xT = x[b].T                                            # (C, T)
    loc_cols = np.r_[np.arange(hi * CHUNK, (hi + 1) * CHUNK),
                     np.arange(lo * CHUNK, (lo + 1) * CHUNK)]
    xloc = xT[:, loc_cols]

    # rope tables -> tabs (128, T)
    posq = loc_cols.astype(np.float32)
    cq, sq = _rope_tables(posq)                            # (32, 512)
    posk = np.arange(T, dtype=np.float32)
    ck, sk = _rope_tables(posk)                            # (32, T)

    # permutation matrices (pair swap)
    p32 = np.zeros((ROPE, ROPE), np.float32)
    for i in range(ROPE // 2):
        p32[2 * i + 1, 2 * i] = 1.0
        p32[2 * i, 2 * i + 1] = 1.0
    p64 = np.zeros((2 * ROPE, 2 * ROPE), np.float32)
    p64[:ROPE, :ROPE] = p32
    p64[ROPE:, ROPE:] = p32

    # causal masks: cols 0:2048 slot0 kb 8..15 ; cols 2048:4096 slot1 kb 0..7
    kmask = np.zeros((P, 16 * CHUNK), np.float32)
    ki = np.arange(P)[:, None]
    qi = np.arange(CHUNK)[None, :]
    for half, (j, kbs) in enumerate(((hi, range(8, 16)), (lo, range(0, 8)))):
        for i, kb in enumerate(kbs):
            m = np.zeros((P, CHUNK), np.float32)
            if kb < 2 * j:
                m[:] = 1.0
            elif kb == 2 * j:
                m = (ki <= qi).astype(np.float32)
            elif kb == 2 * j + 1:
                m = (ki + P <= qi).astype(np.float32)
            col = half * 8 * CHUNK + i * CHUNK
            kmask[:, col:col + CHUNK] = m

    wq = np.asarray(inputs["Wq"], np.float32) * w_ln1[:, None]
    wkva = np.asarray(inputs["Wkva"], np.float32) * w_ln1[:, None]
    wo_nope = np.asarray(inputs["Wo"], np.float32).reshape(H, D, C)[:, :NOPE]
    wgate = np.asarray(inputs["Wgate"], np.float32) * w_ln2[:, None]
    biasg = np.broadcast_to(np.asarray(inputs["expert_bias"], np.float32),
                            (P, E)).copy()

    # MoE weights: index 0 = shared expert; scale x64; ln2 folded into g/u
    wg = np.asarray(inputs["Wg"], np.float32) * w_ln2[None, :, None]
    wu = np.asarray(inputs["Wu"], np.float32) * w_ln2[None, :, None]
    wd = np.asarray(inputs["Wd"], np.float32)
    swg = np.asarray(inputs["sWg"], np.float32)[0] * w_ln2[:, None]
    swu = np.asarray(inputs["sWu"], np.float32)[0] * w_ln2[:, None]
    swd = np.asarray(inputs["sWd"], np.float32)[0]
    wg9 = np.concatenate([swg[None], wg], axis=0) * WSC    # (9, C, I)
    wu9 = np.concatenate([swu[None], wu], axis=0) * WSC
    wd9 = np.concatenate([swd[None], wd], axis=0) * WSC    # (9, I, C)
    # (9, C, I) -> (9, 128, NCB*I): [e, p, cb*I + i] = w[e, cb*128+p, i]
    wg8 = wg9.reshape(E + 1, NCB, P, I).transpose(0, 2, 1, 3).reshape(
        E + 1, P, NCB * I)
    wu8 = wu9.reshape(E + 1, NCB, P, I).transpose(0, 2, 1, 3).reshape(
        E + 1, P, NCB * I)
    wd8 = wd9.reshape(E + 1, NIB, P, C).transpose(0, 2, 1, 3).reshape(
        E + 1, P, NIB * C)

    m = {
        "xt": bf(_blk(xT)),
        "xloc": f32(_blk(xloc)),
        "xlbf": bf(_blk(xloc)),
        "wq": bf(_blk(wq)),
        "wkva": bf(_blk(wkva)),
        "wkvb": bf(inputs["Wkvb"]),
        "wo": bf(_blk(wo_nope.reshape(H * NOPE, C))),
        "cosk": f32(ck), "ssink": f32(sk),
        "cosq": f32(np.vstack([cq, cq])), "ssinq": f32(np.vstack([sq, sq])),
        "perm64": bf(p64), "perm32": bf(p32),
        "ident": np.eye(P, dtype=np.float32),
        "kmask": bf(kmask),
        "wgate": f32(_blk(wgate)),
        "biasg": biasg,
        "wg8": _f8(wg8), "wu8": _f8(wu8), "wd8": _f8(wd8),
    }
    return m


LAST_RESULTS = None


def kernel(**inputs):
    global LAST_RESULTS
    nc = _build()
    in_maps = [_host_inputs(inputs, core) for core in range(8)]
    kw = {}
    if os.environ.get("BASSK_TRACE"):
        kw = dict(trace=True, trace_cores=[0], stitch_traces=False)
    res = bass_utils.run_bass_kernel_spmd(nc, in_maps, core_ids=list(range(8)),
                                          **kw)
    LAST_RESULTS = res
    out = np.empty((B, T, C), np.float32)
    for core in range(8):
        b, q = core // 4, core % 4
        hi, lo = 7 - q, q
        oT = res.results[core]["outT"]                 # (128, NCB, TLOC)
        full = oT.transpose(1, 0, 2).reshape(C, TLOC)  # (C, 512)
        out[b, hi * CHUNK:(hi + 1) * CHUNK] = full[:, :CHUNK].T
        out[b, lo * CHUNK:(lo + 1) * CHUNK] = full[:, CHUNK:].T
    return out


# revision 26
# speedup vs baseline: 15824.4338x; 15824.4338x over previous
# DeepSeek block (MLA attention + top-2-of-8 MoE + shared expert) on 8 TRN2
# NeuronCores, zero-collective sharding.
#
# Core c in [0..8): sequence b = c//4, q = c%4; owns token chunks
# hi = 7-q (slot 0) and lo = q (slot 1), 256 tokens each (causally balanced).
# SPMD: identical program on all cores; per-core data (x columns, rope
# tables, causal masks) arrives as inputs.
#
# v2 layout/perf notes:
# - activations feature-on-partition; matmuls bf16 except MoE which runs
#   fp8e4 DoubleRow (both operands packed [128,2,*], contract 256/matmul).
#   MoE weights pre-scaled x64 on host (fp8e4 min normal 2^-6), down input
#   h carries x8; output rescaled by 1/512 at the final accumulate.
# - one DMA per logical matrix (DRAM tensors pre-arranged (128, blk, cols)).
# - wide ACT ops ([128,1024] exp/silu) to amortize the fixed ACT overhead;
#   causal masks applied only to key blocks 8..15 of slot0 and 0..7 of
#   slot1 (interior blocks are mask-free on every core).
# - expert accumulation on the gpsimd (Pool) engine into SBUF, down
#   projections transient in PSUM.
import os
import numpy as np
import ml_dtypes

import concourse.bacc as bacc
import concourse.mybir as mybir
import concourse.tile as tile
from concourse import bass_utils

F32 = mybir.dt.float32
BF16 = mybir.dt.bfloat16
F8 = mybir.dt.float8e4
AF = mybir.ActivationFunctionType
ALU = mybir.AluOpType
DR = mybir.MatmulPerfMode.DoubleRow

B, T, C, H, D = 2, 2048, 1024, 16, 64
R, ROPE, NOPE = 128, 32, 32
E, I = 8, 512
THETA, EPS = 100000.0, 1e-5
P = 128
NCB = C // P             # 8 C blocks
NIB = I // P             # 4 I blocks
TLOC, CHUNK = 512, 256
KB_SLOT = (16, 8)        # key blocks (of 128) attended per chunk slot

WSC = 64.0               # fp8 weight prescale
HSC = 8.0                # fp8 hidden prescale
OSC = 1.0 / (WSC * HSC)  # down-psum rescale

_CACHE = {}


# =============================================================== device IR
def _emit(nc, tc):
    import contextlib

    def din(name, shape, dt):
        return nc.dram_tensor(name, shape, dt, kind="ExternalInput")

    xt_d = din("xt", (P, NCB, T), BF16)
    xloc_d = din("xloc", (P, NCB, TLOC), F32)
    xlbf_d = din("xlbf", (P, NCB, TLOC), BF16)
    wq_d = din("wq", (P, NCB, H * D), BF16)
    wkva_d = din("wkva", (P, NCB, R + ROPE), BF16)
    wkvb_d = din("wkvb", (R, H * NOPE), BF16)
    wo_d = din("wo", (P, 4, C), BF16)
    cosk_d = din("cosk", (ROPE, T), F32)
    ssink_d = din("ssink", (ROPE, T), F32)
    cosq_d = din("cosq", (2 * ROPE, TLOC), F32)
    ssinq_d = din("ssinq", (2 * ROPE, TLOC), F32)
    perm64_d = din("perm64", (2 * ROPE, 2 * ROPE), BF16)
    perm32_d = din("perm32", (ROPE, ROPE), BF16)
    ident_d = din("ident", (P, P), F32)
    kmask_d = din("kmask", (P, 16 * CHUNK), BF16)
    wgate_d = din("wgate", (P, NCB, E), F32)
    biasg_d = din("biasg", (P, E), F32)
    wg_d = din("wg8", (E + 1, P, NCB * I), F8)   # index 0 = shared expert
    wu_d = din("wu8", (E + 1, P, NCB * I), F8)
    wd_d = din("wd8", (E + 1, P, NIB * C), F8)
    out_d = nc.dram_tensor("outT", (P, NCB, TLOC), F32, kind="ExternalOutput")
    DEBUG = bool(int(os.environ.get("BASSK_DEBUG", "0")))
    if DEBUG:
        dxa_d = nc.dram_tensor("d_xa", (P, NCB, TLOC), F32,
                               kind="ExternalOutput")
        dcm_d = nc.dram_tensor("d_comb", (E, TLOC), BF16,
                               kind="ExternalOutput")
        dem_d = nc.dram_tensor("d_em", (P, 16 * CHUNK), BF16,
                               kind="ExternalOutput")
        dac_d = nc.dram_tensor("d_acc", (P, 2, NIB, TLOC), F32,
                               kind="ExternalOutput")

    whole = contextlib.ExitStack()
    early = contextlib.ExitStack()
    attn = contextlib.ExitStack()

    pc = whole.enter_context(tc.tile_pool(name="pc", bufs=1))
    pw = whole.enter_context(tc.tile_pool(name="pw", bufs=1, side="right"))
    pmid = whole.enter_context(tc.tile_pool(name="pmid", bufs=1))

    # pa: tiles written during the early phase but read by attention
    pa = attn.enter_context(tc.tile_pool(name="pa", bufs=1))
    pt2 = attn.enter_context(tc.tile_pool(name="pt2", bufs=2))

    px = early.enter_context(tc.tile_pool(name="px", bufs=1))
    pt1 = early.enter_context(tc.tile_pool(name="pt1", bufs=2))
    pse = early.enter_context(tc.tile_pool(name="pse", bufs=2, space="PSUM"))

    # ---- constants / tables
    ones_128x1 = pc.tile([P, 1], BF16)
    nc.any.memset(ones_128x1[:], 1.0)
    ones1f = pc.tile([1, 1], F32)
    nc.any.memset(ones1f[:], 1.0)
    eps_sb = pc.tile([1, 1], F32)
    nc.any.memset(eps_sb[:], EPS)
    ident_sb = pc.tile([P, P], F32)
    nc.sync.dma_start(ident_sb[:], ident_d.ap())
    perm64_sb = pc.tile([2 * ROPE, 2 * ROPE], BF16)
    nc.sync.dma_start(perm64_sb[:], perm64_d.ap())
    perm32_sb = pc.tile([ROPE, ROPE], BF16)
    nc.sync.dma_start(perm32_sb[:], perm32_d.ap())
    biasg_sb = pc.tile([P, E], F32)
    nc.sync.dma_start(biasg_sb[:], biasg_d.ap())
    wgate_sb = pc.tile([P, NCB, E], F32)
    nc.sync.dma_start(wgate_sb[:], wgate_d.ap())

    cosk_t = px.tile([ROPE, T], F32)
    nc.sync.dma_start(cosk_t[:], cosk_d.ap())
    ssink_t = px.tile([ROPE, T], F32)
    nc.sync.dma_start(ssink_t[:], ssink_d.ap())
    cosq_t = px.tile([2 * ROPE, TLOC], F32)
    nc.sync.dma_start(cosq_t[:], cosq_d.ap())
    ssinq_t = px.tile([2 * ROPE, TLOC], F32)
    nc.sync.dma_start(ssinq_t[:], ssinq_d.ap())

    # ---- bulk loads
    xt = px.tile([P, NCB, T], BF16)
    for cb in range(NCB):
        nc.sync.dma_start(xt[:, cb, :], xt_d.ap()[:, cb, :])
    xlbf = px.tile([P, NCB, TLOC], BF16)
    nc.sync.dma_start(xlbf[:], xlbf_d.ap())
    wkva_sb = px.tile([P, NCB, R + ROPE], BF16)
    nc.sync.dma_start(wkva_sb[:], wkva_d.ap())
    wq_sb = px.tile([P, NCB, H * D], BF16)
    for cb in range(0, NCB, 2):
        nc.sync.dma_start(wq_sb[:, cb:cb + 2, :], wq_d.ap()[:, cb:cb + 2, :])

    # ---- rmsnorm1 stats: global (keys) then local (queries)
    bc1 = px.tile([P, T], F32)
    for nt in range(T // 512):
        sl = slice(nt * 512, (nt + 1) * 512)
        sps = pse.tile([1, 512], F32, name="sps", tag="accA")
        for cb in range(NCB):
            xq = pt1.tile([P, 512], BF16, name="xq", tag="xq")
            nc.scalar.square(xq[:], xt[:, cb, sl])
            nc.tensor.matmul(sps[:], ones_128x1[:], xq[:],
                             start=(cb == 0), stop=(cb == NCB - 1))
        rr = pt1.tile([1, 512], F32, name="rr", tag="rr", bufs=1)
        nc.scalar.activation(rr[:], sps[:], AF.Sqrt, bias=eps_sb[:],
                             scale=1.0 / C)
        iv = pt1.tile([1, 512], F32, name="iv", tag="iv", bufs=1)
        nc.vector.reciprocal(iv[:], rr[:])
        nc.gpsimd.partition_broadcast(bc1[:, sl], iv[:])

    bc1l = px.tile([P, TLOC], F32)
    spsl = pse.tile([1, TLOC], F32, name="spsl", tag="accA")
    for cb in range(NCB):
        xql = pt1.tile([P, TLOC], BF16, name="xql", tag="xq")
        nc.scalar.square(xql[:], xlbf[:, cb, :])
        nc.tensor.matmul(spsl[:], ones_128x1[:], xql[:],
                         start=(cb == 0), stop=(cb == NCB - 1))
    rrl = pt1.tile([1, TLOC], F32, name="rrl", tag="rr", bufs=1)
    nc.scalar.activation(rrl[:], spsl[:], AF.Sqrt, bias=eps_sb[:],
                         scale=1.0 / C)
    ivl = pt1.tile([1, TLOC], F32, name="ivl", tag="iv", bufs=1)
    nc.vector.reciprocal(ivl[:], rrl[:])
    nc.gpsimd.partition_broadcast(bc1l[:], ivl[:])

    # ---- ckv: kv latent (scaled) + scaled k_rope
    kvlat = pa.tile([R, T], BF16)
    krsc = pa.tile([ROPE, T], BF16)     # scaled raw k_rope
    for nt in range(T // 512):
        sl = slice(nt * 512, (nt + 1) * 512)
        lat_ps = pse.tile([P, 512], F32, name="lat_ps", tag="pA")
        for cb in range(NCB):
            nc.tensor.matmul(lat_ps[:], wkva_sb[:, cb, 0:R], xt[:, cb, sl],
                             start=(cb == 0), stop=(cb == NCB - 1))
        rop_ps = pse.tile([ROPE, 512], F32, name="rop_ps", tag="par")
        for cb in range(NCB):
            nc.tensor.matmul(rop_ps[:], wkva_sb[:, cb, R:R + ROPE],
                             xt[:, cb, sl],
                             start=(cb == 0), stop=(cb == NCB - 1))
        nc.vector.tensor_tensor(kvlat[:, sl], lat_ps[:], bc1[:, sl], ALU.mult)
        nc.vector.tensor_tensor(krsc[:, sl], rop_ps[:], bc1[0:ROPE, sl],
                                ALU.mult)

    # ---- rope K -> kropebf [32, T]
    kropebf = pa.tile([ROPE, T], BF16)
    for nt in range(T // 512):
        sl = slice(nt * 512, (nt + 1) * 512)
        park = pse.tile([ROPE, 512], F32, name="park", tag="par")
        nc.tensor.matmul(park[:], perm32_sb[:], krsc[:, sl])
        t1k = pt1.tile([ROPE, 512], F32, name="t1k", tag="t1q")
        nc.gpsimd.tensor_tensor(t1k[:], krsc[:, sl], cosk_t[:, sl], ALU.mult)
        t2k = pt1.tile([ROPE, 512], F32, name="t2k", tag="t2q")
        nc.vector.tensor_tensor(t2k[:], park[:], ssink_t[:, sl], ALU.mult)
        nc.vector.tensor_tensor(kropebf[:, sl], t1k[:], t2k[:], ALU.add)

    # ---- Q projection + rope (whole TLOC per head-pair)
    qbf = []
    for mb in range(8):
        tl = pa.tile([P, TLOC], BF16, name=f"qbf{mb}")
        qps = pse.tile([P, TLOC], F32, name="qps", tag="pA")
        for cb in range(NCB):
            nc.tensor.matmul(qps[:], wq_sb[:, cb, mb * P:(mb + 1) * P],
                             xlbf[:, cb, :],
                             start=(cb == 0), stop=(cb == NCB - 1))
        nc.vector.tensor_tensor(tl[:], qps[:], bc1l[:], ALU.mult)
        qr = pt1.tile([2 * ROPE, TLOC], BF16, name="qr", tag="qr")
        nc.vector.tensor_tensor(qr[0:ROPE, :], qps[32:64, :], bc1l[32:64, :],
                                ALU.mult)
        nc.vector.tensor_tensor(qr[ROPE:2 * ROPE, :], qps[96:128, :],
                                bc1l[96:128, :], ALU.mult)
        parq = pse.tile([2 * ROPE, TLOC], F32, name="parq", tag="par")
        nc.tensor.matmul(parq[:], perm64_sb[:], qr[:])
        t1q = pt1.tile([2 * ROPE, TLOC], F32, name="t1q", tag="t1q")
        nc.gpsimd.tensor_tensor(t1q[:], qr[:], cosq_t[:], ALU.mult)
        t2q = pt1.tile([2 * ROPE, TLOC], F32, name="t2q", tag="t2q")
        nc.vector.tensor_tensor(t2q[:], parq[:], ssinq_t[:], ALU.mult)
        nc.vector.tensor_tensor(tl[32:64, :], t1q[0:ROPE, :], t2q[0:ROPE, :],
                                ALU.add)
        nc.vector.tensor_tensor(tl[96:128, :], t1q[ROPE:2 * ROPE, :],
                                t2q[ROPE:2 * ROPE, :], ALU.add)
        qbf.append(tl)

    early.close()

    # ---- deferred loads (space freed by the early pools)
    pk = attn.enter_context(tc.tile_pool(name="pk", bufs=1))
    xloc = pk.tile([P, NCB, TLOC], F32)
    nc.sync.dma_start(xloc[:], xloc_d.ap())
    wkvb_sb = pk.tile([R, H * NOPE], BF16)
    nc.sync.dma_start(wkvb_sb[:], wkvb_d.ap())
    kmask_sb = pk.tile([P, 16 * CHUNK], BF16)
    nc.sync.dma_start(kmask_sb[:], kmask_d.ap())
    wo_sb = pk.tile([P, 4, C], BF16)
    nc.sync.dma_start(wo_sb[:], wo_d.ap())

    # ---- k_nope -> kfull assembly; V extended with ones row.
    # rope rows depend only on kropebf: DMA them first; nope rows stream in
    # right after each knope block so scores can start early.
    kfull = []
    for mb in range(H // 2):
        kfull.append(pk.tile([P, T], BF16, name=f"kfull{mb}"))
    for mb in range(H // 2):
        nc.sync.dma_start(kfull[mb][32:64, :], kropebf[:])
        nc.sync.dma_start(kfull[mb][96:128, :], kropebf[:])
    sub = contextlib.ExitStack()
    pkx = sub.enter_context(tc.tile_pool(name="pkx", bufs=1))
    psk = sub.enter_context(tc.tile_pool(name="psk", bufs=2, space="PSUM"))
    for j in range(4):
        tl = pkx.tile([P, T], BF16, name=f"knope{j}")
        for nt in range(T // 512):
            sl = slice(nt * 512, (nt + 1) * 512)
            kps = psk.tile([P, 512], F32, name="kps", tag="pA")
            nc.tensor.matmul(kps[:], wkvb_sb[:, j * P:(j + 1) * P],
                             kvlat[:, sl])
            nc.scalar.copy(tl[:, sl], kps[:])
        for h in (4 * j, 4 * j + 1, 4 * j + 2, 4 * j + 3):
            mb, po = h // 2, (h % 2) * 64
            nc.sync.dma_start(kfull[mb][po:po + 32, :],
                              tl[(h % 4) * 32:(h % 4) * 32 + 32, :])
    vext = []
    for tb in range(16):
        tl = pk.tile([P, H, 34], BF16, name=f"vext{tb}")
        vps = psk.tile([P, H * NOPE], F32, name="vps", tag="pA")
        nc.tensor.matmul(vps[:], kvlat[:, tb * P:(tb + 1) * P], wkvb_sb[:])
        nc.scalar.copy(tl[:, :, 0:NOPE],
                       vps[:].rearrange("p (h d) -> p h d", h=H))
        nc.any.memset(tl[:, :, NOPE:NOPE + 1], 1.0)
        vext.append(tl)
    sub.close()

    # ---- attention core
    core = contextlib.ExitStack()
    psc = core.enter_context(tc.tile_pool(name="psc", bufs=2, space="PSUM"))
    pE = core.enter_context(tc.tile_pool(name="pE", bufs=2))
    yall = []
    for yb in range(4):
        yall.append(pk.tile([P, TLOC], BF16, name=f"yall{yb}"))
    for ch in range(2):
        csl = slice(ch * CHUNK, (ch + 1) * CHUNK)
        nkb = KB_SLOT[ch]
        for h in range(H):
            mb, po = h // 2, (h % 2) * 64
            em = pE.tile([P, nkb * CHUNK], BF16, name="em", tag="em")
            for g in range(nkb // 4):
                sp = psc.tile([P, 4 * CHUNK], F32, name="sp", tag="sc")
                for k4 in range(4):
                    kb = 4 * g + k4
                    nc.tensor.matmul(
                        sp[:, k4 * CHUNK:(k4 + 1) * CHUNK],
                        kfull[mb][po:po + 64, kb * P:(kb + 1) * P],
                        qbf[mb][po:po + 64, csl])
                nc.scalar.activation(em[:, g * 4 * CHUNK:(g + 1) * 4 * CHUNK],
                                     sp[:], AF.Exp, scale=0.125)
            if ch == 0:
                nc.vector.tensor_tensor(em[:, 8 * CHUNK:16 * CHUNK],
                                        em[:, 8 * CHUNK:16 * CHUNK],
                                        kmask_sb[:, 0:8 * CHUNK], ALU.mult)
            else:
                nc.vector.tensor_tensor(em[:], em[:],
                                        kmask_sb[:, 8 * CHUNK:16 * CHUNK],
                                        ALU.mult)
            if DEBUG and ch == 0 and h == 0:
                nc.sync.dma_start(dem_d.ap(), em[:])
            y_ps = psc.tile([NOPE + 1, CHUNK], F32, name="y_ps", tag="yv")
            for kb in range(nkb):
                nc.tensor.matmul(y_ps[:], vext[kb][:, h, 0:NOPE + 1],
                                 em[:, kb * CHUNK:(kb + 1) * CHUNK],
                                 start=(kb == 0), stop=(kb == nkb - 1))
            rr2 = pt2.tile([1, CHUNK], F32, name="rr2", tag="rrA")
            nc.vector.reciprocal(rr2[:], y_ps[NOPE:NOPE + 1, :])
            rb = pt2.tile([NOPE, CHUNK], F32, name="rb", tag="rb")
            nc.gpsimd.partition_broadcast(rb[:], rr2[:])
            yt = yall[h // 4]
            ro = (h % 4) * NOPE
            nc.vector.tensor_tensor(yt[ro:ro + NOPE, csl], y_ps[0:NOPE, :],
                                    rb[:], ALU.mult)
    core.close()

    # ---- Wo + residual -> xa (f32) ; rmsnorm2 ; gate ; comb broadcast
    fin = contextlib.ExitStack()
    psg = fin.enter_context(tc.tile_pool(name="psg", bufs=2, space="PSUM"))
    psh = fin.enter_context(tc.tile_pool(name="psh", bufs=1, space="PSUM"))

    xa = []
    for cb in range(NCB):
        xa.append(pmid.tile([P, TLOC], F32, name=f"xa{cb}"))
    for cb in range(NCB):
        ops = psg.tile([P, TLOC], F32, name="ops", tag="wo")
        for kb in range(4):
            nc.tensor.matmul(ops[:], wo_sb[:, kb, cb * P:(cb + 1) * P],
                             yall[kb][:], start=(kb == 0), stop=(kb == 3))
        nc.vector.scalar_tensor_tensor(xa[cb][:], ops[:], 1.0,
                                       xloc[:, cb, :],
                                       op0=ALU.mult, op1=ALU.add)

    invr2 = pmid.tile([1, TLOC], F32)
    sps2 = psh.tile([1, TLOC], F32, name="sps2", tag="acc2")
    for cb in range(NCB):
        xq2 = pt2.tile([P, TLOC], BF16, name="xq2", tag="xq2")
        nc.vector.tensor_tensor(xq2[:], xa[cb][:], xa[cb][:], ALU.mult)
        nc.tensor.matmul(sps2[:], ones_128x1[:], xq2[:],
                         start=(cb == 0), stop=(cb == NCB - 1))
    rr3 = pt2.tile([1, TLOC], F32, name="rr3", tag="rrA")
    nc.scalar.activation(rr3[:], sps2[:], AF.Sqrt, bias=eps_sb[:],
                         scale=1.0 / C)
    nc.vector.reciprocal(invr2[:], rr3[:])
    bc2 = pt2.tile([P, TLOC], F32, name="bc2", tag="bc2")
    nc.gpsimd.partition_broadcast(bc2[:], invr2[:])

    xmf8 = pmid.tile([P, NCB, TLOC], F8)
    for cb in range(NCB):
        nc.vector.tensor_tensor(xmf8[:, cb, :], xa[cb][:], bc2[:], ALU.mult)
    if DEBUG:
        for cb in range(NCB):
            nc.sync.dma_start(dxa_d.ap()[:, cb, :], xa[cb][:])

    # gate (fp32, from xa scaled by invr2 via transposed column)
    ct_all = pmid.tile([E, TLOC], BF16)
    for tb in range(4):
        tsl = slice(tb * P, (tb + 1) * P)
        g_ps = psh.tile([P, E], F32, name="g_ps", tag="gps")
        for cb in range(NCB):
            nc.tensor.matmul(g_ps[:], xa[cb][:, tsl], wgate_sb[:, cb, :],
                             start=(cb == 0), stop=(cb == NCB - 1))
        ir_ps = psh.tile([P, 1], F32, name="ir_ps", tag="irp")
        nc.tensor.transpose(ir_ps[:], invr2[:, tsl], ones1f[:])
        ir_col = pt2.tile([P, 1], F32, name="ir_col", tag="irc")
        nc.scalar.copy(ir_col[:], ir_ps[:])
        lg = pt2.tile([P, E], F32, name="lg", tag="lg")
        nc.vector.scalar_tensor_tensor(lg[:], g_ps[:], ir_col[:], biasg_sb[:],
                                       op0=ALU.mult, op1=ALU.add)
        m1 = pt2.tile([P, 1], F32, name="m1", tag="m1")
        nc.vector.reduce_max(m1[:], lg[:], axis=mybir.AxisListType.X)
        eq1 = pt2.tile([P, E], F32, name="eq1", tag="eq1")
        nc.vector.tensor_scalar(eq1[:], lg[:], m1[:], None, op0=ALU.is_equal)
        lm = pt2.tile([P, E], F32, name="lm", tag="lm")
        nc.vector.scalar_tensor_tensor(lm[:], eq1[:], -1e9, lg[:],
                                       op0=ALU.mult, op1=ALU.add)
        m2 = pt2.tile([P, 1], F32, name="m2", tag="m2")
        nc.vector.reduce_max(m2[:], lm[:], axis=mybir.AxisListType.X)
        eq2 = pt2.tile([P, E], F32, name="eq2", tag="eq2")
        nc.vector.tensor_scalar(eq2[:], lm[:], m2[:], None, op0=ALU.is_equal)
        dm = pt2.tile([P, 1], F32, name="dm", tag="dm")
        nc.vector.tensor_scalar(dm[:], m1[:], m2[:], None, op0=ALU.subtract)
        w1 = pt2.tile([P, 1], F32, name="w1", tag="w1")
        nc.scalar.activation(w1[:], dm[:], AF.Sigmoid)
        w2 = pt2.tile([P, 1], F32, name="w2", tag="w2")
        nc.vector.tensor_scalar(w2[:], w1[:], -1.0, 1.0, op0=ALU.mult,
                                op1=ALU.add)
        cmb = pt2.tile([P, E], F32, name="cmb", tag="cmb")
        nc.vector.tensor_scalar(cmb[:], eq1[:], w1[:], None, op0=ALU.mult)
        cm2 = pt2.tile([P, E], F32, name="cm2", tag="cm2")
        nc.vector.tensor_scalar(cm2[:], eq2[:], w2[:], None, op0=ALU.mult)
        cmf = pt2.tile([P, E], F32, name="cmf", tag="cmf")
        nc.vector.tensor_tensor(cmf[:], cmb[:], cm2[:], ALU.add)
        ct_ps = psh.tile([E, P], F32, name="ct_ps", tag="ctp")
        nc.tensor.transpose(ct_ps[:], cmf[:], ident_sb[:])
        nc.scalar.copy(ct_all[:, tsl], ct_ps[:])
    if DEBUG:
        nc.sync.dma_start(dcm_d.ap(), ct_all[:])
    bcomb = []
    for e in range(E):
        cte = pt2.tile([1, TLOC], BF16, name="cte", tag="cte")
        nc.sync.dma_start(cte[:], ct_all[e:e + 1, :])
        tl = pmid.tile([P, TLOC], BF16, name=f"bcomb{e}")
        nc.gpsimd.partition_broadcast(tl[:], cte[:])
        bcomb.append(tl)

    fin.close()
    attn.close()

    # ---- MoE: fp8 DoubleRow, quad-of-experts accumulation in PSUM.
    # Quads: [shared], [e1..e4], [e5..e8]. Within a quad the token-half loop
    # is outer so the down psum (4 banks) accumulates all its experts; one
    # flush to SBUF per (quad, half, group). Weights pool holds up to 4
    # experts + 1 prefetch (pw bufs=5).
    moe = contextlib.ExitStack()
    pgu = moe.enter_context(tc.tile_pool(name="pgu", bufs=1, space="PSUM"))
    pwd = moe.enter_context(tc.tile_pool(name="pwd", bufs=1, space="PSUM"))
    pmoe = moe.enter_context(tc.tile_pool(name="pmoe", bufs=2))
    pac2 = moe.enter_context(tc.tile_pool(name="pac2", bufs=1))

    accs = [pac2.tile([P, NIB, TLOC], F32, name=f"acc{g}") for g in range(2)]

    quads = [[0], [1, 2], [3, 4], [5, 6], [7, 8]]
    wq_tiles = {}
    for qi, quad in enumerate(quads):
        for e in quad:
            wgt = pw.tile([P, NCB, I], F8, name="wgt", tag=f"wgt{e % 3}")
            nc.sync.dma_start(wgt[:], wg_d.ap()[e])
            wut = pw.tile([P, NCB, I], F8, name="wut", tag=f"wut{e % 3}")
            nc.sync.dma_start(wut[:], wu_d.ap()[e])
            wdt = pw.tile([P, NIB, C], F8, name="wdt", tag=f"wdt{e % 3}")
            nc.sync.dma_start(wdt[:], wd_d.ap()[e])
            wq_tiles[e] = (wgt, wut, wdt)
        for th in range(2):
            hsl = slice(th * CHUNK, (th + 1) * CHUNK)
            dps = [pwd.tile([P, NIB, CHUNK], F32, name=f"dps{g}", tag=f"d{g}")
                   for g in range(2)]
            hps = []
            for ei, e in enumerate(quad):
                wgt, wut, wdt = wq_tiles[e]
                gp = pgu.tile([P, NIB, CHUNK], F32, name="gp", tag="gp")
                for ib in range(NIB):
                    isl = slice(ib * P, (ib + 1) * P)
                    for j in range(4):
                        nc.tensor.matmul(gp[:, ib, :],
                                         wgt[:, 2 * j:2 * j + 2, isl],
                                         xmf8[:, 2 * j:2 * j + 2, hsl],
                                         start=(j == 0), stop=(j == 3),
                                         perf_mode=DR)
                sg = pmoe.tile([P, NIB, CHUNK], BF16, name="sg", tag="sg")
                nc.scalar.activation(sg[:], gp[:], AF.Silu, scale=1.0 / WSC)
                if e > 0:
                    sgc = pmoe.tile([P, NIB, CHUNK], BF16, name="sgc",
                                    tag="sgc")
                    bce = bcomb[e - 1]
                    for ib in range(NIB):
                        nc.vector.tensor_tensor(sgc[:, ib, :], sg[:, ib, :],
                                                bce[:, hsl], ALU.mult)
                else:
                    sgc = sg
                up = pgu.tile([P, NIB, CHUNK], F32, name="up", tag="up")
                for ib in range(NIB):
                    isl = slice(ib * P, (ib + 1) * P)
                    for j in range(4):
                        nc.tensor.matmul(up[:, ib, :],
                                         wut[:, 2 * j:2 * j + 2, isl],
                                         xmf8[:, 2 * j:2 * j + 2, hsl],
                                         start=(j == 0), stop=(j == 3),
                                         perf_mode=DR)
                hp = pmoe.tile([P, NIB, CHUNK], F8, name="hp", tag="hp",
                               bufs=3)
                nc.vector.scalar_tensor_tensor(hp[:], up[:], 1.0 / HSC,
                                               sgc[:],
                                               op0=ALU.mult, op1=ALU.mult)
                hps.append(hp)
            # region-major down: a PSUM region's accumulation group must
            # finish before the next group in the same bank starts
            # (start_tensor_calc clears has_written bank-wide).
            for cb in range(NCB):
                dst = dps[cb // 4][:, cb % 4, :]
                for ei, e in enumerate(quad):
                    wdt = wq_tiles[e][2]
                    for j in range(2):
                        nc.tensor.matmul(dst, wdt[:, 2 * j:2 * j + 2,
                                                  cb * P:(cb + 1) * P],
                                         hps[ei][:, 2 * j:2 * j + 2, :],
                                         start=(ei == 0 and j == 0),
                                         stop=(ei == len(quad) - 1 and j == 1),
                                         perf_mode=DR)
            for g in range(2):
                if qi == 0:
                    nc.scalar.copy(accs[g][:, :, hsl], dps[g][:])
                else:
                    nc.vector.scalar_tensor_tensor(accs[g][:, :, hsl],
                                                   dps[g][:], 1.0,
                                                   accs[g][:, :, hsl],
                                                   op0=ALU.mult, op1=ALU.add)

    if DEBUG:
        for g in range(2):
            nc.sync.dma_start(dac_d.ap()[:, g], accs[g][:])
    # ---- out = acc/512 + xa
    fo = pac2.tile([P, NCB, TLOC], F32)
    for cb in range(NCB):
        nc.vector.scalar_tensor_tensor(fo[:, cb, :],
                                       accs[cb // 4][:, cb % 4, :], OSC,
                                       xa[cb][:], op0=ALU.mult, op1=ALU.add)
    nc.sync.dma_start(out_d.ap(), fo[:])

    moe.close()
    whole.close()


# =============================================================== host side
def _build():
    if "nc" in _CACHE:
        return _CACHE["nc"]
    nc = bacc.Bacc("TRN2", target_bir_lowering=False, debug=False,
                   num_devices=8)
    with tile.TileContext(nc) as tc:
        _emit(nc, tc)
    nc.compile()
    _CACHE["nc"] = nc
    return nc


def _rope_tables(pos):
    # pos: (N,) positions; returns cos,ssin of shape (ROPE, N) in the
    # row-pair layout (rows 2i/2i+1 both carry angle pos*freq_i; ssin row 2i
    # is -sin, row 2i+1 is +sin).
    freqs = 1.0 / (THETA ** (np.arange(0, ROPE, 2, dtype=np.float32) / ROPE))
    ang = np.outer(freqs, pos.astype(np.float32))          # (16, N)
    cos = np.repeat(np.cos(ang), 2, axis=0).astype(np.float32)
    sin = np.sin(ang).astype(np.float32)
    ssin = np.empty((ROPE, len(pos)), np.float32)
    ssin[0::2] = -sin
    ssin[1::2] = sin
    return cos, ssin


def _blk(a):
    # (C_like, X) -> (128, nb, X): row cb*128+p -> [p, cb, :]
    nb = a.shape[0] // P
    return np.ascontiguousarray(
        a.reshape(nb, P, -1).transpose(1, 0, 2))


def _f8(a):
    return np.clip(np.asarray(a, np.float32), -240.0, 240.0).astype(
        ml_dtypes.float8_e4m3)


def _host_inputs(inputs, core):
    bf = lambda a: np.ascontiguousarray(a).astype(ml_dtypes.bfloat16)
    f32 = lambda a: np.ascontiguousarray(a, dtype=np.float32)
    b, q = core // 4, core % 4
    hi, lo = 7 - q, q           # slot0 = chunk hi, slot1 = chunk lo
    x = np.asarray(inputs["x"], np.float32)
    w_ln1 = np.asarray(inputs["w_ln1"], np.float32)
    w_ln2 = np.asarray(inputs["w_ln2"], np.float32)
    xT = x[b].T                                            # (C, T)
    loc_cols = np.r_[np.arange(hi * CHUNK, (hi + 1) * CHUNK),
                     np.arange(lo * CHUNK, (lo + 1) * CHUNK)]
    xloc = xT[:, loc_cols]

    # rope tables -> tabs (128, T)
    posq = loc_cols.astype(np.float32)
    cq, sq = _rope_tables(posq)                            # (32, 512)
    posk = np.arange(T, dtype=np.float32)
    ck, sk = _rope_tables(posk)                            # (32, T)

    # permutation matrices (pair swap)
    p32 = np.zeros((ROPE, ROPE), np.float32)
    for i in range(ROPE // 2):
        p32[2 * i + 1, 2 * i] = 1.0
        p32[2 * i, 2 * i + 1] = 1.0
    p64 = np.zeros((2 * ROPE, 2 * ROPE), np.float32)
    p64[:ROPE, :ROPE] = p32
    p64[ROPE:, ROPE:] = p32

    # causal masks: cols 0:2048 slot0 kb 8..15 ; cols 2048:4096 slot1 kb 0..7
    kmask = np.zeros((P, 16 * CHUNK), np.float32)
    ki = np.arange(P)[:, None]
    qi = np.arange(CHUNK)[None, :]
    for half, (j, kbs) in enumerate(((hi, range(8, 16)), (lo, range(0, 8)))):
        for i, kb in enumerate(kbs):
            m = np.zeros((P, CHUNK), np.float32)
            if kb < 2 * j:
                m[:] = 1.0
            elif kb == 2 * j:
                m = (ki <= qi).astype(np.float32)
            elif kb == 2 * j + 1:
                m = (ki + P <= qi).astype(np.float32)
            col = half * 8 * CHUNK + i * CHUNK
            kmask[:, col:col + CHUNK] = m

    wq = np.asarray(inputs["Wq"], np.float32) * w_ln1[:, None]
    wkva = np.asarray(inputs["Wkva"], np.float32) * w_ln1[:, None]
    wo_nope = np.asarray(inputs["Wo"], np.float32).reshape(H, D, C)[:, :NOPE]
    wgate = np.asarray(inputs["Wgate"], np.float32) * w_ln2[:, None]
    biasg = np.broadcast_to(np.asarray(inputs["expert_bias"], np.float32),
                            (P, E)).copy()

    # MoE weights: index 0 = shared expert; scale x64; ln2 folded into g/u
    wg = np.asarray(inputs["Wg"], np.float32) * w_ln2[None, :, None]
    wu = np.asarray(inputs["Wu"], np.float32) * w_ln2[None, :, None]
    wd = np.asarray(inputs["Wd"], np.float32)
    swg = np.asarray(inputs["sWg"], np.float32)[0] * w_ln2[:, None]
    swu = np.asarray(inputs["sWu"], np.float32)[0] * w_ln2[:, None]
    swd = np.asarray(inputs["sWd"], np.float32)[0]
    wg9 = np.concatenate([swg[None], wg], axis=0) * WSC    # (9, C, I)
    wu9 = np.concatenate([swu[None], wu], axis=0) * WSC
    wd9 = np.concatenate([swd[None], wd], axis=0) * WSC    # (9, I, C)
    # (9, C, I) -> (9, 128, NCB*I): [e, p, cb*I + i] = w[e, cb*128+p, i]
    wg8 = wg9.reshape(E + 1, NCB, P, I).transpose(0, 2, 1, 3).reshape(
        E + 1, P, NCB * I)
    wu8 = wu9.reshape(E + 1, NCB, P, I).transpose(0, 2, 1, 3).reshape(
        E + 1, P, NCB * I)
    wd8 = wd9.reshape(E + 1, NIB, P, C).transpose(0, 2, 1, 3).reshape(
        E + 1, P, NIB * C)

    m = {
        "xt": bf(_blk(xT)),
        "xloc": f32(_blk(xloc)),
        "xlbf": bf(_blk(xloc)),
        "wq": bf(_blk(wq)),
        "wkva": bf(_blk(wkva)),
        "wkvb": bf(inputs["Wkvb"]),
        "wo": bf(_blk(wo_nope.reshape(H * NOPE, C))),
        "cosk": f32(ck), "ssink": f32(sk),
        "cosq": f32(np.vstack([cq, cq])), "ssinq": f32(np.vstack([sq, sq])),
        "perm64": bf(p64), "perm32": bf(p32),
        "ident": np.eye(P, dtype=np.float32),
        "kmask": bf(kmask),
        "wgate": f32(_blk(wgate)),
        "biasg": biasg,
        "wg8": _f8(wg8), "wu8": _f8(wu8), "wd8": _f8(wd8),
    }
    return m


LAST_RESULTS = None


def kernel(**inputs):
    global LAST_RESULTS
    nc = _build()
    in_maps = [_host_inputs(inputs, core) for core in range(8)]
    kw = {}
    if os.environ.get("BASSK_TRACE"):
        kw = dict(trace=True, trace_cores=[0], stitch_traces=False)
    res = bass_utils.run_bass_kernel_spmd(nc, in_maps, core_ids=list(range(8)),
                                          **kw)
    LAST_RESULTS = res
    out = np.empty((B, T, C), np.float32)
    for core in range(8):
        b, q = core // 4, core % 4
        hi, lo = 7 - q, q
        oT = res.results[core]["outT"]                 # (128, NCB, TLOC)
        full = oT.transpose(1, 0, 2).reshape(C, TLOC)  # (C, 512)
        out[b, hi * CHUNK:(hi + 1) * CHUNK] = full[:, :CHUNK].T
        out[b, lo * CHUNK:(lo + 1) * CHUNK] = full[:, CHUNK:].T
    return out


# revision 31
# speedup vs baseline: 16083.1724x; 1.0164x over previous
# DeepSeek block (MLA attention + top-2-of-8 MoE + shared expert) on 8 TRN2
# NeuronCores, zero-collective sharding.
#
# Core c in [0..8): sequence b = c//4, q = c%4; owns token chunks
# hi = 7-q (slot 0) and lo = q (slot 1), 256 tokens each (causally balanced).
# SPMD: identical program on all cores; per-core data (x columns, rope
# tables, causal masks) arrives as inputs.
#
# v2 layout/perf notes:
# - activations feature-on-partition; matmuls bf16 except MoE which runs
#   fp8e4 DoubleRow (both operands packed [128,2,*], contract 256/matmul).
#   MoE weights pre-scaled x64 on host (fp8e4 min normal 2^-6), down input
#   h carries x8; output rescaled by 1/512 at the final accumulate.
# - one DMA per logical matrix (DRAM tensors pre-arranged (128, blk, cols)).
# - wide ACT ops ([128,1024] exp/silu) to amortize the fixed ACT overhead;
#   causal masks applied only to key blocks 8..15 of slot0 and 0..7 of
#   slot1 (interior blocks are mask-free on every core).
# - expert accumulation on the gpsimd (Pool) engine into SBUF, down
#   projections transient in PSUM.
import os
import numpy as np
import ml_dtypes

import concourse.bacc as bacc
import concourse.mybir as mybir
import concourse.tile as tile
from concourse import bass_utils

F32 = mybir.dt.float32
BF16 = mybir.dt.bfloat16
F8 = mybir.dt.float8e4
AF = mybir.ActivationFunctionType
ALU = mybir.AluOpType
DR = mybir.MatmulPerfMode.DoubleRow

B, T, C, H, D = 2, 2048, 1024, 16, 64
R, ROPE, NOPE = 128, 32, 32
E, I = 8, 512
THETA, EPS = 100000.0, 1e-5
P = 128
NCB = C // P             # 8 C blocks
NIB = I // P             # 4 I blocks
TLOC, CHUNK = 512, 256
KB_SLOT = (16, 8)        # key blocks (of 128) attended per chunk slot

WSC = 64.0               # fp8 weight prescale
HSC = 8.0                # fp8 hidden prescale
OSC = 1.0 / (WSC * HSC)  # down-psum rescale

_CACHE = {}


# =============================================================== device IR
def _emit(nc, tc):
    import contextlib

    def din(name, shape, dt):
        return nc.dram_tensor(name, shape, dt, kind="ExternalInput")

    xt_d = din("xt", (P, NCB, T), BF16)
    xloc_d = din("xloc", (P, NCB, TLOC), F32)
    xlbf_d = din("xlbf", (P, NCB, TLOC), BF16)
    wq_d = din("wq", (P, NCB, H * D), BF16)
    wkva_d = din("wkva", (P, NCB, R + ROPE), BF16)
    wkvb_d = din("wkvb", (R, H * NOPE), BF16)
    wo_d = din("wo", (P, 4, C), BF16)
    cosk_d = din("cosk", (ROPE, T), F32)
    ssink_d = din("ssink", (ROPE, T), F32)
    cosq_d = din("cosq", (2 * ROPE, TLOC), F32)
    ssinq_d = din("ssinq", (2 * ROPE, TLOC), F32)
    perm64_d = din("perm64", (2 * ROPE, 2 * ROPE), BF16)
    perm32_d = din("perm32", (ROPE, ROPE), BF16)
    ident_d = din("ident", (P, P), F32)
    kmask_d = din("kmask", (P, 16 * CHUNK), BF16)
    wgate_d = din("wgate", (P, NCB, E), F32)
    biasg_d = din("biasg", (P, E), F32)
    wg_d = din("wg8", (E + 1, P, NCB * I), F8)   # index 0 = shared expert
    wu_d = din("wu8", (E + 1, P, NCB * I), F8)
    wd_d = din("wd8", (E + 1, P, NIB * C), F8)
    out_d = nc.dram_tensor("outT", (P, NCB, TLOC), F32, kind="ExternalOutput")
    DEBUG = bool(int(os.environ.get("BASSK_DEBUG", "0")))
    if DEBUG:
        dxa_d = nc.dram_tensor("d_xa", (P, NCB, TLOC), F32,
                               kind="ExternalOutput")
        dcm_d = nc.dram_tensor("d_comb", (E, TLOC), BF16,
                               kind="ExternalOutput")
        dem_d = nc.dram_tensor("d_em", (P, 16 * CHUNK), BF16,
                               kind="ExternalOutput")
        dac_d = nc.dram_tensor("d_acc", (P, 2, NIB, TLOC), F32,
                               kind="ExternalOutput")

    whole = contextlib.ExitStack()
    early = contextlib.ExitStack()
    attn = contextlib.ExitStack()

    pc = whole.enter_context(tc.tile_pool(name="pc", bufs=1))
    pw = whole.enter_context(tc.tile_pool(name="pw", bufs=1, side="right"))
    pmid = whole.enter_context(tc.tile_pool(name="pmid", bufs=1))

    # pa: tiles written during the early phase but read by attention
    pa = attn.enter_context(tc.tile_pool(name="pa", bufs=1))
    pt2 = attn.enter_context(tc.tile_pool(name="pt2", bufs=2))

    px = early.enter_context(tc.tile_pool(name="px", bufs=1))
    pt1 = early.enter_context(tc.tile_pool(name="pt1", bufs=2))
    pse = early.enter_context(tc.tile_pool(name="pse", bufs=2, space="PSUM"))

    # ---- constants / tables
    ones_128x1 = pc.tile([P, 1], BF16)
    nc.any.memset(ones_128x1[:], 1.0)
    ones1f = pc.tile([1, 1], F32)
    nc.any.memset(ones1f[:], 1.0)
    eps_sb = pc.tile([1, 1], F32)
    nc.any.memset(eps_sb[:], EPS)
    ident_sb = pc.tile([P, P], F32)
    nc.sync.dma_start(ident_sb[:], ident_d.ap())
    perm64_sb = pc.tile([2 * ROPE, 2 * ROPE], BF16)
    nc.sync.dma_start(perm64_sb[:], perm64_d.ap())
    perm32_sb = pc.tile([ROPE, ROPE], BF16)
    nc.sync.dma_start(perm32_sb[:], perm32_d.ap())
    biasg_sb = pc.tile([P, E], F32)
    nc.sync.dma_start(biasg_sb[:], biasg_d.ap())
    wgate_sb = pc.tile([P, NCB, E], F32)
    nc.sync.dma_start(wgate_sb[:], wgate_d.ap())

    cosk_t = px.tile([ROPE, T], F32)
    nc.sync.dma_start(cosk_t[:], cosk_d.ap())
    ssink_t = px.tile([ROPE, T], F32)
    nc.sync.dma_start(ssink_t[:], ssink_d.ap())
    cosq_t = px.tile([2 * ROPE, TLOC], F32)
    nc.sync.dma_start(cosq_t[:], cosq_d.ap())
    ssinq_t = px.tile([2 * ROPE, TLOC], F32)
    nc.sync.dma_start(ssinq_t[:], ssinq_d.ap())

    # ---- bulk loads
    xt = px.tile([P, NCB, T], BF16)
    for cb in range(NCB):
        nc.sync.dma_start(xt[:, cb, :], xt_d.ap()[:, cb, :])
    xlbf = px.tile([P, NCB, TLOC], BF16)
    nc.sync.dma_start(xlbf[:], xlbf_d.ap())
    wkva_sb = px.tile([P, NCB, R + ROPE], BF16)
    nc.sync.dma_start(wkva_sb[:], wkva_d.ap())
    wq_sb = px.tile([P, NCB, H * D], BF16)
    for cb in range(0, NCB, 2):
        nc.sync.dma_start(wq_sb[:, cb:cb + 2, :], wq_d.ap()[:, cb:cb + 2, :])

    # ---- prefetch first MoE expert weights (slots 0..2) before the
    # attention-section DMAs claim the SP queue
    wq_tiles = {}
    for e in (0, 1, 2):
        wgt = pw.tile([P, NCB, I], F8, name="wgt", tag=f"wgt{e % 3}")
        nc.sync.dma_start(wgt[:], wg_d.ap()[e])
        wut = pw.tile([P, NCB, I], F8, name="wut", tag=f"wut{e % 3}")
        nc.sync.dma_start(wut[:], wu_d.ap()[e])
        wdt = pw.tile([P, NIB, C], F8, name="wdt", tag=f"wdt{e % 3}")
        nc.sync.dma_start(wdt[:], wd_d.ap()[e])
        wq_tiles[e] = (wgt, wut, wdt)

    # ---- rmsnorm1 stats: global (keys) then local (queries)
    bc1 = px.tile([P, T], F32)
    for nt in range(T // 512):
        sl = slice(nt * 512, (nt + 1) * 512)
        sps = pse.tile([1, 512], F32, name="sps", tag="accA")
        for cb in range(NCB):
            xq = pt1.tile([P, 512], BF16, name="xq", tag="xq")
            nc.scalar.square(xq[:], xt[:, cb, sl])
            nc.tensor.matmul(sps[:], ones_128x1[:], xq[:],
                             start=(cb == 0), stop=(cb == NCB - 1))
        rr = pt1.tile([1, 512], F32, name="rr", tag="rr", bufs=1)
        nc.scalar.activation(rr[:], sps[:], AF.Sqrt, bias=eps_sb[:],
                             scale=1.0 / C)
        iv = pt1.tile([1, 512], F32, name="iv", tag="iv", bufs=1)
        nc.vector.reciprocal(iv[:], rr[:])
        nc.gpsimd.partition_broadcast(bc1[:, sl], iv[:])

    bc1l = px.tile([P, TLOC], F32)
    spsl = pse.tile([1, TLOC], F32, name="spsl", tag="accA")
    for cb in range(NCB):
        xql = pt1.tile([P, TLOC], BF16, name="xql", tag="xq")
        nc.scalar.square(xql[:], xlbf[:, cb, :])
        nc.tensor.matmul(spsl[:], ones_128x1[:], xql[:],
                         start=(cb == 0), stop=(cb == NCB - 1))
    rrl = pt1.tile([1, TLOC], F32, name="rrl", tag="rr", bufs=1)
    nc.scalar.activation(rrl[:], spsl[:], AF.Sqrt, bias=eps_sb[:],
                         scale=1.0 / C)
    ivl = pt1.tile([1, TLOC], F32, name="ivl", tag="iv", bufs=1)
    nc.vector.reciprocal(ivl[:], rrl[:])
    nc.gpsimd.partition_broadcast(bc1l[:], ivl[:])

    # ---- ckv: kv latent (scaled) + scaled k_rope
    kvlat = pa.tile([R, T], BF16)
    krsc = pa.tile([ROPE, T], BF16)     # scaled raw k_rope
    for nt in range(T // 512):
        sl = slice(nt * 512, (nt + 1) * 512)
        lat_ps = pse.tile([P, 512], F32, name="lat_ps", tag="pA")
        for cb in range(NCB):
            nc.tensor.matmul(lat_ps[:], wkva_sb[:, cb, 0:R], xt[:, cb, sl],
                             start=(cb == 0), stop=(cb == NCB - 1))
        rop_ps = pse.tile([ROPE, 512], F32, name="rop_ps", tag="par")
        for cb in range(NCB):
            nc.tensor.matmul(rop_ps[:], wkva_sb[:, cb, R:R + ROPE],
                             xt[:, cb, sl],
                             start=(cb == 0), stop=(cb == NCB - 1))
        nc.vector.tensor_tensor(kvlat[:, sl], lat_ps[:], bc1[:, sl], ALU.mult)
        nc.vector.tensor_tensor(krsc[:, sl], rop_ps[:], bc1[0:ROPE, sl],
                                ALU.mult)

    # ---- rope K -> kropebf [32, T]
    kropebf = pa.tile([ROPE, T], BF16)
    for nt in range(T // 512):
        sl = slice(nt * 512, (nt + 1) * 512)
        park = pse.tile([ROPE, 512], F32, name="park", tag="par")
        nc.tensor.matmul(park[:], perm32_sb[:], krsc[:, sl])
        t1k = pt1.tile([ROPE, 512], F32, name="t1k", tag="t1q")
        nc.gpsimd.tensor_tensor(t1k[:], krsc[:, sl], cosk_t[:, sl], ALU.mult)
        t2k = pt1.tile([ROPE, 512], F32, name="t2k", tag="t2q")
        nc.vector.tensor_tensor(t2k[:], park[:], ssink_t[:, sl], ALU.mult)
        nc.vector.tensor_tensor(kropebf[:, sl], t1k[:], t2k[:], ALU.add)

    # ---- Q projection + rope (whole TLOC per head-pair)
    qbf = []
    for mb in range(8):
        tl = pa.tile([P, TLOC], BF16, name=f"qbf{mb}")
        qps = pse.tile([P, TLOC], F32, name="qps", tag="pA")
        for cb in range(NCB):
            nc.tensor.matmul(qps[:], wq_sb[:, cb, mb * P:(mb + 1) * P],
                             xlbf[:, cb, :],
                             start=(cb == 0), stop=(cb == NCB - 1))
        nc.vector.tensor_tensor(tl[:], qps[:], bc1l[:], ALU.mult)
        qr = pt1.tile([2 * ROPE, TLOC], BF16, name="qr", tag="qr")
        nc.vector.tensor_tensor(qr[0:ROPE, :], qps[32:64, :], bc1l[32:64, :],
                                ALU.mult)
        nc.vector.tensor_tensor(qr[ROPE:2 * ROPE, :], qps[96:128, :],
                                bc1l[96:128, :], ALU.mult)
        parq = pse.tile([2 * ROPE, TLOC], F32, name="parq", tag="par")
        nc.tensor.matmul(parq[:], perm64_sb[:], qr[:])
        t1q = pt1.tile([2 * ROPE, TLOC], F32, name="t1q", tag="t1q")
        nc.gpsimd.tensor_tensor(t1q[:], qr[:], cosq_t[:], ALU.mult)
        t2q = pt1.tile([2 * ROPE, TLOC], F32, name="t2q", tag="t2q")
        nc.vector.tensor_tensor(t2q[:], parq[:], ssinq_t[:], ALU.mult)
        nc.vector.tensor_tensor(tl[32:64, :], t1q[0:ROPE, :], t2q[0:ROPE, :],
                                ALU.add)
        nc.vector.tensor_tensor(tl[96:128, :], t1q[ROPE:2 * ROPE, :],
                                t2q[ROPE:2 * ROPE, :], ALU.add)
        qbf.append(tl)

    early.close()

    # ---- deferred loads (space freed by the early pools)
    pk = attn.enter_context(tc.tile_pool(name="pk", bufs=1))
    xloc = pk.tile([P, NCB, TLOC], F32)
    nc.sync.dma_start(xloc[:], xloc_d.ap())
    wkvb_sb = pk.tile([R, H * NOPE], BF16)
    nc.sync.dma_start(wkvb_sb[:], wkvb_d.ap())
    kmask_sb = pk.tile([P, 16 * CHUNK], BF16)
    nc.sync.dma_start(kmask_sb[:], kmask_d.ap())
    wo_sb = pk.tile([P, 4, C], BF16)
    nc.sync.dma_start(wo_sb[:], wo_d.ap())

    # ---- k_nope -> kfull assembly; V extended with ones row.
    # rope rows depend only on kropebf: DMA them first; nope rows stream in
    # right after each knope block so scores can start early.
    kfull = []
    for mb in range(H // 2):
        kfull.append(pk.tile([P, T], BF16, name=f"kfull{mb}"))
    for mb in range(H // 2):
        nc.sync.dma_start(kfull[mb][32:64, :], kropebf[:])
        nc.sync.dma_start(kfull[mb][96:128, :], kropebf[:])
    sub = contextlib.ExitStack()
    pkx = sub.enter_context(tc.tile_pool(name="pkx", bufs=1))
    psk = sub.enter_context(tc.tile_pool(name="psk", bufs=2, space="PSUM"))
    for j in range(4):
        tl = pkx.tile([P, T], BF16, name=f"knope{j}")
        for nt in range(T // 512):
            sl = slice(nt * 512, (nt + 1) * 512)
            kps = psk.tile([P, 512], F32, name="kps", tag="pA")
            nc.tensor.matmul(kps[:], wkvb_sb[:, j * P:(j + 1) * P],
                             kvlat[:, sl])
            nc.scalar.copy(tl[:, sl], kps[:])
        for h in (4 * j, 4 * j + 1, 4 * j + 2, 4 * j + 3):
            mb, po = h // 2, (h % 2) * 64
            nc.sync.dma_start(kfull[mb][po:po + 32, :],
                              tl[(h % 4) * 32:(h % 4) * 32 + 32, :])
    vext = []
    for tb in range(16):
        tl = pk.tile([P, H, 34], BF16, name=f"vext{tb}")
        vps = psk.tile([P, H * NOPE], F32, name="vps", tag="pA")
        nc.tensor.matmul(vps[:], kvlat[:, tb * P:(tb + 1) * P], wkvb_sb[:])
        nc.scalar.copy(tl[:, :, 0:NOPE],
                       vps[:].rearrange("p (h d) -> p h d", h=H))
        nc.any.memset(tl[:, :, NOPE:NOPE + 1], 1.0)
        vext.append(tl)
    sub.close()

    # ---- attention core
    core = contextlib.ExitStack()
    psc = core.enter_context(tc.tile_pool(name="psc", bufs=2, space="PSUM"))
    pE = core.enter_context(tc.tile_pool(name="pE", bufs=2))
    yall = []
    for yb in range(4):
        yall.append(pk.tile([P, TLOC], BF16, name=f"yall{yb}"))
    for ch in range(2):
        csl = slice(ch * CHUNK, (ch + 1) * CHUNK)
        nkb = KB_SLOT[ch]
        for h in range(H):
            mb, po = h // 2, (h % 2) * 64
            em = pE.tile([P, nkb * CHUNK], BF16, name="em", tag="em")
            for g in range(nkb // 4):
                sp = psc.tile([P, 4 * CHUNK], F32, name="sp", tag="sc")
                for k4 in range(4):
                    kb = 4 * g + k4
                    nc.tensor.matmul(
                        sp[:, k4 * CHUNK:(k4 + 1) * CHUNK],
                        kfull[mb][po:po + 64, kb * P:(kb + 1) * P],
                        qbf[mb][po:po + 64, csl])
                nc.scalar.activation(em[:, g * 4 * CHUNK:(g + 1) * 4 * CHUNK],
                                     sp[:], AF.Exp, scale=0.125)
            if ch == 0:
                nc.vector.tensor_tensor(em[:, 8 * CHUNK:16 * CHUNK],
                                        em[:, 8 * CHUNK:16 * CHUNK],
                                        kmask_sb[:, 0:8 * CHUNK], ALU.mult)
            else:
                nc.vector.tensor_tensor(em[:], em[:],
                                        kmask_sb[:, 8 * CHUNK:16 * CHUNK],
                                        ALU.mult)
            if DEBUG and ch == 0 and h == 0:
                nc.sync.dma_start(dem_d.ap(), em[:])
            y_ps = psc.tile([NOPE + 1, CHUNK], F32, name="y_ps", tag="yv")
            for kb in range(nkb):
                nc.tensor.matmul(y_ps[:], vext[kb][:, h, 0:NOPE + 1],
                                 em[:, kb * CHUNK:(kb + 1) * CHUNK],
                                 start=(kb == 0), stop=(kb == nkb - 1))
            rr2 = pt2.tile([1, CHUNK], F32, name="rr2", tag="rrA")
            nc.vector.reciprocal(rr2[:], y_ps[NOPE:NOPE + 1, :])
            rb = pt2.tile([NOPE, CHUNK], F32, name="rb", tag="rb")
            nc.gpsimd.partition_broadcast(rb[:], rr2[:])
            yt = yall[h // 4]
            ro = (h % 4) * NOPE
            nc.vector.tensor_tensor(yt[ro:ro + NOPE, csl], y_ps[0:NOPE, :],
                                    rb[:], ALU.mult)
    core.close()

    # ---- Wo + residual -> xa (f32) ; rmsnorm2 ; gate ; comb broadcast
    fin = contextlib.ExitStack()
    psg = fin.enter_context(tc.tile_pool(name="psg", bufs=2, space="PSUM"))
    psh = fin.enter_context(tc.tile_pool(name="psh", bufs=1, space="PSUM"))

    xa = []
    for cb in range(NCB):
        xa.append(pmid.tile([P, TLOC], F32, name=f"xa{cb}"))
    for cb in range(NCB):
        ops = psg.tile([P, TLOC], F32, name="ops", tag="wo")
        for kb in range(4):
            nc.tensor.matmul(ops[:], wo_sb[:, kb, cb * P:(cb + 1) * P],
                             yall[kb][:], start=(kb == 0), stop=(kb == 3))
        nc.vector.scalar_tensor_tensor(xa[cb][:], ops[:], 1.0,
                                       xloc[:, cb, :],
                                       op0=ALU.mult, op1=ALU.add)

    invr2 = pmid.tile([1, TLOC], F32)
    sps2 = psh.tile([1, TLOC], F32, name="sps2", tag="acc2")
    for cb in range(NCB):
        xq2 = pt2.tile([P, TLOC], BF16, name="xq2", tag="xq2")
        nc.vector.tensor_tensor(xq2[:], xa[cb][:], xa[cb][:], ALU.mult)
        nc.tensor.matmul(sps2[:], ones_128x1[:], xq2[:],
                         start=(cb == 0), stop=(cb == NCB - 1))
    rr3 = pt2.tile([1, TLOC], F32, name="rr3", tag="rrA")
    nc.scalar.activation(rr3[:], sps2[:], AF.Sqrt, bias=eps_sb[:],
                         scale=1.0 / C)
    nc.vector.reciprocal(invr2[:], rr3[:])
    bc2 = pt2.tile([P, TLOC], F32, name="bc2", tag="bc2")
    nc.gpsimd.partition_broadcast(bc2[:], invr2[:])

    xmf8 = pmid.tile([P, NCB, TLOC], F8)
    for cb in range(NCB):
        nc.vector.tensor_tensor(xmf8[:, cb, :], xa[cb][:], bc2[:], ALU.mult)
    if DEBUG:
        for cb in range(NCB):
            nc.sync.dma_start(dxa_d.ap()[:, cb, :], xa[cb][:])

    # gate (fp32, from xa scaled by invr2 via transposed column)
    ct_all = pmid.tile([E, TLOC], BF16)
    for tb in range(4):
        tsl = slice(tb * P, (tb + 1) * P)
        g_ps = psh.tile([P, E], F32, name="g_ps", tag="gps")
        for cb in range(NCB):
            nc.tensor.matmul(g_ps[:], xa[cb][:, tsl], wgate_sb[:, cb, :],
                             start=(cb == 0), stop=(cb == NCB - 1))
        ir_ps = psh.tile([P, 1], F32, name="ir_ps", tag="irp")
        nc.tensor.transpose(ir_ps[:], invr2[:, tsl], ones1f[:])
        ir_col = pt2.tile([P, 1], F32, name="ir_col", tag="irc")
        nc.scalar.copy(ir_col[:], ir_ps[:])
        lg = pt2.tile([P, E], F32, name="lg", tag="lg")
        nc.vector.scalar_tensor_tensor(lg[:], g_ps[:], ir_col[:], biasg_sb[:],
                                       op0=ALU.mult, op1=ALU.add)
        m1 = pt2.tile([P, 1], F32, name="m1", tag="m1")
        nc.vector.reduce_max(m1[:], lg[:], axis=mybir.AxisListType.X)
        eq1 = pt2.tile([P, E], F32, name="eq1", tag="eq1")
        nc.vector.tensor_scalar(eq1[:], lg[:], m1[:], None, op0=ALU.is_equal)
        lm = pt2.tile([P, E], F32, name="lm", tag="lm")
        nc.vector.scalar_tensor_tensor(lm[:], eq1[:], -1e9, lg[:],
                                       op0=ALU.mult, op1=ALU.add)
        m2 = pt2.tile([P, 1], F32, name="m2", tag="m2")
        nc.vector.reduce_max(m2[:], lm[:], axis=mybir.AxisListType.X)
        eq2 = pt2.tile([P, E], F32, name="eq2", tag="eq2")
        nc.vector.tensor_scalar(eq2[:], lm[:], m2[:], None, op0=ALU.is_equal)
        dm = pt2.tile([P, 1], F32, name="dm", tag="dm")
        nc.vector.tensor_scalar(dm[:], m1[:], m2[:], None, op0=ALU.subtract)
        w1 = pt2.tile([P, 1], F32, name="w1", tag="w1")
        nc.scalar.activation(w1[:], dm[:], AF.Sigmoid)
        w2 = pt2.tile([P, 1], F32, name="w2", tag="w2")
        nc.vector.tensor_scalar(w2[:], w1[:], -1.0, 1.0, op0=ALU.mult,
                                op1=ALU.add)
        cmb = pt2.tile([P, E], F32, name="cmb", tag="cmb")
        nc.vector.tensor_scalar(cmb[:], eq1[:], w1[:], None, op0=ALU.mult)
        cm2 = pt2.tile([P, E], F32, name="cm2", tag="cm2")
        nc.vector.tensor_scalar(cm2[:], eq2[:], w2[:], None, op0=ALU.mult)
        cmf = pt2.tile([P, E], F32, name="cmf", tag="cmf")
        nc.vector.tensor_tensor(cmf[:], cmb[:], cm2[:], ALU.add)
        ct_ps = psh.tile([E, P], F32, name="ct_ps", tag="ctp")
        nc.tensor.transpose(ct_ps[:], cmf[:], ident_sb[:])
        nc.scalar.copy(ct_all[:, tsl], ct_ps[:])
    if DEBUG:
        nc.sync.dma_start(dcm_d.ap(), ct_all[:])
    bcomb = []
    for e in range(E):
        cte = pt2.tile([1, TLOC], BF16, name="cte", tag="cte")
        nc.sync.dma_start(cte[:], ct_all[e:e + 1, :])
        tl = pmid.tile([P, TLOC], BF16, name=f"bcomb{e}")
        nc.gpsimd.partition_broadcast(tl[:], cte[:])
        bcomb.append(tl)

    fin.close()
    attn.close()

    # ---- MoE: fp8 DoubleRow, quad-of-experts accumulation in PSUM.
    # Quads: [shared], [e1..e4], [e5..e8]. Within a quad the token-half loop
    # is outer so the down psum (4 banks) accumulates all its experts; one
    # flush to SBUF per (quad, half, group). Weights pool holds up to 4
    # experts + 1 prefetch (pw bufs=5).
    moe = contextlib.ExitStack()
    pgu = moe.enter_context(tc.tile_pool(name="pgu", bufs=1, space="PSUM"))
    pwd = moe.enter_context(tc.tile_pool(name="pwd", bufs=1, space="PSUM"))
    pmoe = moe.enter_context(tc.tile_pool(name="pmoe", bufs=2))
    pac2 = moe.enter_context(tc.tile_pool(name="pac2", bufs=1))

    accs = [pac2.tile([P, NIB, TLOC], F32, name=f"acc{g}") for g in range(2)]

    pwm = moe.enter_context(tc.tile_pool(name="pwm", bufs=1, side="right"))
    quads = [[0], [1, 2], [3, 4, 5, 6, 7, 8]]
    for qi, quad in enumerate(quads):
        for e in quad:
            if e in wq_tiles:
                continue
            pl = pw if e < 6 else pwm
            wgt = pl.tile([P, NCB, I], F8, name="wgt", tag=f"wgt{e % 3}")
            nc.sync.dma_start(wgt[:], wg_d.ap()[e])
            wut = pl.tile([P, NCB, I], F8, name="wut", tag=f"wut{e % 3}")
            nc.sync.dma_start(wut[:], wu_d.ap()[e])
            wdt = pl.tile([P, NIB, C], F8, name="wdt", tag=f"wdt{e % 3}")
            nc.sync.dma_start(wdt[:], wd_d.ap()[e])
            wq_tiles[e] = (wgt, wut, wdt)
        for th in range(2):
            hsl = slice(th * CHUNK, (th + 1) * CHUNK)
            dps = [pwd.tile([P, NIB, CHUNK], F32, name=f"dps{g}", tag=f"d{g}")
                   for g in range(2)]
            hps = []
            for ei, e in enumerate(quad):
                wgt, wut, wdt = wq_tiles[e]
                gp = pgu.tile([P, NIB, CHUNK], F32, name="gp", tag="gp")
                for ib in range(NIB):
                    isl = slice(ib * P, (ib + 1) * P)
                    for j in range(4):
                        nc.tensor.matmul(gp[:, ib, :],
                                         wgt[:, 2 * j:2 * j + 2, isl],
                                         xmf8[:, 2 * j:2 * j + 2, hsl],
                                         start=(j == 0), stop=(j == 3),
                                         perf_mode=DR)
                sg = pmoe.tile([P, NIB, CHUNK], BF16, name="sg", tag="sg")
                nc.scalar.activation(sg[:], gp[:], AF.Silu, scale=1.0 / WSC)
                if e > 0:
                    sgc = pmoe.tile([P, NIB, CHUNK], BF16, name="sgc",
                                    tag="sgc")
                    bce = bcomb[e - 1]
                    for ib in range(NIB):
                        nc.vector.tensor_tensor(sgc[:, ib, :], sg[:, ib, :],
                                                bce[:, hsl], ALU.mult)
                else:
                    sgc = sg
                up = pgu.tile([P, NIB, CHUNK], F32, name="up", tag="up")
                for ib in range(NIB):
                    isl = slice(ib * P, (ib + 1) * P)
                    for j in range(4):
                        nc.tensor.matmul(up[:, ib, :],
                                         wut[:, 2 * j:2 * j + 2, isl],
                                         xmf8[:, 2 * j:2 * j + 2, hsl],
                                         start=(j == 0), stop=(j == 3),
                                         perf_mode=DR)
                hp = pmoe.tile([P, NIB, CHUNK], F8, name="hp", tag="hp",
                               bufs=7)
                nc.vector.scalar_tensor_tensor(hp[:], up[:], 1.0 / HSC,
                                               sgc[:],
                                               op0=ALU.mult, op1=ALU.mult)
                hps.append(hp)
            # region-major down: a PSUM region's accumulation group must
            # finish before the next group in the same bank starts
            # (start_tensor_calc clears has_written bank-wide).
            for cb in range(NCB):
                dst = dps[cb // 4][:, cb % 4, :]
                for ei, e in enumerate(quad):
                    wdt = wq_tiles[e][2]
                    for j in range(2):
                        nc.tensor.matmul(dst, wdt[:, 2 * j:2 * j + 2,
                                                  cb * P:(cb + 1) * P],
                                         hps[ei][:, 2 * j:2 * j + 2, :],
                                         start=(ei == 0 and j == 0),
                                         stop=(ei == len(quad) - 1 and j == 1),
                                         perf_mode=DR)
            for g in range(2):
                if qi == 0:
                    nc.scalar.copy(accs[g][:, :, hsl], dps[g][:])
                else:
                    nc.vector.scalar_tensor_tensor(accs[g][:, :, hsl],
                                                   dps[g][:], 1.0,
                                                   accs[g][:, :, hsl],
                                                   op0=ALU.mult, op1=ALU.add)

    if DEBUG:
        for g in range(2):
            nc.sync.dma_start(dac_d.ap()[:, g], accs[g][:])
    # ---- out = acc/512 + xa
    fo = pac2.tile([P, NCB, TLOC], F32)
    for cb in range(NCB):
        nc.vector.scalar_tensor_tensor(fo[:, cb, :],
                                       accs[cb // 4][:, cb % 4, :], OSC,
                                       xa[cb][:], op0=ALU.mult, op1=ALU.add)
    nc.sync.dma_start(out_d.ap(), fo[:])

    moe.close()
    whole.close()


# =============================================================== host side
def _build():
    if "nc" in _CACHE:
        return _CACHE["nc"]
    nc = bacc.Bacc("TRN2", target_bir_lowering=False, debug=False,
                   num_devices=8)
    with tile.TileContext(nc) as tc:
        _emit(nc, tc)
    nc.compile()
    _CACHE["nc"] = nc
    return nc


def _rope_tables(pos):
    # pos: (N,) positions; returns cos,ssin of shape (ROPE, N) in the
    # row-pair layout (rows 2i/2i+1 both carry angle pos*freq_i; ssin row 2i
    # is -sin, row 2i+1 is +sin).
    freqs = 1.0 / (THETA ** (np.arange(0, ROPE, 2, dtype=np.float32) / ROPE))
    ang = np.outer(freqs, pos.astype(np.float32))          # (16, N)
    cos = np.repeat(np.cos(ang), 2, axis=0).astype(np.float32)
    sin = np.sin(ang).astype(np.float32)
    ssin = np.empty((ROPE, len(pos)), np.float32)
    ssin[0::2] = -sin
    ssin[1::2] = sin
    return cos, ssin


def _blk(a):
    # (C_like, X) -> (128, nb, X): row cb*128+p -> [p, cb, :]
    nb = a.shape[0] // P
    return np.ascontiguousarray(
        a.reshape(nb, P, -1).transpose(1, 0, 2))


def _f8(a):
    return np.clip(np.asarray(a, np.float32), -240.0, 240.0).astype(
        ml_dtypes.float8_e4m3)


def _host_shared(inputs):
    bf = lambda a: np.ascontiguousarray(a).astype(ml_dtypes.bfloat16)
    f32 = lambda a: np.ascontiguousarray(a, dtype=np.float32)
    w_ln1 = np.asarray(inputs["w_ln1"], np.float32)
    w_ln2 = np.asarray(inputs["w_ln2"], np.float32)
    posk = np.arange(T, dtype=np.float32)
    ck, sk = _rope_tables(posk)
    p32 = np.zeros((ROPE, ROPE), np.float32)
    for i in range(ROPE // 2):
        p32[2 * i + 1, 2 * i] = 1.0
        p32[2 * i, 2 * i + 1] = 1.0
    p64 = np.zeros((2 * ROPE, 2 * ROPE), np.float32)
    p64[:ROPE, :ROPE] = p32
    p64[ROPE:, ROPE:] = p32
    wq = np.asarray(inputs["Wq"], np.float32) * w_ln1[:, None]
    wkva = np.asarray(inputs["Wkva"], np.float32) * w_ln1[:, None]
    wo_nope = np.asarray(inputs["Wo"], np.float32).reshape(H, D, C)[:, :NOPE]
    wgate = np.asarray(inputs["Wgate"], np.float32) * w_ln2[:, None]
    biasg = np.broadcast_to(np.asarray(inputs["expert_bias"], np.float32),
                            (P, E)).copy()
    wg = np.asarray(inputs["Wg"], np.float32) * w_ln2[None, :, None]
    wu = np.asarray(inputs["Wu"], np.float32) * w_ln2[None, :, None]
    wd = np.asarray(inputs["Wd"], np.float32)
    swg = np.asarray(inputs["sWg"], np.float32)[0] * w_ln2[:, None]
    swu = np.asarray(inputs["sWu"], np.float32)[0] * w_ln2[:, None]
    swd = np.asarray(inputs["sWd"], np.float32)[0]
    wg9 = np.concatenate([swg[None], wg], axis=0) * WSC    # (9, C, I)
    wu9 = np.concatenate([swu[None], wu], axis=0) * WSC
    wd9 = np.concatenate([swd[None], wd], axis=0) * WSC    # (9, I, C)
    wg8 = wg9.reshape(E + 1, NCB, P, I).transpose(0, 2, 1, 3).reshape(
        E + 1, P, NCB * I)
    wu8 = wu9.reshape(E + 1, NCB, P, I).transpose(0, 2, 1, 3).reshape(
        E + 1, P, NCB * I)
    wd8 = wd9.reshape(E + 1, NIB, P, C).transpose(0, 2, 1, 3).reshape(
        E + 1, P, NIB * C)
    xT = [np.asarray(inputs["x"], np.float32)[b].T for b in range(B)]
    return {
        "shared": {
            "wq": bf(_blk(wq)),
            "wkva": bf(_blk(wkva)),
            "wkvb": bf(inputs["Wkvb"]),
            "wo": bf(_blk(wo_nope.reshape(H * NOPE, C))),
            "cosk": f32(ck), "ssink": f32(sk),
            "perm64": bf(p64), "perm32": bf(p32),
            "ident": np.eye(P, dtype=np.float32),
            "wgate": f32(_blk(wgate)),
            "biasg": biasg,
            "wg8": _f8(wg8), "wu8": _f8(wu8), "wd8": _f8(wd8),
        },
        "xT": xT,
        "xt_bf": [bf(_blk(xT[b])) for b in range(B)],
    }


def _host_inputs(inputs, core, shared):
    bf = lambda a: np.ascontiguousarray(a).astype(ml_dtypes.bfloat16)
    f32 = lambda a: np.ascontiguousarray(a, dtype=np.float32)
    b, q = core // 4, core % 4
    hi, lo = 7 - q, q           # slot0 = chunk hi, slot1 = chunk lo
    xT = shared["xT"][b]                                   # (C, T)
    loc_cols = np.r_[np.arange(hi * CHUNK, (hi + 1) * CHUNK),
                     np.arange(lo * CHUNK, (lo + 1) * CHUNK)]
    xloc = xT[:, loc_cols]
    posq = loc_cols.astype(np.float32)
    cq, sq = _rope_tables(posq)                            # (32, 512)

    # causal masks: cols 0:2048 slot0 kb 8..15 ; cols 2048:4096 slot1 kb 0..7
    kmask = np.zeros((P, 16 * CHUNK), np.float32)
    ki = np.arange(P)[:, None]
    qi = np.arange(CHUNK)[None, :]
    for half, (j, kbs) in enumerate(((hi, range(8, 16)), (lo, range(0, 8)))):
        for i, kb in enumerate(kbs):
            m = np.zeros((P, CHUNK), np.float32)
            if kb < 2 * j:
                m[:] = 1.0
            elif kb == 2 * j:
                m = (ki <= qi).astype(np.float32)
            elif kb == 2 * j + 1:
                m = (ki + P <= qi).astype(np.float32)
            col = half * 8 * CHUNK + i * CHUNK
            kmask[:, col:col + CHUNK] = m

    m = dict(shared["shared"])
    m.update({
        "xt": shared["xt_bf"][b],
        "xloc": f32(_blk(xloc)),
        "xlbf": bf(_blk(xloc)),
        "cosq": f32(np.vstack([cq, cq])), "ssinq": f32(np.vstack([sq, sq])),
        "kmask": bf(kmask),
    })
    return m


LAST_RESULTS = None


def kernel(**inputs):
    global LAST_RESULTS
    nc = _build()
    shared = _host_shared(inputs)
    in_maps = [_host_inputs(inputs, core, shared) for core in range(8)]
    kw = {}
    if os.environ.get("BASSK_TRACE"):
        kw = dict(trace=True, trace_cores=[0], stitch_traces=False)
    res = bass_utils.run_bass_kernel_spmd(nc, in_maps, core_ids=list(range(8)),
                                          **kw)
    LAST_RESULTS = res
    out = np.empty((B, T, C), np.float32)
    for core in range(8):
        b, q = core // 4, core % 4
        hi, lo = 7 - q, q
        oT = res.results[core]["outT"]                 # (128, NCB, TLOC)
        full = oT.transpose(1, 0, 2).reshape(C, TLOC)  # (C, 512)
        out[b, hi * CHUNK:(hi + 1) * CHUNK] = full[:, :CHUNK].T
        out[b, lo * CHUNK:(lo + 1) * CHUNK] = full[:, CHUNK:].T
    return out


# revision 32
# speedup vs baseline: 16197.4075x; 1.0071x over previous
# DeepSeek block (MLA attention + top-2-of-8 MoE + shared expert) on 8 TRN2
# NeuronCores, zero-collective sharding.
#
# Core c in [0..8): sequence b = c//4, q = c%4; owns token chunks
# hi = 7-q (slot 0) and lo = q (slot 1), 256 tokens each (causally balanced).
# SPMD: identical program on all cores; per-core data (x columns, rope
# tables, causal masks) arrives as inputs.
#
# v2 layout/perf notes:
# - activations feature-on-partition; matmuls bf16 except MoE which runs
#   fp8e4 DoubleRow (both operands packed [128,2,*], contract 256/matmul).
#   MoE weights pre-scaled x64 on host (fp8e4 min normal 2^-6), down input
#   h carries x8; output rescaled by 1/512 at the final accumulate.
# - one DMA per logical matrix (DRAM tensors pre-arranged (128, blk, cols)).
# - wide ACT ops ([128,1024] exp/silu) to amortize the fixed ACT overhead;
#   causal masks applied only to key blocks 8..15 of slot0 and 0..7 of
#   slot1 (interior blocks are mask-free on every core).
# - expert accumulation on the gpsimd (Pool) engine into SBUF, down
#   projections transient in PSUM.
import os
import numpy as np
import ml_dtypes

import concourse.bacc as bacc
import concourse.mybir as mybir
import concourse.tile as tile
from concourse import bass_utils

F32 = mybir.dt.float32
BF16 = mybir.dt.bfloat16
F8 = mybir.dt.float8e4
AF = mybir.ActivationFunctionType
ALU = mybir.AluOpType
DR = mybir.MatmulPerfMode.DoubleRow

B, T, C, H, D = 2, 2048, 1024, 16, 64
R, ROPE, NOPE = 128, 32, 32
E, I = 8, 512
THETA, EPS = 100000.0, 1e-5
P = 128
NCB = C // P             # 8 C blocks
NIB = I // P             # 4 I blocks
TLOC, CHUNK = 512, 256
KB_SLOT = (16, 8)        # key blocks (of 128) attended per chunk slot

WSC = 64.0               # fp8 weight prescale
HSC = 8.0                # fp8 hidden prescale
OSC = 1.0 / (WSC * HSC)  # down-psum rescale

_CACHE = {}


# =============================================================== device IR
def _emit(nc, tc):
    import contextlib

    def din(name, shape, dt):
        return nc.dram_tensor(name, shape, dt, kind="ExternalInput")

    xt_d = din("xt", (P, NCB, T), BF16)
    xloc_d = din("xloc", (P, NCB, TLOC), F32)
    xlbf_d = din("xlbf", (P, NCB, TLOC), BF16)
    wq_d = din("wq", (P, NCB, H * D), BF16)
    wkva_d = din("wkva", (P, NCB, R + ROPE), BF16)
    wkvb_d = din("wkvb", (R, H * NOPE), BF16)
    wo_d = din("wo", (P, 4, C), BF16)
    cosk_d = din("cosk", (ROPE, T), F32)
    ssink_d = din("ssink", (ROPE, T), F32)
    cosq_d = din("cosq", (2 * ROPE, TLOC), F32)
    ssinq_d = din("ssinq", (2 * ROPE, TLOC), F32)
    perm64_d = din("perm64", (2 * ROPE, 2 * ROPE), BF16)
    perm32_d = din("perm32", (ROPE, ROPE), BF16)
    ident_d = din("ident", (P, P), F32)
    kmask_d = din("kmask", (P, 16 * CHUNK), BF16)
    wgate_d = din("wgate", (P, NCB, E), F32)
    biasg_d = din("biasg", (P, E), F32)
    wg_d = din("wg8", (E + 1, P, NCB * I), F8)   # index 0 = shared expert
    wu_d = din("wu8", (E + 1, P, NCB * I), F8)
    wd_d = din("wd8", (E + 1, P, NIB * C), F8)
    out_d = nc.dram_tensor("outT", (P, NCB, TLOC), F32, kind="ExternalOutput")
    DEBUG = bool(int(os.environ.get("BASSK_DEBUG", "0")))
    if DEBUG:
        dxa_d = nc.dram_tensor("d_xa", (P, NCB, TLOC), F32,
                               kind="ExternalOutput")
        dcm_d = nc.dram_tensor("d_comb", (E, TLOC), BF16,
                               kind="ExternalOutput")
        dem_d = nc.dram_tensor("d_em", (P, 16 * CHUNK), BF16,
                               kind="ExternalOutput")
        dac_d = nc.dram_tensor("d_acc", (P, 2, NIB, TLOC), F32,
                               kind="ExternalOutput")

    whole = contextlib.ExitStack()
    early = contextlib.ExitStack()
    attn = contextlib.ExitStack()

    pc = whole.enter_context(tc.tile_pool(name="pc", bufs=1))
    pw = whole.enter_context(tc.tile_pool(name="pw", bufs=1, side="right"))
    pmid = whole.enter_context(tc.tile_pool(name="pmid", bufs=1))

    # pa: tiles written during the early phase but read by attention
    pa = attn.enter_context(tc.tile_pool(name="pa", bufs=1))
    pt2 = attn.enter_context(tc.tile_pool(name="pt2", bufs=2))

    px = early.enter_context(tc.tile_pool(name="px", bufs=1))
    pt1 = early.enter_context(tc.tile_pool(name="pt1", bufs=2))
    pse = early.enter_context(tc.tile_pool(name="pse", bufs=2, space="PSUM"))

    # ---- constants / tables
    ones_128x1 = pc.tile([P, 1], BF16)
    nc.any.memset(ones_128x1[:], 1.0)
    ones1f = pc.tile([1, 1], F32)
    nc.any.memset(ones1f[:], 1.0)
    eps_sb = pc.tile([1, 1], F32)
    nc.any.memset(eps_sb[:], EPS)
    ident_sb = pc.tile([P, P], F32)
    nc.sync.dma_start(ident_sb[:], ident_d.ap())
    perm64_sb = pc.tile([2 * ROPE, 2 * ROPE], BF16)
    nc.sync.dma_start(perm64_sb[:], perm64_d.ap())
    perm32_sb = pc.tile([ROPE, ROPE], BF16)
    nc.sync.dma_start(perm32_sb[:], perm32_d.ap())
    biasg_sb = pc.tile([P, E], F32)
    nc.sync.dma_start(biasg_sb[:], biasg_d.ap())
    wgate_sb = pc.tile([P, NCB, E], F32)
    nc.sync.dma_start(wgate_sb[:], wgate_d.ap())

    cosk_t = px.tile([ROPE, T], F32)
    nc.sync.dma_start(cosk_t[:], cosk_d.ap())
    ssink_t = px.tile([ROPE, T], F32)
    nc.sync.dma_start(ssink_t[:], ssink_d.ap())
    cosq_t = px.tile([2 * ROPE, TLOC], F32)
    nc.sync.dma_start(cosq_t[:], cosq_d.ap())
    ssinq_t = px.tile([2 * ROPE, TLOC], F32)
    nc.sync.dma_start(ssinq_t[:], ssinq_d.ap())

    # ---- bulk loads
    xt = px.tile([P, NCB, T], BF16)
    for cb in range(NCB):
        nc.sync.dma_start(xt[:, cb, :], xt_d.ap()[:, cb, :])
    xlbf = px.tile([P, NCB, TLOC], BF16)
    nc.sync.dma_start(xlbf[:], xlbf_d.ap())
    wkva_sb = px.tile([P, NCB, R + ROPE], BF16)
    nc.sync.dma_start(wkva_sb[:], wkva_d.ap())
    wq_sb = px.tile([P, NCB, H * D], BF16)
    for cb in range(0, NCB, 2):
        nc.sync.dma_start(wq_sb[:, cb:cb + 2, :], wq_d.ap()[:, cb:cb + 2, :])

    # ---- prefetch first MoE expert weights (slots 0..2) before the
    # attention-section DMAs claim the SP queue
    wq_tiles = {}
    for e in (0, 1, 2):
        wgt = pw.tile([P, NCB, I], F8, name="wgt", tag=f"wgt{e % 3}")
        nc.sync.dma_start(wgt[:], wg_d.ap()[e])
        wut = pw.tile([P, NCB, I], F8, name="wut", tag=f"wut{e % 3}")
        nc.sync.dma_start(wut[:], wu_d.ap()[e])
        wdt = pw.tile([P, NIB, C], F8, name="wdt", tag=f"wdt{e % 3}")
        nc.sync.dma_start(wdt[:], wd_d.ap()[e])
        wq_tiles[e] = (wgt, wut, wdt)

    # ---- rmsnorm1 stats: global (keys) then local (queries)
    bc1 = px.tile([P, T], F32)
    for nt in range(T // 512):
        sl = slice(nt * 512, (nt + 1) * 512)
        sps = pse.tile([1, 512], F32, name="sps", tag="accA")
        for cb in range(NCB):
            xq = pt1.tile([P, 512], BF16, name="xq", tag="xq")
            nc.scalar.square(xq[:], xt[:, cb, sl])
            nc.tensor.matmul(sps[:], ones_128x1[:], xq[:],
                             start=(cb == 0), stop=(cb == NCB - 1))
        rr = pt1.tile([1, 512], F32, name="rr", tag="rr", bufs=1)
        nc.scalar.activation(rr[:], sps[:], AF.Sqrt, bias=eps_sb[:],
                             scale=1.0 / C)
        iv = pt1.tile([1, 512], F32, name="iv", tag="iv", bufs=1)
        nc.vector.reciprocal(iv[:], rr[:])
        nc.gpsimd.partition_broadcast(bc1[:, sl], iv[:])

    bc1l = px.tile([P, TLOC], F32)
    spsl = pse.tile([1, TLOC], F32, name="spsl", tag="accA")
    for cb in range(NCB):
        xql = pt1.tile([P, TLOC], BF16, name="xql", tag="xq")
        nc.scalar.square(xql[:], xlbf[:, cb, :])
        nc.tensor.matmul(spsl[:], ones_128x1[:], xql[:],
                         start=(cb == 0), stop=(cb == NCB - 1))
    rrl = pt1.tile([1, TLOC], F32, name="rrl", tag="rr", bufs=1)
    nc.scalar.activation(rrl[:], spsl[:], AF.Sqrt, bias=eps_sb[:],
                         scale=1.0 / C)
    ivl = pt1.tile([1, TLOC], F32, name="ivl", tag="iv", bufs=1)
    nc.vector.reciprocal(ivl[:], rrl[:])
    nc.gpsimd.partition_broadcast(bc1l[:], ivl[:])

    # ---- ckv: kv latent (scaled) + scaled k_rope
    kvlat = pa.tile([R, T], BF16)
    krsc = pa.tile([ROPE, T], BF16)     # scaled raw k_rope
    for nt in range(T // 512):
        sl = slice(nt * 512, (nt + 1) * 512)
        lat_ps = pse.tile([P, 512], F32, name="lat_ps", tag="pA")
        for cb in range(NCB):
            nc.tensor.matmul(lat_ps[:], wkva_sb[:, cb, 0:R], xt[:, cb, sl],
                             start=(cb == 0), stop=(cb == NCB - 1))
        rop_ps = pse.tile([ROPE, 512], F32, name="rop_ps", tag="par")
        for cb in range(NCB):
            nc.tensor.matmul(rop_ps[:], wkva_sb[:, cb, R:R + ROPE],
                             xt[:, cb, sl],
                             start=(cb == 0), stop=(cb == NCB - 1))
        nc.vector.tensor_tensor(kvlat[:, sl], lat_ps[:], bc1[:, sl], ALU.mult)
        nc.vector.tensor_tensor(krsc[:, sl], rop_ps[:], bc1[0:ROPE, sl],
                                ALU.mult)

    # ---- rope K -> kropebf [32, T]
    kropebf = pa.tile([ROPE, T], BF16)
    for nt in range(T // 512):
        sl = slice(nt * 512, (nt + 1) * 512)
        park = pse.tile([ROPE, 512], F32, name="park", tag="par")
        nc.tensor.matmul(park[:], perm32_sb[:], krsc[:, sl])
        t1k = pt1.tile([ROPE, 512], F32, name="t1k", tag="t1q")
        nc.gpsimd.tensor_tensor(t1k[:], krsc[:, sl], cosk_t[:, sl], ALU.mult)
        t2k = pt1.tile([ROPE, 512], F32, name="t2k", tag="t2q")
        nc.vector.tensor_tensor(t2k[:], park[:], ssink_t[:, sl], ALU.mult)
        nc.vector.tensor_tensor(kropebf[:, sl], t1k[:], t2k[:], ALU.add)

    # ---- Q projection + rope (whole TLOC per head-pair)
    qbf = []
    for mb in range(8):
        tl = pa.tile([P, TLOC], BF16, name=f"qbf{mb}")
        qps = pse.tile([P, TLOC], F32, name="qps", tag="pA")
        for cb in range(NCB):
            nc.tensor.matmul(qps[:], wq_sb[:, cb, mb * P:(mb + 1) * P],
                             xlbf[:, cb, :],
                             start=(cb == 0), stop=(cb == NCB - 1))
        nc.vector.tensor_tensor(tl[:], qps[:], bc1l[:], ALU.mult)
        qr = pt1.tile([2 * ROPE, TLOC], BF16, name="qr", tag="qr")
        nc.vector.tensor_tensor(qr[0:ROPE, :], qps[32:64, :], bc1l[32:64, :],
                                ALU.mult)
        nc.vector.tensor_tensor(qr[ROPE:2 * ROPE, :], qps[96:128, :],
                                bc1l[96:128, :], ALU.mult)
        parq = pse.tile([2 * ROPE, TLOC], F32, name="parq", tag="par")
        nc.tensor.matmul(parq[:], perm64_sb[:], qr[:])
        t1q = pt1.tile([2 * ROPE, TLOC], F32, name="t1q", tag="t1q")
        nc.gpsimd.tensor_tensor(t1q[:], qr[:], cosq_t[:], ALU.mult)
        t2q = pt1.tile([2 * ROPE, TLOC], F32, name="t2q", tag="t2q")
        nc.vector.tensor_tensor(t2q[:], parq[:], ssinq_t[:], ALU.mult)
        nc.vector.tensor_tensor(tl[32:64, :], t1q[0:ROPE, :], t2q[0:ROPE, :],
                                ALU.add)
        nc.vector.tensor_tensor(tl[96:128, :], t1q[ROPE:2 * ROPE, :],
                                t2q[ROPE:2 * ROPE, :], ALU.add)
        qbf.append(tl)

    early.close()

    # ---- deferred loads (space freed by the early pools)
    pk = attn.enter_context(tc.tile_pool(name="pk", bufs=1))
    xloc = pk.tile([P, NCB, TLOC], F32)
    nc.sync.dma_start(xloc[:], xloc_d.ap())
    wkvb_sb = pk.tile([R, H * NOPE], BF16)
    nc.sync.dma_start(wkvb_sb[:], wkvb_d.ap())
    kmask_sb = pk.tile([P, 16 * CHUNK], BF16)
    nc.sync.dma_start(kmask_sb[:], kmask_d.ap())
    wo_sb = pk.tile([P, 4, C], BF16)
    nc.sync.dma_start(wo_sb[:], wo_d.ap())

    # ---- k_nope -> kfull assembly; V extended with ones row.
    # rope rows depend only on kropebf: DMA them first; nope rows stream in
    # right after each knope block so scores can start early.
    kfull = []
    for mb in range(H // 2):
        kfull.append(pk.tile([P, T], BF16, name=f"kfull{mb}"))
    for mb in range(H // 2):
        nc.sync.dma_start(kfull[mb][32:64, :], kropebf[:])
        nc.sync.dma_start(kfull[mb][96:128, :], kropebf[:])
    sub = contextlib.ExitStack()
    pkx = sub.enter_context(tc.tile_pool(name="pkx", bufs=1))
    psk = sub.enter_context(tc.tile_pool(name="psk", bufs=2, space="PSUM"))
    for j in range(4):
        tl = pkx.tile([P, T], BF16, name=f"knope{j}")
        for nt in range(T // 512):
            sl = slice(nt * 512, (nt + 1) * 512)
            kps = psk.tile([P, 512], F32, name="kps", tag="pA")
            nc.tensor.matmul(kps[:], wkvb_sb[:, j * P:(j + 1) * P],
                             kvlat[:, sl])
            nc.scalar.copy(tl[:, sl], kps[:])
        for h in (4 * j, 4 * j + 1, 4 * j + 2, 4 * j + 3):
            mb, po = h // 2, (h % 2) * 64
            nc.sync.dma_start(kfull[mb][po:po + 32, :],
                              tl[(h % 4) * 32:(h % 4) * 32 + 32, :])
    vext = []
    for tb in range(16):
        tl = pk.tile([P, H, 34], BF16, name=f"vext{tb}")
        vps = psk.tile([P, H * NOPE], F32, name="vps", tag="pA")
        nc.tensor.matmul(vps[:], kvlat[:, tb * P:(tb + 1) * P], wkvb_sb[:])
        nc.scalar.copy(tl[:, :, 0:NOPE],
                       vps[:].rearrange("p (h d) -> p h d", h=H))
        nc.any.memset(tl[:, :, NOPE:NOPE + 1], 1.0)
        vext.append(tl)
    sub.close()

    # ---- attention core
    core = contextlib.ExitStack()
    psc = core.enter_context(tc.tile_pool(name="psc", bufs=2, space="PSUM"))
    pE = core.enter_context(tc.tile_pool(name="pE", bufs=2))
    yall = []
    for yb in range(4):
        yall.append(pk.tile([P, TLOC], BF16, name=f"yall{yb}"))
    for ch in range(2):
        csl = slice(ch * CHUNK, (ch + 1) * CHUNK)
        nkb = KB_SLOT[ch]
        for h in range(H):
            mb, po = h // 2, (h % 2) * 64
            em = pE.tile([P, nkb * CHUNK], BF16, name="em", tag="em")
            for g in range(nkb // 4):
                sp = psc.tile([P, 4 * CHUNK], F32, name="sp", tag="sc")
                for k4 in range(4):
                    kb = 4 * g + k4
                    nc.tensor.matmul(
                        sp[:, k4 * CHUNK:(k4 + 1) * CHUNK],
                        kfull[mb][po:po + 64, kb * P:(kb + 1) * P],
                        qbf[mb][po:po + 64, csl])
                nc.scalar.activation(em[:, g * 4 * CHUNK:(g + 1) * 4 * CHUNK],
                                     sp[:], AF.Exp, scale=0.125)
            if ch == 0:
                nc.vector.tensor_tensor(em[:, 8 * CHUNK:16 * CHUNK],
                                        em[:, 8 * CHUNK:16 * CHUNK],
                                        kmask_sb[:, 0:8 * CHUNK], ALU.mult)
            else:
                nc.vector.tensor_tensor(em[:], em[:],
                                        kmask_sb[:, 8 * CHUNK:16 * CHUNK],
                                        ALU.mult)
            if DEBUG and ch == 0 and h == 0:
                nc.sync.dma_start(dem_d.ap(), em[:])
            y_ps = psc.tile([NOPE + 1, CHUNK], F32, name="y_ps", tag="yv")
            for kb in range(nkb):
                nc.tensor.matmul(y_ps[:], vext[kb][:, h, 0:NOPE + 1],
                                 em[:, kb * CHUNK:(kb + 1) * CHUNK],
                                 start=(kb == 0), stop=(kb == nkb - 1))
            rr2 = pt2.tile([1, CHUNK], F32, name="rr2", tag="rrA")
            nc.vector.reciprocal(rr2[:], y_ps[NOPE:NOPE + 1, :])
            rb = pt2.tile([NOPE, CHUNK], F32, name="rb", tag="rb")
            nc.gpsimd.partition_broadcast(rb[:], rr2[:])
            yt = yall[h // 4]
            ro = (h % 4) * NOPE
            nc.vector.tensor_tensor(yt[ro:ro + NOPE, csl], y_ps[0:NOPE, :],
                                    rb[:], ALU.mult)
    core.close()

    # ---- Wo + residual -> xa (f32) ; rmsnorm2 ; gate ; comb broadcast
    fin = contextlib.ExitStack()
    psg = fin.enter_context(tc.tile_pool(name="psg", bufs=2, space="PSUM"))
    psh = fin.enter_context(tc.tile_pool(name="psh", bufs=1, space="PSUM"))

    xa = []
    for cb in range(NCB):
        xa.append(pmid.tile([P, TLOC], F32, name=f"xa{cb}"))
    for cb in range(NCB):
        ops = psg.tile([P, TLOC], F32, name="ops", tag="wo")
        for kb in range(4):
            nc.tensor.matmul(ops[:], wo_sb[:, kb, cb * P:(cb + 1) * P],
                             yall[kb][:], start=(kb == 0), stop=(kb == 3))
        nc.vector.scalar_tensor_tensor(xa[cb][:], ops[:], 1.0,
                                       xloc[:, cb, :],
                                       op0=ALU.mult, op1=ALU.add)

    invr2 = pmid.tile([1, TLOC], F32)
    sps2 = psh.tile([1, TLOC], F32, name="sps2", tag="acc2")
    for cb in range(NCB):
        xq2 = pt2.tile([P, TLOC], BF16, name="xq2", tag="xq2")
        nc.vector.tensor_tensor(xq2[:], xa[cb][:], xa[cb][:], ALU.mult)
        nc.tensor.matmul(sps2[:], ones_128x1[:], xq2[:],
                         start=(cb == 0), stop=(cb == NCB - 1))
    rr3 = pt2.tile([1, TLOC], F32, name="rr3", tag="rrA")
    nc.scalar.activation(rr3[:], sps2[:], AF.Sqrt, bias=eps_sb[:],
                         scale=1.0 / C)
    nc.vector.reciprocal(invr2[:], rr3[:])
    bc2 = pt2.tile([P, TLOC], F32, name="bc2", tag="bc2")
    nc.gpsimd.partition_broadcast(bc2[:], invr2[:])

    xmf8 = pmid.tile([P, NCB, TLOC], F8)
    for cb in range(NCB):
        nc.vector.tensor_tensor(xmf8[:, cb, :], xa[cb][:], bc2[:], ALU.mult)
    if DEBUG:
        for cb in range(NCB):
            nc.sync.dma_start(dxa_d.ap()[:, cb, :], xa[cb][:])

    # gate (fp32, from xa scaled by invr2 via transposed column)
    ct_all = pmid.tile([E, TLOC], BF16)
    for tb in range(4):
        tsl = slice(tb * P, (tb + 1) * P)
        g_ps = psh.tile([P, E], F32, name="g_ps", tag="gps")
        for cb in range(NCB):
            nc.tensor.matmul(g_ps[:], xa[cb][:, tsl], wgate_sb[:, cb, :],
                             start=(cb == 0), stop=(cb == NCB - 1))
        ir_ps = psh.tile([P, 1], F32, name="ir_ps", tag="irp")
        nc.tensor.transpose(ir_ps[:], invr2[:, tsl], ones1f[:])
        ir_col = pt2.tile([P, 1], F32, name="ir_col", tag="irc")
        nc.scalar.copy(ir_col[:], ir_ps[:])
        lg = pt2.tile([P, E], F32, name="lg", tag="lg")
        nc.vector.scalar_tensor_tensor(lg[:], g_ps[:], ir_col[:], biasg_sb[:],
                                       op0=ALU.mult, op1=ALU.add)
        m1 = pt2.tile([P, 1], F32, name="m1", tag="m1")
        nc.vector.reduce_max(m1[:], lg[:], axis=mybir.AxisListType.X)
        eq1 = pt2.tile([P, E], F32, name="eq1", tag="eq1")
        nc.vector.tensor_scalar(eq1[:], lg[:], m1[:], None, op0=ALU.is_equal)
        lm = pt2.tile([P, E], F32, name="lm", tag="lm")
        nc.vector.scalar_tensor_tensor(lm[:], eq1[:], -1e9, lg[:],
                                       op0=ALU.mult, op1=ALU.add)
        m2 = pt2.tile([P, 1], F32, name="m2", tag="m2")
        nc.vector.reduce_max(m2[:], lm[:], axis=mybir.AxisListType.X)
        eq2 = pt2.tile([P, E], F32, name="eq2", tag="eq2")
        nc.vector.tensor_scalar(eq2[:], lm[:], m2[:], None, op0=ALU.is_equal)
        dm = pt2.tile([P, 1], F32, name="dm", tag="dm")
        nc.vector.tensor_scalar(dm[:], m1[:], m2[:], None, op0=ALU.subtract)
        w1 = pt2.tile([P, 1], F32, name="w1", tag="w1")
        nc.scalar.activation(w1[:], dm[:], AF.Sigmoid)
        w2 = pt2.tile([P, 1], F32, name="w2", tag="w2")
        nc.vector.tensor_scalar(w2[:], w1[:], -1.0, 1.0, op0=ALU.mult,
                                op1=ALU.add)
        cmb = pt2.tile([P, E], F32, name="cmb", tag="cmb")
        nc.vector.tensor_scalar(cmb[:], eq1[:], w1[:], None, op0=ALU.mult)
        cm2 = pt2.tile([P, E], F32, name="cm2", tag="cm2")
        nc.vector.tensor_scalar(cm2[:], eq2[:], w2[:], None, op0=ALU.mult)
        cmf = pt2.tile([P, E], F32, name="cmf", tag="cmf")
        nc.vector.tensor_tensor(cmf[:], cmb[:], cm2[:], ALU.add)
        ct_ps = psh.tile([E, P], F32, name="ct_ps", tag="ctp")
        nc.tensor.transpose(ct_ps[:], cmf[:], ident_sb[:])
        nc.scalar.copy(ct_all[:, tsl], ct_ps[:])
    if DEBUG:
        nc.sync.dma_start(dcm_d.ap(), ct_all[:])
    bcomb = []
    for e in range(E):
        cte = pt2.tile([1, TLOC], BF16, name="cte", tag="cte")
        nc.sync.dma_start(cte[:], ct_all[e:e + 1, :])
        tl = pmid.tile([P, TLOC], BF16, name=f"bcomb{e}")
        nc.gpsimd.partition_broadcast(tl[:], cte[:])
        bcomb.append(tl)

    fin.close()
    attn.close()

    # ---- MoE: fp8 DoubleRow, quad-of-experts accumulation in PSUM.
    # Quads: [shared], [e1..e4], [e5..e8]. Within a quad the token-half loop
    # is outer so the down psum (4 banks) accumulates all its experts; one
    # flush to SBUF per (quad, half, group). Weights pool holds up to 4
    # experts + 1 prefetch (pw bufs=5).
    moe = contextlib.ExitStack()
    pgu = moe.enter_context(tc.tile_pool(name="pgu", bufs=1, space="PSUM"))
    pwd = moe.enter_context(tc.tile_pool(name="pwd", bufs=1, space="PSUM"))
    pmoe = moe.enter_context(tc.tile_pool(name="pmoe", bufs=2))
    pac2 = moe.enter_context(tc.tile_pool(name="pac2", bufs=1))

    accs = [pac2.tile([P, NIB, TLOC], F32, name=f"acc{g}") for g in range(2)]

    pwm = moe.enter_context(tc.tile_pool(name="pwm", bufs=1, side="right"))
    quads = [[0], [1, 2], [3, 4, 5, 6, 7, 8]]
    for qi, quad in enumerate(quads):
        for e in quad:
            if e in wq_tiles:
                continue
            pl = pw if e < 6 else pwm
            wgt = pl.tile([P, NCB, I], F8, name="wgt", tag=f"wgt{e % 3}")
            nc.sync.dma_start(wgt[:], wg_d.ap()[e])
            wut = pl.tile([P, NCB, I], F8, name="wut", tag=f"wut{e % 3}")
            nc.sync.dma_start(wut[:], wu_d.ap()[e])
            wdt = pl.tile([P, NIB, C], F8, name="wdt", tag=f"wdt{e % 3}")
            nc.sync.dma_start(wdt[:], wd_d.ap()[e])
            wq_tiles[e] = (wgt, wut, wdt)
        for th in range(2):
            hsl = slice(th * CHUNK, (th + 1) * CHUNK)
            dps = [pwd.tile([P, NIB, CHUNK], F32, name=f"dps{g}", tag=f"d{g}")
                   for g in range(2)]
            hps = []
            for ei, e in enumerate(quad):
                wgt, wut, wdt = wq_tiles[e]
                gp = pgu.tile([P, NIB, CHUNK], F32, name="gp", tag="gp")
                for ib in range(NIB):
                    isl = slice(ib * P, (ib + 1) * P)
                    for j in range(4):
                        nc.tensor.matmul(gp[:, ib, :],
                                         wgt[:, 2 * j:2 * j + 2, isl],
                                         xmf8[:, 2 * j:2 * j + 2, hsl],
                                         start=(j == 0), stop=(j == 3),
                                         perf_mode=DR)
                sg = pmoe.tile([P, NIB, CHUNK], BF16, name="sg", tag="sg")
                nc.scalar.activation(sg[:], gp[:], AF.Silu, scale=1.0 / WSC)
                if e > 0:
                    sgc = pmoe.tile([P, NIB, CHUNK], BF16, name="sgc",
                                    tag="sgc")
                    bce = bcomb[e - 1]
                    for ib in range(NIB):
                        nc.vector.tensor_tensor(sgc[:, ib, :], sg[:, ib, :],
                                                bce[:, hsl], ALU.mult)
                else:
                    sgc = sg
                up = pgu.tile([P, NIB, CHUNK], F32, name="up", tag="up")
                for ib in range(NIB):
                    isl = slice(ib * P, (ib + 1) * P)
                    for j in range(4):
                        nc.tensor.matmul(up[:, ib, :],
                                         wut[:, 2 * j:2 * j + 2, isl],
                                         xmf8[:, 2 * j:2 * j + 2, hsl],
                                         start=(j == 0), stop=(j == 3),
                                         perf_mode=DR)
                hp = pmoe.tile([P, NIB, CHUNK], F8, name="hp", tag="hp",
                               bufs=7)
                nc.vector.scalar_tensor_tensor(hp[:], up[:], 1.0 / HSC,
                                               sgc[:],
                                               op0=ALU.mult, op1=ALU.mult)
                hps.append(hp)
            # region-major down: a PSUM region's accumulation group must
            # finish before the next group in the same bank starts
            # (start_tensor_calc clears has_written bank-wide).
            for cb in range(NCB):
                dst = dps[cb // 4][:, cb % 4, :]
                for ei, e in enumerate(quad):
                    wdt = wq_tiles[e][2]
                    for j in range(2):
                        nc.tensor.matmul(dst, wdt[:, 2 * j:2 * j + 2,
                                                  cb * P:(cb + 1) * P],
                                         hps[ei][:, 2 * j:2 * j + 2, :],
                                         start=(ei == 0 and j == 0),
                                         stop=(ei == len(quad) - 1 and j == 1),
                                         perf_mode=DR)
            for g in range(2):
                if qi == 0:
                    nc.scalar.copy(accs[g][:, :, hsl], dps[g][:])
                else:
                    nc.vector.scalar_tensor_tensor(accs[g][:, :, hsl],
                                                   dps[g][:], 1.0,
                                                   accs[g][:, :, hsl],
                                                   op0=ALU.mult, op1=ALU.add)

    if DEBUG:
        for g in range(2):
            nc.sync.dma_start(dac_d.ap()[:, g], accs[g][:])
    # ---- out = acc/512 + xa (per accumulator group, so the first half of
    # the output DMA overlaps the last experts' flushes)
    fo = pac2.tile([P, NCB, TLOC], F32)
    for g in range(2):
        for k in range(NIB):
            cb = 4 * g + k
            nc.vector.scalar_tensor_tensor(fo[:, cb, :], accs[g][:, k, :],
                                           OSC, xa[cb][:],
                                           op0=ALU.mult, op1=ALU.add)
        nc.sync.dma_start(out_d.ap()[:, 4 * g:4 * g + 4, :],
                          fo[:, 4 * g:4 * g + 4, :])

    moe.close()
    whole.close()


# =============================================================== host side
def _build():
    if "nc" in _CACHE:
        return _CACHE["nc"]
    nc = bacc.Bacc("TRN2", target_bir_lowering=False, debug=False,
                   num_devices=8)
    with tile.TileContext(nc) as tc:
        _emit(nc, tc)
    nc.compile()
    _CACHE["nc"] = nc
    return nc


def _rope_tables(pos):
    # pos: (N,) positions; returns cos,ssin of shape (ROPE, N) in the
    # row-pair layout (rows 2i/2i+1 both carry angle pos*freq_i; ssin row 2i
    # is -sin, row 2i+1 is +sin).
    freqs = 1.0 / (THETA ** (np.arange(0, ROPE, 2, dtype=np.float32) / ROPE))
    ang = np.outer(freqs, pos.astype(np.float32))          # (16, N)
    cos = np.repeat(np.cos(ang), 2, axis=0).astype(np.float32)
    sin = np.sin(ang).astype(np.float32)
    ssin = np.empty((ROPE, len(pos)), np.float32)
    ssin[0::2] = -sin
    ssin[1::2] = sin
    return cos, ssin


def _blk(a):
    # (C_like, X) -> (128, nb, X): row cb*128+p -> [p, cb, :]
    nb = a.shape[0] // P
    return np.ascontiguousarray(
        a.reshape(nb, P, -1).transpose(1, 0, 2))


def _f8(a):
    return np.clip(np.asarray(a, np.float32), -240.0, 240.0).astype(
        ml_dtypes.float8_e4m3)


def _host_shared(inputs):
    bf = lambda a: np.ascontiguousarray(a).astype(ml_dtypes.bfloat16)
    f32 = lambda a: np.ascontiguousarray(a, dtype=np.float32)
    w_ln1 = np.asarray(inputs["w_ln1"], np.float32)
    w_ln2 = np.asarray(inputs["w_ln2"], np.float32)
    posk = np.arange(T, dtype=np.float32)
    ck, sk = _rope_tables(posk)
    p32 = np.zeros((ROPE, ROPE), np.float32)
    for i in range(ROPE // 2):
        p32[2 * i + 1, 2 * i] = 1.0
        p32[2 * i, 2 * i + 1] = 1.0
    p64 = np.zeros((2 * ROPE, 2 * ROPE), np.float32)
    p64[:ROPE, :ROPE] = p32
    p64[ROPE:, ROPE:] = p32
    wq = np.asarray(inputs["Wq"], np.float32) * w_ln1[:, None]
    wkva = np.asarray(inputs["Wkva"], np.float32) * w_ln1[:, None]
    wo_nope = np.asarray(inputs["Wo"], np.float32).reshape(H, D, C)[:, :NOPE]
    wgate = np.asarray(inputs["Wgate"], np.float32) * w_ln2[:, None]
    biasg = np.broadcast_to(np.asarray(inputs["expert_bias"], np.float32),
                            (P, E)).copy()
    wg = np.asarray(inputs["Wg"], np.float32) * w_ln2[None, :, None]
    wu = np.asarray(inputs["Wu"], np.float32) * w_ln2[None, :, None]
    wd = np.asarray(inputs["Wd"], np.float32)
    swg = np.asarray(inputs["sWg"], np.float32)[0] * w_ln2[:, None]
    swu = np.asarray(inputs["sWu"], np.float32)[0] * w_ln2[:, None]
    swd = np.asarray(inputs["sWd"], np.float32)[0]
    wg9 = np.concatenate([swg[None], wg], axis=0) * WSC    # (9, C, I)
    wu9 = np.concatenate([swu[None], wu], axis=0) * WSC
    wd9 = np.concatenate([swd[None], wd], axis=0) * WSC    # (9, I, C)
    wg8 = wg9.reshape(E + 1, NCB, P, I).transpose(0, 2, 1, 3).reshape(
        E + 1, P, NCB * I)
    wu8 = wu9.reshape(E + 1, NCB, P, I).transpose(0, 2, 1, 3).reshape(
        E + 1, P, NCB * I)
    wd8 = wd9.reshape(E + 1, NIB, P, C).transpose(0, 2, 1, 3).reshape(
        E + 1, P, NIB * C)
    xT = [np.asarray(inputs["x"], np.float32)[b].T for b in range(B)]
    return {
        "shared": {
            "wq": bf(_blk(wq)),
            "wkva": bf(_blk(wkva)),
            "wkvb": bf(inputs["Wkvb"]),
            "wo": bf(_blk(wo_nope.reshape(H * NOPE, C))),
            "cosk": f32(ck), "ssink": f32(sk),
            "perm64": bf(p64), "perm32": bf(p32),
            "ident": np.eye(P, dtype=np.float32),
            "wgate": f32(_blk(wgate)),
            "biasg": biasg,
            "wg8": _f8(wg8), "wu8": _f8(wu8), "wd8": _f8(wd8),
        },
        "xT": xT,
        "xt_bf": [bf(_blk(xT[b])) for b in range(B)],
    }


def _host_inputs(inputs, core, shared):
    bf = lambda a: np.ascontiguousarray(a).astype(ml_dtypes.bfloat16)
    f32 = lambda a: np.ascontiguousarray(a, dtype=np.float32)
    b, q = core // 4, core % 4
    hi, lo = 7 - q, q           # slot0 = chunk hi, slot1 = chunk lo
    xT = shared["xT"][b]                                   # (C, T)
    loc_cols = np.r_[np.arange(hi * CHUNK, (hi + 1) * CHUNK),
                     np.arange(lo * CHUNK, (lo + 1) * CHUNK)]
    xloc = xT[:, loc_cols]
    posq = loc_cols.astype(np.float32)
    cq, sq = _rope_tables(posq)                            # (32, 512)

    # causal masks: cols 0:2048 slot0 kb 8..15 ; cols 2048:4096 slot1 kb 0..7
    kmask = np.zeros((P, 16 * CHUNK), np.float32)
    ki = np.arange(P)[:, None]
    qi = np.arange(CHUNK)[None, :]
    for half, (j, kbs) in enumerate(((hi, range(8, 16)), (lo, range(0, 8)))):
        for i, kb in enumerate(kbs):
            m = np.zeros((P, CHUNK), np.float32)
            if kb < 2 * j:
                m[:] = 1.0
            elif kb == 2 * j:
                m = (ki <= qi).astype(np.float32)
            elif kb == 2 * j + 1:
                m = (ki + P <= qi).astype(np.float32)
            col = half * 8 * CHUNK + i * CHUNK
            kmask[:, col:col + CHUNK] = m

    m = dict(shared["shared"])
    m.update({
        "xt": shared["xt_bf"][b],
        "xloc": f32(_blk(xloc)),
        "xlbf": bf(_blk(xloc)),
        "cosq": f32(np.vstack([cq, cq])), "ssinq": f32(np.vstack([sq, sq])),
        "kmask": bf(kmask),
    })
    return m


LAST_RESULTS = None


def kernel(**inputs):
    global LAST_RESULTS
    nc = _build()
    shared = _host_shared(inputs)
    in_maps = [_host_inputs(inputs, core, shared) for core in range(8)]
    kw = {}
    if os.environ.get("BASSK_TRACE"):
        kw = dict(trace=True, trace_cores=[0], stitch_traces=False)
    res = bass_utils.run_bass_kernel_spmd(nc, in_maps, core_ids=list(range(8)),
                                          **kw)
    LAST_RESULTS = res
    out = np.empty((B, T, C), np.float32)
    for core in range(8):
        b, q = core // 4, core % 4
        hi, lo = 7 - q, q
        oT = res.results[core]["outT"]                 # (128, NCB, TLOC)
        full = oT.transpose(1, 0, 2).reshape(C, TLOC)  # (C, 512)
        out[b, hi * CHUNK:(hi + 1) * CHUNK] = full[:, :CHUNK].T
        out[b, lo * CHUNK:(lo + 1) * CHUNK] = full[:, CHUNK:].T
    return out


# revision 34
# speedup vs baseline: 16415.9886x; 1.0135x over previous
# DeepSeek block (MLA attention + top-2-of-8 MoE + shared expert) on 8 TRN2
# NeuronCores, zero-collective sharding.
#
# Core c in [0..8): sequence b = c//4, q = c%4; owns token chunks
# hi = 7-q (slot 0) and lo = q (slot 1), 256 tokens each (causally balanced).
# SPMD: identical program on all cores; per-core data (x columns, rope
# tables, causal masks) arrives as inputs.
#
# v2 layout/perf notes:
# - activations feature-on-partition; matmuls bf16 except MoE which runs
#   fp8e4 DoubleRow (both operands packed [128,2,*], contract 256/matmul).
#   MoE weights pre-scaled x64 on host (fp8e4 min normal 2^-6), down input
#   h carries x8; output rescaled by 1/512 at the final accumulate.
# - one DMA per logical matrix (DRAM tensors pre-arranged (128, blk, cols)).
# - wide ACT ops ([128,1024] exp/silu) to amortize the fixed ACT overhead;
#   causal masks applied only to key blocks 8..15 of slot0 and 0..7 of
#   slot1 (interior blocks are mask-free on every core).
# - expert accumulation on the gpsimd (Pool) engine into SBUF, down
#   projections transient in PSUM.
import os
import numpy as np
import ml_dtypes

import concourse.bacc as bacc
import concourse.mybir as mybir
import concourse.tile as tile
from concourse import bass_utils

F32 = mybir.dt.float32
BF16 = mybir.dt.bfloat16
F8 = mybir.dt.float8e4
AF = mybir.ActivationFunctionType
ALU = mybir.AluOpType
DR = mybir.MatmulPerfMode.DoubleRow

B, T, C, H, D = 2, 2048, 1024, 16, 64
R, ROPE, NOPE = 128, 32, 32
E, I = 8, 512
THETA, EPS = 100000.0, 1e-5
P = 128
NCB = C // P             # 8 C blocks
NIB = I // P             # 4 I blocks
TLOC, CHUNK = 512, 256
KB_SLOT = (16, 8)        # key blocks (of 128) attended per chunk slot

WSC = 64.0               # fp8 weight prescale
HSC = 8.0                # fp8 hidden prescale
OSC = 1.0 / (WSC * HSC)  # down-psum rescale

_CACHE = {}


# =============================================================== device IR
def _emit(nc, tc):
    import contextlib

    def din(name, shape, dt):
        return nc.dram_tensor(name, shape, dt, kind="ExternalInput")

    xt_d = din("xt", (P, NCB, T), BF16)
    xloc_d = din("xloc", (P, NCB, TLOC), F32)
    xlbf_d = din("xlbf", (P, NCB, TLOC), BF16)
    wq_d = din("wq", (P, NCB, H * D), BF16)
    wkva_d = din("wkva", (P, NCB, R + ROPE), BF16)
    wkvb_d = din("wkvb", (R, H * NOPE), BF16)
    wo_d = din("wo", (P, 4, C), BF16)
    cosk_d = din("cosk", (ROPE, T), F32)
    ssink_d = din("ssink", (ROPE, T), F32)
    cosq_d = din("cosq", (2 * ROPE, TLOC), F32)
    ssinq_d = din("ssinq", (2 * ROPE, TLOC), F32)
    perm64_d = din("perm64", (2 * ROPE, 2 * ROPE), BF16)
    perm32_d = din("perm32", (ROPE, ROPE), BF16)
    ident_d = din("ident", (P, P), F32)
    kmask_d = din("kmask", (P, 16 * CHUNK), BF16)
    wgate_d = din("wgate", (P, NCB, E), F32)
    biasg_d = din("biasg", (P, E), F32)
    wg_d = din("wg8", (E + 1, P, NCB * I), F8)   # index 0 = shared expert
    wu_d = din("wu8", (E + 1, P, NCB * I), F8)
    wd_d = din("wd8", (E + 1, P, NIB * C), F8)
    out_d = nc.dram_tensor("outT", (P, NCB, TLOC), F32, kind="ExternalOutput")
    DEBUG = bool(int(os.environ.get("BASSK_DEBUG", "0")))
    if DEBUG:
        dxa_d = nc.dram_tensor("d_xa", (P, NCB, TLOC), F32,
                               kind="ExternalOutput")
        dcm_d = nc.dram_tensor("d_comb", (E, TLOC), BF16,
                               kind="ExternalOutput")
        dem_d = nc.dram_tensor("d_em", (P, 16 * CHUNK), BF16,
                               kind="ExternalOutput")
        dac_d = nc.dram_tensor("d_acc", (P, 2, NIB, TLOC), F32,
                               kind="ExternalOutput")

    whole = contextlib.ExitStack()
    early = contextlib.ExitStack()
    attn = contextlib.ExitStack()

    pc = whole.enter_context(tc.tile_pool(name="pc", bufs=1))
    pw = whole.enter_context(tc.tile_pool(name="pw", bufs=1, side="right"))
    pmid = whole.enter_context(tc.tile_pool(name="pmid", bufs=1))

    # pa: tiles written during the early phase but read by attention
    pa = attn.enter_context(tc.tile_pool(name="pa", bufs=1))
    pt2 = attn.enter_context(tc.tile_pool(name="pt2", bufs=2))

    px = early.enter_context(tc.tile_pool(name="px", bufs=1))
    pt1 = early.enter_context(tc.tile_pool(name="pt1", bufs=2))
    pse = early.enter_context(tc.tile_pool(name="pse", bufs=2, space="PSUM"))

    # ---- constants / tables
    ones_128x1 = pc.tile([P, 1], BF16)
    nc.any.memset(ones_128x1[:], 1.0)
    ones1f = pc.tile([1, 1], F32)
    nc.any.memset(ones1f[:], 1.0)
    eps_sb = pc.tile([1, 1], F32)
    nc.any.memset(eps_sb[:], EPS)
    ident_sb = pc.tile([P, P], F32)
    nc.sync.dma_start(ident_sb[:], ident_d.ap())
    perm64_sb = pc.tile([2 * ROPE, 2 * ROPE], BF16)
    nc.sync.dma_start(perm64_sb[:], perm64_d.ap())
    perm32_sb = pc.tile([ROPE, ROPE], BF16)
    nc.sync.dma_start(perm32_sb[:], perm32_d.ap())
    biasg_sb = pc.tile([P, E], F32)
    nc.sync.dma_start(biasg_sb[:], biasg_d.ap())
    wgate_sb = pc.tile([P, NCB, E], F32)
    nc.sync.dma_start(wgate_sb[:], wgate_d.ap())

    cosk_t = px.tile([ROPE, T], F32)
    nc.sync.dma_start(cosk_t[:], cosk_d.ap())
    ssink_t = px.tile([ROPE, T], F32)
    nc.sync.dma_start(ssink_t[:], ssink_d.ap())
    cosq_t = px.tile([2 * ROPE, TLOC], F32)
    nc.sync.dma_start(cosq_t[:], cosq_d.ap())
    ssinq_t = px.tile([2 * ROPE, TLOC], F32)
    nc.sync.dma_start(ssinq_t[:], ssinq_d.ap())

    # ---- bulk loads
    xt = px.tile([P, NCB, T], BF16)
    for cb in range(NCB):
        nc.sync.dma_start(xt[:, cb, :], xt_d.ap()[:, cb, :])
    xlbf = px.tile([P, NCB, TLOC], BF16)
    nc.sync.dma_start(xlbf[:], xlbf_d.ap())
    wkva_sb = px.tile([P, NCB, R + ROPE], BF16)
    nc.sync.dma_start(wkva_sb[:], wkva_d.ap())
    wq_sb = px.tile([P, NCB, H * D], BF16)
    for cb in range(0, NCB, 2):
        nc.sync.dma_start(wq_sb[:, cb:cb + 2, :], wq_d.ap()[:, cb:cb + 2, :])

    # ---- prefetch first MoE expert weights (slots 0..2) before the
    # attention-section DMAs claim the SP queue
    wq_tiles = {}
    for e in (0, 1, 2):
        wgt = pw.tile([P, NCB, I], F8, name="wgt", tag=f"wgt{e % 3}")
        nc.sync.dma_start(wgt[:], wg_d.ap()[e])
        wut = pw.tile([P, NCB, I], F8, name="wut", tag=f"wut{e % 3}")
        nc.sync.dma_start(wut[:], wu_d.ap()[e])
        wdt = pw.tile([P, NIB, C], F8, name="wdt", tag=f"wdt{e % 3}")
        nc.sync.dma_start(wdt[:], wd_d.ap()[e])
        wq_tiles[e] = (wgt, wut, wdt)

    # ---- rmsnorm1 stats: global (keys) then local (queries)
    bc1 = px.tile([P, T], F32)
    for nt in range(T // 512):
        sl = slice(nt * 512, (nt + 1) * 512)
        sps = pse.tile([1, 512], F32, name="sps", tag="accA")
        for cb in range(NCB):
            xq = pt1.tile([P, 512], BF16, name="xq", tag="xq")
            nc.scalar.square(xq[:], xt[:, cb, sl])
            nc.tensor.matmul(sps[:], ones_128x1[:], xq[:],
                             start=(cb == 0), stop=(cb == NCB - 1))
        rr = pt1.tile([1, 512], F32, name="rr", tag="rr", bufs=1)
        nc.scalar.activation(rr[:], sps[:], AF.Sqrt, bias=eps_sb[:],
                             scale=1.0 / C)
        iv = pt1.tile([1, 512], F32, name="iv", tag="iv", bufs=1)
        nc.vector.reciprocal(iv[:], rr[:])
        nc.gpsimd.partition_broadcast(bc1[:, sl], iv[:])

    bc1l = px.tile([P, TLOC], BF16)
    spsl = pse.tile([1, TLOC], F32, name="spsl", tag="accA")
    for cb in range(NCB):
        xql = pt1.tile([P, TLOC], BF16, name="xql", tag="xq")
        nc.scalar.square(xql[:], xlbf[:, cb, :])
        nc.tensor.matmul(spsl[:], ones_128x1[:], xql[:],
                         start=(cb == 0), stop=(cb == NCB - 1))
    rrl = pt1.tile([1, TLOC], F32, name="rrl", tag="rr", bufs=1)
    nc.scalar.activation(rrl[:], spsl[:], AF.Sqrt, bias=eps_sb[:],
                         scale=1.0 / C)
    ivl = pt1.tile([1, TLOC], BF16, name="ivl", tag="ivb", bufs=1)
    with nc.allow_low_precision(reason="rms scale in bf16 (0.4% on q norm)"):
        nc.vector.reciprocal(ivl[:], rrl[:])
    nc.gpsimd.partition_broadcast(bc1l[:], ivl[:])
    # normalize local x in place (bf16 2x): Q projections then need no
    # per-column rescale, so their psum extracts become ACT copies
    for cb in range(NCB):
        nc.vector.tensor_tensor(xlbf[:, cb, :], xlbf[:, cb, :], bc1l[:],
                                ALU.mult)

    # ---- ckv: kv latent (scaled) + scaled k_rope
    kvlat = pa.tile([R, T], BF16)
    krsc = pa.tile([ROPE, T], BF16)     # scaled raw k_rope
    for nt in range(T // 512):
        sl = slice(nt * 512, (nt + 1) * 512)
        lat_ps = pse.tile([P, 512], F32, name="lat_ps", tag="pA")
        for cb in range(NCB):
            nc.tensor.matmul(lat_ps[:], wkva_sb[:, cb, 0:R], xt[:, cb, sl],
                             start=(cb == 0), stop=(cb == NCB - 1))
        rop_ps = pse.tile([ROPE, 512], F32, name="rop_ps", tag="par")
        for cb in range(NCB):
            nc.tensor.matmul(rop_ps[:], wkva_sb[:, cb, R:R + ROPE],
                             xt[:, cb, sl],
                             start=(cb == 0), stop=(cb == NCB - 1))
        nc.vector.tensor_tensor(kvlat[:, sl], lat_ps[:], bc1[:, sl], ALU.mult)
        nc.vector.tensor_tensor(krsc[:, sl], rop_ps[:], bc1[0:ROPE, sl],
                                ALU.mult)

    # ---- rope K -> kropebf [32, T]
    kropebf = pa.tile([ROPE, T], BF16)
    for nt in range(T // 512):
        sl = slice(nt * 512, (nt + 1) * 512)
        park = pse.tile([ROPE, 512], F32, name="park", tag="par")
        nc.tensor.matmul(park[:], perm32_sb[:], krsc[:, sl])
        t1k = pt1.tile([ROPE, 512], F32, name="t1k", tag="t1q")
        nc.gpsimd.tensor_tensor(t1k[:], krsc[:, sl], cosk_t[:, sl], ALU.mult)
        t2k = pt1.tile([ROPE, 512], F32, name="t2k", tag="t2q")
        nc.vector.tensor_tensor(t2k[:], park[:], ssink_t[:, sl], ALU.mult)
        nc.vector.tensor_tensor(kropebf[:, sl], t1k[:], t2k[:], ALU.add)

    # ---- Q projection + rope (whole TLOC per head-pair)
    qbf = []
    for mb in range(8):
        tl = pa.tile([P, TLOC], BF16, name=f"qbf{mb}")
        qps = pse.tile([P, TLOC], F32, name="qps", tag="pA")
        for cb in range(NCB):
            nc.tensor.matmul(qps[:], wq_sb[:, cb, mb * P:(mb + 1) * P],
                             xlbf[:, cb, :],
                             start=(cb == 0), stop=(cb == NCB - 1))
        nc.scalar.copy(tl[:], qps[:])
        qr = pt1.tile([2 * ROPE, TLOC], BF16, name="qr", tag="qr")
        nc.scalar.copy(qr[0:ROPE, :], qps[32:64, :])
        nc.scalar.copy(qr[ROPE:2 * ROPE, :], qps[96:128, :])
        parq = pse.tile([2 * ROPE, TLOC], F32, name="parq", tag="par")
        nc.tensor.matmul(parq[:], perm64_sb[:], qr[:])
        t1q = pt1.tile([2 * ROPE, TLOC], F32, name="t1q", tag="t1q")
        nc.gpsimd.tensor_tensor(t1q[:], qr[:], cosq_t[:], ALU.mult)
        t2q = pt1.tile([2 * ROPE, TLOC], F32, name="t2q", tag="t2q")
        nc.vector.tensor_tensor(t2q[:], parq[:], ssinq_t[:], ALU.mult)
        nc.gpsimd.tensor_tensor(tl[32:64, :], t1q[0:ROPE, :], t2q[0:ROPE, :],
                                ALU.add)
        nc.gpsimd.tensor_tensor(tl[96:128, :], t1q[ROPE:2 * ROPE, :],
                                t2q[ROPE:2 * ROPE, :], ALU.add)
        qbf.append(tl)

    early.close()

    # ---- deferred loads (space freed by the early pools)
    pk = attn.enter_context(tc.tile_pool(name="pk", bufs=1))
    xloc = pk.tile([P, NCB, TLOC], F32)
    nc.sync.dma_start(xloc[:], xloc_d.ap())
    wkvb_sb = pk.tile([R, H * NOPE], BF16)
    nc.sync.dma_start(wkvb_sb[:], wkvb_d.ap())
    kmask_sb = pk.tile([P, 16 * CHUNK], BF16)
    nc.sync.dma_start(kmask_sb[:], kmask_d.ap())
    wo_sb = pk.tile([P, 4, C], BF16)
    nc.sync.dma_start(wo_sb[:], wo_d.ap())

    # ---- k_nope -> kfull assembly; V extended with ones row.
    # rope rows depend only on kropebf: DMA them first; nope rows stream in
    # right after each knope block so scores can start early.
    kfull = []
    for mb in range(H // 2):
        kfull.append(pk.tile([P, T], BF16, name=f"kfull{mb}"))
    for mb in range(H // 2):
        nc.sync.dma_start(kfull[mb][32:64, :], kropebf[:])
        nc.sync.dma_start(kfull[mb][96:128, :], kropebf[:])
    sub = contextlib.ExitStack()
    pkx = sub.enter_context(tc.tile_pool(name="pkx", bufs=1))
    psk = sub.enter_context(tc.tile_pool(name="psk", bufs=2, space="PSUM"))
    for j in range(4):
        tl = pkx.tile([P, T], BF16, name=f"knope{j}")
        for nt in range(T // 512):
            sl = slice(nt * 512, (nt + 1) * 512)
            kps = psk.tile([P, 512], F32, name="kps", tag="pA")
            nc.tensor.matmul(kps[:], wkvb_sb[:, j * P:(j + 1) * P],
                             kvlat[:, sl])
            nc.scalar.copy(tl[:, sl], kps[:])
        for h in (4 * j, 4 * j + 1, 4 * j + 2, 4 * j + 3):
            mb, po = h // 2, (h % 2) * 64
            nc.sync.dma_start(kfull[mb][po:po + 32, :],
                              tl[(h % 4) * 32:(h % 4) * 32 + 32, :])
    vext = []
    for tb in range(16):
        tl = pk.tile([P, H, 34], BF16, name=f"vext{tb}")
        vps = psk.tile([P, H * NOPE], F32, name="vps", tag="pA")
        nc.tensor.matmul(vps[:], kvlat[:, tb * P:(tb + 1) * P], wkvb_sb[:])
        nc.scalar.copy(tl[:, :, 0:NOPE],
                       vps[:].rearrange("p (h d) -> p h d", h=H))
        nc.any.memset(tl[:, :, NOPE:NOPE + 1], 1.0)
        vext.append(tl)
    sub.close()

    # ---- attention core
    core = contextlib.ExitStack()
    psc = core.enter_context(tc.tile_pool(name="psc", bufs=2, space="PSUM"))
    pE = core.enter_context(tc.tile_pool(name="pE", bufs=2))
    yall = []
    for yb in range(4):
        yall.append(pk.tile([P, TLOC], BF16, name=f"yall{yb}"))
    for ch in range(2):
        csl = slice(ch * CHUNK, (ch + 1) * CHUNK)
        nkb = KB_SLOT[ch]
        for h in range(H):
            mb, po = h // 2, (h % 2) * 64
            em = pE.tile([P, nkb * CHUNK], BF16, name="em", tag="em")
            for g in range(nkb // 4):
                sp = psc.tile([P, 4 * CHUNK], F32, name="sp", tag="sc")
                for k4 in range(4):
                    kb = 4 * g + k4
                    nc.tensor.matmul(
                        sp[:, k4 * CHUNK:(k4 + 1) * CHUNK],
                        kfull[mb][po:po + 64, kb * P:(kb + 1) * P],
                        qbf[mb][po:po + 64, csl])
                nc.scalar.activation(em[:, g * 4 * CHUNK:(g + 1) * 4 * CHUNK],
                                     sp[:], AF.Exp, scale=0.125)
            if ch == 0:
                nc.vector.tensor_tensor(em[:, 8 * CHUNK:16 * CHUNK],
                                        em[:, 8 * CHUNK:16 * CHUNK],
                                        kmask_sb[:, 0:8 * CHUNK], ALU.mult)
            else:
                nc.vector.tensor_tensor(em[:], em[:],
                                        kmask_sb[:, 8 * CHUNK:16 * CHUNK],
                                        ALU.mult)
            if DEBUG and ch == 0 and h == 0:
                nc.sync.dma_start(dem_d.ap(), em[:])
            y_ps = psc.tile([NOPE + 1, CHUNK], F32, name="y_ps", tag="yv")
            for kb in range(nkb):
                nc.tensor.matmul(y_ps[:], vext[kb][:, h, 0:NOPE + 1],
                                 em[:, kb * CHUNK:(kb + 1) * CHUNK],
                                 start=(kb == 0), stop=(kb == nkb - 1))
            rr2 = pt2.tile([1, CHUNK], F32, name="rr2", tag="rrA")
            nc.vector.reciprocal(rr2[:], y_ps[NOPE:NOPE + 1, :])
            rb = pt2.tile([NOPE, CHUNK], F32, name="rb", tag="rb")
            nc.gpsimd.partition_broadcast(rb[:], rr2[:])
            yt = yall[h // 4]
            ro = (h % 4) * NOPE
            nc.vector.tensor_tensor(yt[ro:ro + NOPE, csl], y_ps[0:NOPE, :],
                                    rb[:], ALU.mult)
    core.close()

    # ---- Wo + residual -> xa (f32) ; rmsnorm2 ; gate ; comb broadcast
    fin = contextlib.ExitStack()
    psg = fin.enter_context(tc.tile_pool(name="psg", bufs=2, space="PSUM"))
    psh = fin.enter_context(tc.tile_pool(name="psh", bufs=1, space="PSUM"))

    xa = []
    for cb in range(NCB):
        xa.append(pmid.tile([P, TLOC], F32, name=f"xa{cb}"))
    for cb in range(NCB):
        ops = psg.tile([P, TLOC], F32, name="ops", tag="wo")
        for kb in range(4):
            nc.tensor.matmul(ops[:], wo_sb[:, kb, cb * P:(cb + 1) * P],
                             yall[kb][:], start=(kb == 0), stop=(kb == 3))
        nc.vector.scalar_tensor_tensor(xa[cb][:], ops[:], 1.0,
                                       xloc[:, cb, :],
                                       op0=ALU.mult, op1=ALU.add)

    invr2 = pmid.tile([1, TLOC], F32)
    sps2 = psh.tile([1, TLOC], F32, name="sps2", tag="acc2")
    for cb in range(NCB):
        xq2 = pt2.tile([P, TLOC], BF16, name="xq2", tag="xq2")
        nc.vector.tensor_tensor(xq2[:], xa[cb][:], xa[cb][:], ALU.mult)
        nc.tensor.matmul(sps2[:], ones_128x1[:], xq2[:],
                         start=(cb == 0), stop=(cb == NCB - 1))
    rr3 = pt2.tile([1, TLOC], F32, name="rr3", tag="rrA")
    nc.scalar.activation(rr3[:], sps2[:], AF.Sqrt, bias=eps_sb[:],
                         scale=1.0 / C)
    nc.vector.reciprocal(invr2[:], rr3[:])
    bc2 = pt2.tile([P, TLOC], F32, name="bc2", tag="bc2")
    nc.gpsimd.partition_broadcast(bc2[:], invr2[:])

    xmf8 = pmid.tile([P, NCB, TLOC], F8)
    for cb in range(NCB):
        nc.vector.tensor_tensor(xmf8[:, cb, :], xa[cb][:], bc2[:], ALU.mult)
    if DEBUG:
        for cb in range(NCB):
            nc.sync.dma_start(dxa_d.ap()[:, cb, :], xa[cb][:])

    # gate (fp32, from xa scaled by invr2 via transposed column)
    ct_all = pmid.tile([E, TLOC], BF16)
    for tb in range(4):
        tsl = slice(tb * P, (tb + 1) * P)
        g_ps = psh.tile([P, E], F32, name="g_ps", tag="gps")
        for cb in range(NCB):
            nc.tensor.matmul(g_ps[:], xa[cb][:, tsl], wgate_sb[:, cb, :],
                             start=(cb == 0), stop=(cb == NCB - 1))
        ir_ps = psh.tile([P, 1], F32, name="ir_ps", tag="irp")
        nc.tensor.transpose(ir_ps[:], invr2[:, tsl], ones1f[:])
        ir_col = pt2.tile([P, 1], F32, name="ir_col", tag="irc")
        nc.scalar.copy(ir_col[:], ir_ps[:])
        lg = pt2.tile([P, E], F32, name="lg", tag="lg")
        nc.vector.scalar_tensor_tensor(lg[:], g_ps[:], ir_col[:], biasg_sb[:],
                                       op0=ALU.mult, op1=ALU.add)
        m1 = pt2.tile([P, 1], F32, name="m1", tag="m1")
        nc.vector.reduce_max(m1[:], lg[:], axis=mybir.AxisListType.X)
        eq1 = pt2.tile([P, E], F32, name="eq1", tag="eq1")
        nc.vector.tensor_scalar(eq1[:], lg[:], m1[:], None, op0=ALU.is_equal)
        lm = pt2.tile([P, E], F32, name="lm", tag="lm")
        nc.vector.scalar_tensor_tensor(lm[:], eq1[:], -1e9, lg[:],
                                       op0=ALU.mult, op1=ALU.add)
        m2 = pt2.tile([P, 1], F32, name="m2", tag="m2")
        nc.vector.reduce_max(m2[:], lm[:], axis=mybir.AxisListType.X)
        eq2 = pt2.tile([P, E], F32, name="eq2", tag="eq2")
        nc.vector.tensor_scalar(eq2[:], lm[:], m2[:], None, op0=ALU.is_equal)
        dm = pt2.tile([P, 1], F32, name="dm", tag="dm")
        nc.vector.tensor_scalar(dm[:], m1[:], m2[:], None, op0=ALU.subtract)
        w1 = pt2.tile([P, 1], F32, name="w1", tag="w1")
        nc.scalar.activation(w1[:], dm[:], AF.Sigmoid)
        w2 = pt2.tile([P, 1], F32, name="w2", tag="w2")
        nc.vector.tensor_scalar(w2[:], w1[:], -1.0, 1.0, op0=ALU.mult,
                                op1=ALU.add)
        cmb = pt2.tile([P, E], F32, name="cmb", tag="cmb")
        nc.vector.tensor_scalar(cmb[:], eq1[:], w1[:], None, op0=ALU.mult)
        cm2 = pt2.tile([P, E], F32, name="cm2", tag="cm2")
        nc.vector.tensor_scalar(cm2[:], eq2[:], w2[:], None, op0=ALU.mult)
        cmf = pt2.tile([P, E], F32, name="cmf", tag="cmf")
        nc.vector.tensor_tensor(cmf[:], cmb[:], cm2[:], ALU.add)
        ct_ps = psh.tile([E, P], F32, name="ct_ps", tag="ctp")
        nc.tensor.transpose(ct_ps[:], cmf[:], ident_sb[:])
        nc.scalar.copy(ct_all[:, tsl], ct_ps[:])
    if DEBUG:
        nc.sync.dma_start(dcm_d.ap(), ct_all[:])
    bcomb = []
    for e in range(E):
        cte = pt2.tile([1, TLOC], BF16, name="cte", tag="cte")
        nc.sync.dma_start(cte[:], ct_all[e:e + 1, :])
        tl = pmid.tile([P, TLOC], BF16, name=f"bcomb{e}")
        nc.gpsimd.partition_broadcast(tl[:], cte[:])
        bcomb.append(tl)

    fin.close()
    attn.close()

    # ---- MoE: fp8 DoubleRow, quad-of-experts accumulation in PSUM.
    # Quads: [shared], [e1..e4], [e5..e8]. Within a quad the token-half loop
    # is outer so the down psum (4 banks) accumulates all its experts; one
    # flush to SBUF per (quad, half, group). Weights pool holds up to 4
    # experts + 1 prefetch (pw bufs=5).
    moe = contextlib.ExitStack()
    pgu = moe.enter_context(tc.tile_pool(name="pgu", bufs=1, space="PSUM"))
    pwd = moe.enter_context(tc.tile_pool(name="pwd", bufs=1, space="PSUM"))
    pmoe = moe.enter_context(tc.tile_pool(name="pmoe", bufs=2))
    pac2 = moe.enter_context(tc.tile_pool(name="pac2", bufs=1))

    accs = [pac2.tile([P, NIB, TLOC], F32, name=f"acc{g}") for g in range(2)]

    pwm = moe.enter_context(tc.tile_pool(name="pwm", bufs=1, side="right"))
    quads = [[0], [1, 2], [3, 4, 5, 6, 7, 8]]
    for qi, quad in enumerate(quads):
        for e in quad:
            if e in wq_tiles:
                continue
            pl = pw if e < 6 else pwm
            wgt = pl.tile([P, NCB, I], F8, name="wgt", tag=f"wgt{e % 3}")
            nc.sync.dma_start(wgt[:], wg_d.ap()[e])
            wut = pl.tile([P, NCB, I], F8, name="wut", tag=f"wut{e % 3}")
            nc.sync.dma_start(wut[:], wu_d.ap()[e])
            wdt = pl.tile([P, NIB, C], F8, name="wdt", tag=f"wdt{e % 3}")
            nc.sync.dma_start(wdt[:], wd_d.ap()[e])
            wq_tiles[e] = (wgt, wut, wdt)
        for th in range(2):
            hsl = slice(th * CHUNK, (th + 1) * CHUNK)
            dps = [pwd.tile([P, NIB, CHUNK], F32, name=f"dps{g}", tag=f"d{g}")
                   for g in range(2)]
            hps = []
            for ei, e in enumerate(quad):
                wgt, wut, wdt = wq_tiles[e]
                gp = pgu.tile([P, NIB, CHUNK], F32, name="gp", tag="gp")
                for ib in range(NIB):
                    isl = slice(ib * P, (ib + 1) * P)
                    for j in range(4):
                        nc.tensor.matmul(gp[:, ib, :],
                                         wgt[:, 2 * j:2 * j + 2, isl],
                                         xmf8[:, 2 * j:2 * j + 2, hsl],
                                         start=(j == 0), stop=(j == 3),
                                         perf_mode=DR)
                sg = pmoe.tile([P, NIB, CHUNK], BF16, name="sg", tag="sg")
                nc.scalar.activation(sg[:], gp[:], AF.Silu, scale=1.0 / WSC)
                if e > 0:
                    sgc = pmoe.tile([P, NIB, CHUNK], BF16, name="sgc",
                                    tag="sgc")
                    bce = bcomb[e - 1]
                    for ib in range(NIB):
                        nc.vector.tensor_tensor(sgc[:, ib, :], sg[:, ib, :],
                                                bce[:, hsl], ALU.mult)
                else:
                    sgc = sg
                up = pgu.tile([P, NIB, CHUNK], F32, name="up", tag="up")
                for ib in range(NIB):
                    isl = slice(ib * P, (ib + 1) * P)
                    for j in range(4):
                        nc.tensor.matmul(up[:, ib, :],
                                         wut[:, 2 * j:2 * j + 2, isl],
                                         xmf8[:, 2 * j:2 * j + 2, hsl],
                                         start=(j == 0), stop=(j == 3),
                                         perf_mode=DR)
                hp = pmoe.tile([P, NIB, CHUNK], F8, name="hp", tag="hp",
                               bufs=7)
                nc.vector.scalar_tensor_tensor(hp[:], up[:], 1.0 / HSC,
                                               sgc[:],
                                               op0=ALU.mult, op1=ALU.mult)
                hps.append(hp)
            # region-major down: a PSUM region's accumulation group must
            # finish before the next group in the same bank starts
            # (start_tensor_calc clears has_written bank-wide).
            for cb in range(NCB):
                dst = dps[cb // 4][:, cb % 4, :]
                for ei, e in enumerate(quad):
                    wdt = wq_tiles[e][2]
                    for j in range(2):
                        nc.tensor.matmul(dst, wdt[:, 2 * j:2 * j + 2,
                                                  cb * P:(cb + 1) * P],
                                         hps[ei][:, 2 * j:2 * j + 2, :],
                                         start=(ei == 0 and j == 0),
                                         stop=(ei == len(quad) - 1 and j == 1),
                                         perf_mode=DR)
            for g in range(2):
                if qi == 0:
                    nc.scalar.copy(accs[g][:, :, hsl], dps[g][:])
                else:
                    nc.vector.scalar_tensor_tensor(accs[g][:, :, hsl],
                                                   dps[g][:], 1.0,
                                                   accs[g][:, :, hsl],
                                                   op0=ALU.mult, op1=ALU.add)

    if DEBUG:
        for g in range(2):
            nc.sync.dma_start(dac_d.ap()[:, g], accs[g][:])
    # ---- out = acc/512 + xa (per accumulator group, so the first half of
    # the output DMA overlaps the last experts' flushes)
    fo = pac2.tile([P, NCB, TLOC], F32)
    for g in range(2):
        for k in range(NIB):
            cb = 4 * g + k
            nc.vector.scalar_tensor_tensor(fo[:, cb, :], accs[g][:, k, :],
                                           OSC, xa[cb][:],
                                           op0=ALU.mult, op1=ALU.add)
        nc.sync.dma_start(out_d.ap()[:, 4 * g:4 * g + 4, :],
                          fo[:, 4 * g:4 * g + 4, :])

    moe.close()
    whole.close()


# =============================================================== host side
def _build():
    if "nc" in _CACHE:
        return _CACHE["nc"]
    nc = bacc.Bacc("TRN2", target_bir_lowering=False, debug=False,
                   num_devices=8)
    with tile.TileContext(nc) as tc:
        _emit(nc, tc)
    nc.compile()
    _CACHE["nc"] = nc
    return nc


def _rope_tables(pos):
    # pos: (N,) positions; returns cos,ssin of shape (ROPE, N) in the
    # row-pair layout (rows 2i/2i+1 both carry angle pos*freq_i; ssin row 2i
    # is -sin, row 2i+1 is +sin).
    freqs = 1.0 / (THETA ** (np.arange(0, ROPE, 2, dtype=np.float32) / ROPE))
    ang = np.outer(freqs, pos.astype(np.float32))          # (16, N)
    cos = np.repeat(np.cos(ang), 2, axis=0).astype(np.float32)
    sin = np.sin(ang).astype(np.float32)
    ssin = np.empty((ROPE, len(pos)), np.float32)
    ssin[0::2] = -sin
    ssin[1::2] = sin
    return cos, ssin


def _blk(a):
    # (C_like, X) -> (128, nb, X): row cb*128+p -> [p, cb, :]
    nb = a.shape[0] // P
    return np.ascontiguousarray(
        a.reshape(nb, P, -1).transpose(1, 0, 2))


def _f8(a):
    return np.clip(np.asarray(a, np.float32), -240.0, 240.0).astype(
        ml_dtypes.float8_e4m3)


def _host_shared(inputs):
    bf = lambda a: np.ascontiguousarray(a).astype(ml_dtypes.bfloat16)
    f32 = lambda a: np.ascontiguousarray(a, dtype=np.float32)
    w_ln1 = np.asarray(inputs["w_ln1"], np.float32)
    w_ln2 = np.asarray(inputs["w_ln2"], np.float32)
    posk = np.arange(T, dtype=np.float32)
    ck, sk = _rope_tables(posk)
    p32 = np.zeros((ROPE, ROPE), np.float32)
    for i in range(ROPE // 2):
        p32[2 * i + 1, 2 * i] = 1.0
        p32[2 * i, 2 * i + 1] = 1.0
    p64 = np.zeros((2 * ROPE, 2 * ROPE), np.float32)
    p64[:ROPE, :ROPE] = p32
    p64[ROPE:, ROPE:] = p32
    wq = np.asarray(inputs["Wq"], np.float32) * w_ln1[:, None]
    wkva = np.asarray(inputs["Wkva"], np.float32) * w_ln1[:, None]
    wo_nope = np.asarray(inputs["Wo"], np.float32).reshape(H, D, C)[:, :NOPE]
    wgate = np.asarray(inputs["Wgate"], np.float32) * w_ln2[:, None]
    biasg = np.broadcast_to(np.asarray(inputs["expert_bias"], np.float32),
                            (P, E)).copy()
    wg = np.asarray(inputs["Wg"], np.float32) * w_ln2[None, :, None]
    wu = np.asarray(inputs["Wu"], np.float32) * w_ln2[None, :, None]
    wd = np.asarray(inputs["Wd"], np.float32)
    swg = np.asarray(inputs["sWg"], np.float32)[0] * w_ln2[:, None]
    swu = np.asarray(inputs["sWu"], np.float32)[0] * w_ln2[:, None]
    swd = np.asarray(inputs["sWd"], np.float32)[0]
    wg9 = np.concatenate([swg[None], wg], axis=0) * WSC    # (9, C, I)
    wu9 = np.concatenate([swu[None], wu], axis=0) * WSC
    wd9 = np.concatenate([swd[None], wd], axis=0) * WSC    # (9, I, C)
    wg8 = wg9.reshape(E + 1, NCB, P, I).transpose(0, 2, 1, 3).reshape(
        E + 1, P, NCB * I)
    wu8 = wu9.reshape(E + 1, NCB, P, I).transpose(0, 2, 1, 3).reshape(
        E + 1, P, NCB * I)
    wd8 = wd9.reshape(E + 1, NIB, P, C).transpose(0, 2, 1, 3).reshape(
        E + 1, P, NIB * C)
    xT = [np.asarray(inputs["x"], np.float32)[b].T for b in range(B)]
    return {
        "shared": {
            "wq": bf(_blk(wq)),
            "wkva": bf(_blk(wkva)),
            "wkvb": bf(inputs["Wkvb"]),
            "wo": bf(_blk(wo_nope.reshape(H * NOPE, C))),
            "cosk": f32(ck), "ssink": f32(sk),
            "perm64": bf(p64), "perm32": bf(p32),
            "ident": np.eye(P, dtype=np.float32),
            "wgate": f32(_blk(wgate)),
            "biasg": biasg,
            "wg8": _f8(wg8), "wu8": _f8(wu8), "wd8": _f8(wd8),
        },
        "xT": xT,
        "xt_bf": [bf(_blk(xT[b])) for b in range(B)],
    }


def _host_inputs(inputs, core, shared):
    bf = lambda a: np.ascontiguousarray(a).astype(ml_dtypes.bfloat16)
    f32 = lambda a: np.ascontiguousarray(a, dtype=np.float32)
    b, q = core // 4, core % 4
    hi, lo = 7 - q, q           # slot0 = chunk hi, slot1 = chunk lo
    xT = shared["xT"][b]                                   # (C, T)
    loc_cols = np.r_[np.arange(hi * CHUNK, (hi + 1) * CHUNK),
                     np.arange(lo * CHUNK, (lo + 1) * CHUNK)]
    xloc = xT[:, loc_cols]
    posq = loc_cols.astype(np.float32)
    cq, sq = _rope_tables(posq)                            # (32, 512)

    # causal masks: cols 0:2048 slot0 kb 8..15 ; cols 2048:4096 slot1 kb 0..7
    kmask = np.zeros((P, 16 * CHUNK), np.float32)
    ki = np.arange(P)[:, None]
    qi = np.arange(CHUNK)[None, :]
    for half, (j, kbs) in enumerate(((hi, range(8, 16)), (lo, range(0, 8)))):
        for i, kb in enumerate(kbs):
            m = np.zeros((P, CHUNK), np.float32)
            if kb < 2 * j:
                m[:] = 1.0
            elif kb == 2 * j:
                m = (ki <= qi).astype(np.float32)
            elif kb == 2 * j + 1:
                m = (ki + P <= qi).astype(np.float32)
            col = half * 8 * CHUNK + i * CHUNK
            kmask[:, col:col + CHUNK] = m

    m = dict(shared["shared"])
    m.update({
        "xt": shared["xt_bf"][b],
        "xloc": f32(_blk(xloc)),
        "xlbf": bf(_blk(xloc)),
        "cosq": f32(np.vstack([cq, cq])), "ssinq": f32(np.vstack([sq, sq])),
        "kmask": bf(kmask),
    })
    return m


LAST_RESULTS = None


def kernel(**inputs):
    global LAST_RESULTS
    nc = _build()
    shared = _host_shared(inputs)
    in_maps = [_host_inputs(inputs, core, shared) for core in range(8)]
    kw = {}
    if os.environ.get("BASSK_TRACE"):
        kw = dict(trace=True, trace_cores=[0], stitch_traces=False)
    res = bass_utils.run_bass_kernel_spmd(nc, in_maps, core_ids=list(range(8)),
                                          **kw)
    LAST_RESULTS = res
    out = np.empty((B, T, C), np.float32)
    for core in range(8):
        b, q = core // 4, core % 4
        hi, lo = 7 - q, q
        oT = res.results[core]["outT"]                 # (128, NCB, TLOC)
        full = oT.transpose(1, 0, 2).reshape(C, TLOC)  # (C, 512)
        out[b, hi * CHUNK:(hi + 1) * CHUNK] = full[:, :CHUNK].T
        out[b, lo * CHUNK:(lo + 1) * CHUNK] = full[:, CHUNK:].T
    return out


# revision 35
# speedup vs baseline: 16799.3307x; 1.0234x over previous
# DeepSeek block (MLA attention + top-2-of-8 MoE + shared expert) on 8 TRN2
# NeuronCores, zero-collective sharding.
#
# Core c in [0..8): sequence b = c//4, q = c%4; owns token chunks
# hi = 7-q (slot 0) and lo = q (slot 1), 256 tokens each (causally balanced).
# SPMD: identical program on all cores; per-core data (x columns, rope
# tables, causal masks) arrives as inputs.
#
# v2 layout/perf notes:
# - activations feature-on-partition; matmuls bf16 except MoE which runs
#   fp8e4 DoubleRow (both operands packed [128,2,*], contract 256/matmul).
#   MoE weights pre-scaled x64 on host (fp8e4 min normal 2^-6), down input
#   h carries x8; output rescaled by 1/512 at the final accumulate.
# - one DMA per logical matrix (DRAM tensors pre-arranged (128, blk, cols)).
# - wide ACT ops ([128,1024] exp/silu) to amortize the fixed ACT overhead;
#   causal masks applied only to key blocks 8..15 of slot0 and 0..7 of
#   slot1 (interior blocks are mask-free on every core).
# - expert accumulation on the gpsimd (Pool) engine into SBUF, down
#   projections transient in PSUM.
import os
import numpy as np
import ml_dtypes

import concourse.bacc as bacc
import concourse.mybir as mybir
import concourse.tile as tile
from concourse import bass_utils

F32 = mybir.dt.float32
BF16 = mybir.dt.bfloat16
F8 = mybir.dt.float8e4
AF = mybir.ActivationFunctionType
ALU = mybir.AluOpType
DR = mybir.MatmulPerfMode.DoubleRow

B, T, C, H, D = 2, 2048, 1024, 16, 64
R, ROPE, NOPE = 128, 32, 32
E, I = 8, 512
THETA, EPS = 100000.0, 1e-5
P = 128
NCB = C // P             # 8 C blocks
NIB = I // P             # 4 I blocks
TLOC, CHUNK = 512, 256
KB_SLOT = (16, 8)        # key blocks (of 128) attended per chunk slot

WSC = 64.0               # fp8 weight prescale
HSC = 8.0                # fp8 hidden prescale
OSC = 1.0 / (WSC * HSC)  # down-psum rescale

_CACHE = {}


# =============================================================== device IR
def _emit(nc, tc):
    import contextlib

    def din(name, shape, dt):
        return nc.dram_tensor(name, shape, dt, kind="ExternalInput")

    xt_d = din("xt", (P, NCB, T), BF16)
    xloc_d = din("xloc", (P, NCB, TLOC), F32)
    xlbf_d = din("xlbf", (P, NCB, TLOC), BF16)
    wq_d = din("wq", (P, NCB, H * D), BF16)
    wkva_d = din("wkva", (P, NCB, R + ROPE), BF16)
    wkvb_d = din("wkvb", (R, H * NOPE), BF16)
    wo_d = din("wo", (P, 4, C), BF16)
    cosk_d = din("cosk", (ROPE, T), F32)
    ssink_d = din("ssink", (ROPE, T), F32)
    cosq_d = din("cosq", (2 * ROPE, TLOC), F32)
    ssinq_d = din("ssinq", (2 * ROPE, TLOC), F32)
    perm64_d = din("perm64", (2 * ROPE, 2 * ROPE), BF16)
    perm32_d = din("perm32", (ROPE, ROPE), BF16)
    ident_d = din("ident", (P, P), F32)
    kmask_d = din("kmask", (P, 16 * CHUNK), BF16)
    wgate_d = din("wgate", (P, NCB, E), F32)
    biasg_d = din("biasg", (P, E), F32)
    wg_d = din("wg8", (E + 1, P, NCB * I), F8)   # index 0 = shared expert
    wu_d = din("wu8", (E + 1, P, NCB * I), F8)
    wd_d = din("wd8", (E + 1, P, NIB * C), F8)
    out_d = nc.dram_tensor("outT", (P, NCB, TLOC), F32, kind="ExternalOutput")
    DEBUG = bool(int(os.environ.get("BASSK_DEBUG", "0")))
    if DEBUG:
        dxa_d = nc.dram_tensor("d_xa", (P, NCB, TLOC), F32,
                               kind="ExternalOutput")
        dcm_d = nc.dram_tensor("d_comb", (E, TLOC), BF16,
                               kind="ExternalOutput")
        dem_d = nc.dram_tensor("d_em", (P, 16 * CHUNK), BF16,
                               kind="ExternalOutput")
        dac_d = nc.dram_tensor("d_acc", (P, 2, NIB, TLOC), F32,
                               kind="ExternalOutput")

    whole = contextlib.ExitStack()
    early = contextlib.ExitStack()
    attn = contextlib.ExitStack()

    pc = whole.enter_context(tc.tile_pool(name="pc", bufs=1))
    pw = whole.enter_context(tc.tile_pool(name="pw", bufs=1, side="right"))
    pmid = whole.enter_context(tc.tile_pool(name="pmid", bufs=1))

    # pa: tiles written during the early phase but read by attention
    pa = attn.enter_context(tc.tile_pool(name="pa", bufs=1))
    pt2 = attn.enter_context(tc.tile_pool(name="pt2", bufs=2))

    px = early.enter_context(tc.tile_pool(name="px", bufs=1))
    pt1 = early.enter_context(tc.tile_pool(name="pt1", bufs=2))
    pse = early.enter_context(tc.tile_pool(name="pse", bufs=2, space="PSUM"))

    # ---- constants / tables
    ones_128x1 = pc.tile([P, 1], BF16)
    nc.any.memset(ones_128x1[:], 1.0)
    ones1f = pc.tile([1, 1], F32)
    nc.any.memset(ones1f[:], 1.0)
    eps_sb = pc.tile([1, 1], F32)
    nc.any.memset(eps_sb[:], EPS)
    ident_sb = pc.tile([P, P], F32)
    nc.sync.dma_start(ident_sb[:], ident_d.ap())
    perm64_sb = pc.tile([2 * ROPE, 2 * ROPE], BF16)
    nc.sync.dma_start(perm64_sb[:], perm64_d.ap())
    perm32_sb = pc.tile([ROPE, ROPE], BF16)
    nc.sync.dma_start(perm32_sb[:], perm32_d.ap())
    biasg_sb = pc.tile([P, E], F32)
    nc.sync.dma_start(biasg_sb[:], biasg_d.ap())
    wgate_sb = pc.tile([P, NCB, E], F32)
    nc.sync.dma_start(wgate_sb[:], wgate_d.ap())

    cosk_t = px.tile([ROPE, T], F32)
    nc.sync.dma_start(cosk_t[:], cosk_d.ap())
    ssink_t = px.tile([ROPE, T], F32)
    nc.sync.dma_start(ssink_t[:], ssink_d.ap())
    cosq_t = px.tile([2 * ROPE, TLOC], F32)
    nc.sync.dma_start(cosq_t[:], cosq_d.ap())
    ssinq_t = px.tile([2 * ROPE, TLOC], F32)
    nc.sync.dma_start(ssinq_t[:], ssinq_d.ap())

    # ---- bulk loads
    xt = px.tile([P, NCB, T], BF16)
    for cb in range(NCB):
        nc.sync.dma_start(xt[:, cb, :], xt_d.ap()[:, cb, :])
    xlbf = px.tile([P, NCB, TLOC], BF16)
    nc.sync.dma_start(xlbf[:], xlbf_d.ap())
    wkva_sb = px.tile([P, NCB, R + ROPE], BF16)
    nc.sync.dma_start(wkva_sb[:], wkva_d.ap())
    wq_sb = px.tile([P, NCB, H * D], BF16)
    for cb in range(0, NCB, 2):
        nc.sync.dma_start(wq_sb[:, cb:cb + 2, :], wq_d.ap()[:, cb:cb + 2, :])

    # ---- prefetch first MoE expert weights (slots 0..2) before the
    # attention-section DMAs claim the SP queue
    wq_tiles = {}
    for e in (0, 1, 2):
        wgt = pw.tile([P, NCB, I], F8, name="wgt", tag=f"wgt{e % 3}")
        nc.sync.dma_start(wgt[:], wg_d.ap()[e])
        wut = pw.tile([P, NCB, I], F8, name="wut", tag=f"wut{e % 3}")
        nc.sync.dma_start(wut[:], wu_d.ap()[e])
        wdt = pw.tile([P, NIB, C], F8, name="wdt", tag=f"wdt{e % 3}")
        nc.sync.dma_start(wdt[:], wd_d.ap()[e])
        wq_tiles[e] = (wgt, wut, wdt)

    # ---- rmsnorm1 stats: global (keys) then local (queries)
    bc1 = px.tile([P, T], F32)
    for nt in range(T // 512):
        sl = slice(nt * 512, (nt + 1) * 512)
        sps = pse.tile([1, 512], F32, name="sps", tag="accA")
        for cb in range(NCB):
            xq = pt1.tile([P, 512], BF16, name="xq", tag="xq")
            nc.scalar.square(xq[:], xt[:, cb, sl])
            nc.tensor.matmul(sps[:], ones_128x1[:], xq[:],
                             start=(cb == 0), stop=(cb == NCB - 1))
        rr = pt1.tile([1, 512], F32, name="rr", tag="rr", bufs=1)
        nc.scalar.activation(rr[:], sps[:], AF.Sqrt, bias=eps_sb[:],
                             scale=1.0 / C)
        iv = pt1.tile([1, 512], F32, name="iv", tag="iv", bufs=1)
        nc.vector.reciprocal(iv[:], rr[:])
        nc.gpsimd.partition_broadcast(bc1[:, sl], iv[:])

    bc1l = px.tile([P, TLOC], BF16)
    spsl = pse.tile([1, TLOC], F32, name="spsl", tag="accA")
    for cb in range(NCB):
        xql = pt1.tile([P, TLOC], BF16, name="xql", tag="xq")
        nc.scalar.square(xql[:], xlbf[:, cb, :])
        nc.tensor.matmul(spsl[:], ones_128x1[:], xql[:],
                         start=(cb == 0), stop=(cb == NCB - 1))
    rrl = pt1.tile([1, TLOC], F32, name="rrl", tag="rr", bufs=1)
    nc.scalar.activation(rrl[:], spsl[:], AF.Sqrt, bias=eps_sb[:],
                         scale=1.0 / C)
    ivl = pt1.tile([1, TLOC], BF16, name="ivl", tag="ivb", bufs=1)
    with nc.allow_low_precision(reason="rms scale in bf16 (0.4% on q norm)"):
        nc.vector.reciprocal(ivl[:], rrl[:])
    nc.gpsimd.partition_broadcast(bc1l[:], ivl[:])
    # normalize local x in place (bf16 2x): Q projections then need no
    # per-column rescale, so their psum extracts become ACT copies
    for cb in range(NCB):
        nc.vector.tensor_tensor(xlbf[:, cb, :], xlbf[:, cb, :], bc1l[:],
                                ALU.mult)

    # ---- ckv: kv latent (scaled) + scaled k_rope
    kvlat = pa.tile([R, T], BF16)
    krsc = pa.tile([ROPE, T], BF16)     # scaled raw k_rope
    for nt in range(T // 512):
        sl = slice(nt * 512, (nt + 1) * 512)
        lat_ps = pse.tile([P, 512], F32, name="lat_ps", tag="pA")
        for cb in range(NCB):
            nc.tensor.matmul(lat_ps[:], wkva_sb[:, cb, 0:R], xt[:, cb, sl],
                             start=(cb == 0), stop=(cb == NCB - 1))
        rop_ps = pse.tile([ROPE, 512], F32, name="rop_ps", tag="par")
        for cb in range(NCB):
            nc.tensor.matmul(rop_ps[:], wkva_sb[:, cb, R:R + ROPE],
                             xt[:, cb, sl],
                             start=(cb == 0), stop=(cb == NCB - 1))
        nc.vector.tensor_tensor(kvlat[:, sl], lat_ps[:], bc1[:, sl], ALU.mult)
        nc.vector.tensor_tensor(krsc[:, sl], rop_ps[:], bc1[0:ROPE, sl],
                                ALU.mult)

    # ---- rope K -> kropebf [32, T]
    kropebf = pa.tile([ROPE, T], BF16)
    for nt in range(T // 512):
        sl = slice(nt * 512, (nt + 1) * 512)
        park = pse.tile([ROPE, 512], F32, name="park", tag="par")
        nc.tensor.matmul(park[:], perm32_sb[:], krsc[:, sl])
        t1k = pt1.tile([ROPE, 512], F32, name="t1k", tag="t1q")
        nc.gpsimd.tensor_tensor(t1k[:], krsc[:, sl], cosk_t[:, sl], ALU.mult)
        t2k = pt1.tile([ROPE, 512], F32, name="t2k", tag="t2q")
        nc.vector.tensor_tensor(t2k[:], park[:], ssink_t[:, sl], ALU.mult)
        nc.vector.tensor_tensor(kropebf[:, sl], t1k[:], t2k[:], ALU.add)

    # ---- Q projection + rope (whole TLOC per head-pair)
    qbf = []
    for mb in range(8):
        tl = pa.tile([P, TLOC], BF16, name=f"qbf{mb}")
        qps = pse.tile([P, TLOC], F32, name="qps", tag="pA")
        for cb in range(NCB):
            nc.tensor.matmul(qps[:], wq_sb[:, cb, mb * P:(mb + 1) * P],
                             xlbf[:, cb, :],
                             start=(cb == 0), stop=(cb == NCB - 1))
        nc.scalar.copy(tl[:], qps[:])
        qr = pt1.tile([2 * ROPE, TLOC], BF16, name="qr", tag="qr")
        nc.scalar.copy(qr[0:ROPE, :], qps[32:64, :])
        nc.scalar.copy(qr[ROPE:2 * ROPE, :], qps[96:128, :])
        parq = pse.tile([2 * ROPE, TLOC], F32, name="parq", tag="par")
        nc.tensor.matmul(parq[:], perm64_sb[:], qr[:])
        t1q = pt1.tile([2 * ROPE, TLOC], F32, name="t1q", tag="t1q")
        nc.gpsimd.tensor_tensor(t1q[:], qr[:], cosq_t[:], ALU.mult)
        t2q = pt1.tile([2 * ROPE, TLOC], F32, name="t2q", tag="t2q")
        nc.vector.tensor_tensor(t2q[:], parq[:], ssinq_t[:], ALU.mult)
        eng1 = nc.gpsimd if mb % 2 == 0 else nc.vector
        eng2 = nc.vector if mb % 2 == 0 else nc.gpsimd
        eng1.tensor_tensor(tl[32:64, :], t1q[0:ROPE, :], t2q[0:ROPE, :],
                           ALU.add)
        eng2.tensor_tensor(tl[96:128, :], t1q[ROPE:2 * ROPE, :],
                           t2q[ROPE:2 * ROPE, :], ALU.add)
        qbf.append(tl)

    early.close()

    # ---- deferred loads (space freed by the early pools)
    pk = attn.enter_context(tc.tile_pool(name="pk", bufs=1))
    xloc = pk.tile([P, NCB, TLOC], F32)
    nc.sync.dma_start(xloc[:], xloc_d.ap())
    wkvb_sb = pk.tile([R, H * NOPE], BF16)
    nc.sync.dma_start(wkvb_sb[:], wkvb_d.ap())
    kmask_sb = pk.tile([P, 16 * CHUNK], BF16)
    nc.sync.dma_start(kmask_sb[:], kmask_d.ap())
    wo_sb = pk.tile([P, 4, C], BF16)
    nc.sync.dma_start(wo_sb[:], wo_d.ap())

    # ---- k_nope -> kfull assembly; V extended with ones row.
    # rope rows depend only on kropebf: DMA them first; nope rows stream in
    # right after each knope block so scores can start early.
    kfull = []
    for mb in range(H // 2):
        kfull.append(pk.tile([P, T], BF16, name=f"kfull{mb}"))
    for mb in range(H // 2):
        nc.sync.dma_start(kfull[mb][32:64, :], kropebf[:])
        nc.sync.dma_start(kfull[mb][96:128, :], kropebf[:])
    sub = contextlib.ExitStack()
    pkx = sub.enter_context(tc.tile_pool(name="pkx", bufs=1))
    psk = sub.enter_context(tc.tile_pool(name="psk", bufs=2, space="PSUM"))
    for j in range(4):
        tl = pkx.tile([P, T], BF16, name=f"knope{j}")
        for nt in range(T // 512):
            sl = slice(nt * 512, (nt + 1) * 512)
            kps = psk.tile([P, 512], F32, name="kps", tag="pA")
            nc.tensor.matmul(kps[:], wkvb_sb[:, j * P:(j + 1) * P],
                             kvlat[:, sl])
            nc.scalar.copy(tl[:, sl], kps[:])
        for h in (4 * j, 4 * j + 1, 4 * j + 2, 4 * j + 3):
            mb, po = h // 2, (h % 2) * 64
            nc.sync.dma_start(kfull[mb][po:po + 32, :],
                              tl[(h % 4) * 32:(h % 4) * 32 + 32, :])
    vext = []
    for tb in range(16):
        tl = pk.tile([P, H, 34], BF16, name=f"vext{tb}")
        vps = psk.tile([P, H * NOPE], F32, name="vps", tag="pA")
        nc.tensor.matmul(vps[:], kvlat[:, tb * P:(tb + 1) * P], wkvb_sb[:])
        nc.scalar.copy(tl[:, :, 0:NOPE],
                       vps[:].rearrange("p (h d) -> p h d", h=H))
        nc.any.memset(tl[:, :, NOPE:NOPE + 1], 1.0)
        vext.append(tl)
    sub.close()

    # ---- attention core
    core = contextlib.ExitStack()
    psc = core.enter_context(tc.tile_pool(name="psc", bufs=2, space="PSUM"))
    pE = core.enter_context(tc.tile_pool(name="pE", bufs=2))
    yall = []
    for yb in range(4):
        yall.append(pk.tile([P, TLOC], BF16, name=f"yall{yb}"))
    for ch in range(2):
        csl = slice(ch * CHUNK, (ch + 1) * CHUNK)
        nkb = KB_SLOT[ch]
        for h in range(H):
            mb, po = h // 2, (h % 2) * 64
            em = pE.tile([P, nkb * CHUNK], BF16, name="em", tag="em")
            for g in range(nkb // 4):
                sp = psc.tile([P, 4 * CHUNK], F32, name="sp", tag="sc")
                for k4 in range(4):
                    kb = 4 * g + k4
                    nc.tensor.matmul(
                        sp[:, k4 * CHUNK:(k4 + 1) * CHUNK],
                        kfull[mb][po:po + 64, kb * P:(kb + 1) * P],
                        qbf[mb][po:po + 64, csl])
                nc.scalar.activation(em[:, g * 4 * CHUNK:(g + 1) * 4 * CHUNK],
                                     sp[:], AF.Exp, scale=0.125)
            if ch == 0:
                nc.vector.tensor_tensor(em[:, 8 * CHUNK:16 * CHUNK],
                                        em[:, 8 * CHUNK:16 * CHUNK],
                                        kmask_sb[:, 0:8 * CHUNK], ALU.mult)
            else:
                nc.vector.tensor_tensor(em[:], em[:],
                                        kmask_sb[:, 8 * CHUNK:16 * CHUNK],
                                        ALU.mult)
            if DEBUG and ch == 0 and h == 0:
                nc.sync.dma_start(dem_d.ap(), em[:])
            y_ps = psc.tile([NOPE + 1, CHUNK], F32, name="y_ps", tag="yv")
            for kb in range(nkb):
                nc.tensor.matmul(y_ps[:], vext[kb][:, h, 0:NOPE + 1],
                                 em[:, kb * CHUNK:(kb + 1) * CHUNK],
                                 start=(kb == 0), stop=(kb == nkb - 1))
            rr2 = pt2.tile([1, CHUNK], F32, name="rr2", tag="rrA")
            nc.vector.reciprocal(rr2[:], y_ps[NOPE:NOPE + 1, :])
            rb = pt2.tile([NOPE, CHUNK], F32, name="rb", tag="rb")
            nc.gpsimd.partition_broadcast(rb[:], rr2[:])
            yt = yall[h // 4]
            ro = (h % 4) * NOPE
            nc.vector.tensor_tensor(yt[ro:ro + NOPE, csl], y_ps[0:NOPE, :],
                                    rb[:], ALU.mult)
    core.close()

    # ---- Wo + residual -> xa (f32) ; rmsnorm2 ; gate ; comb broadcast
    fin = contextlib.ExitStack()
    psg = fin.enter_context(tc.tile_pool(name="psg", bufs=2, space="PSUM"))
    psh = fin.enter_context(tc.tile_pool(name="psh", bufs=1, space="PSUM"))

    xa = []
    for cb in range(NCB):
        xa.append(pmid.tile([P, TLOC], F32, name=f"xa{cb}"))
    for cb in range(NCB):
        ops = psg.tile([P, TLOC], F32, name="ops", tag="wo")
        for kb in range(4):
            nc.tensor.matmul(ops[:], wo_sb[:, kb, cb * P:(cb + 1) * P],
                             yall[kb][:], start=(kb == 0), stop=(kb == 3))
        nc.vector.scalar_tensor_tensor(xa[cb][:], ops[:], 1.0,
                                       xloc[:, cb, :],
                                       op0=ALU.mult, op1=ALU.add)

    invr2 = pmid.tile([1, TLOC], F32)
    sps2 = psh.tile([1, TLOC], F32, name="sps2", tag="acc2")
    for cb in range(NCB):
        xq2 = pt2.tile([P, TLOC], BF16, name="xq2", tag="xq2")
        nc.vector.tensor_tensor(xq2[:], xa[cb][:], xa[cb][:], ALU.mult)
        nc.tensor.matmul(sps2[:], ones_128x1[:], xq2[:],
                         start=(cb == 0), stop=(cb == NCB - 1))
    rr3 = pt2.tile([1, TLOC], F32, name="rr3", tag="rrA")
    nc.scalar.activation(rr3[:], sps2[:], AF.Sqrt, bias=eps_sb[:],
                         scale=1.0 / C)
    nc.vector.reciprocal(invr2[:], rr3[:])
    bc2 = pt2.tile([P, TLOC], F32, name="bc2", tag="bc2")
    nc.gpsimd.partition_broadcast(bc2[:], invr2[:])

    xmf8 = pmid.tile([P, NCB, TLOC], F8)
    for cb in range(NCB):
        nc.vector.tensor_tensor(xmf8[:, cb, :], xa[cb][:], bc2[:], ALU.mult)
    if DEBUG:
        for cb in range(NCB):
            nc.sync.dma_start(dxa_d.ap()[:, cb, :], xa[cb][:])

    # gate (fp32, from xa scaled by invr2 via transposed column)
    ct_all = pmid.tile([E, TLOC], BF16)
    for tb in range(4):
        tsl = slice(tb * P, (tb + 1) * P)
        g_ps = psh.tile([P, E], F32, name="g_ps", tag="gps")
        for cb in range(NCB):
            nc.tensor.matmul(g_ps[:], xa[cb][:, tsl], wgate_sb[:, cb, :],
                             start=(cb == 0), stop=(cb == NCB - 1))
        ir_ps = psh.tile([P, 1], F32, name="ir_ps", tag="irp")
        nc.tensor.transpose(ir_ps[:], invr2[:, tsl], ones1f[:])
        ir_col = pt2.tile([P, 1], F32, name="ir_col", tag="irc")
        nc.scalar.copy(ir_col[:], ir_ps[:])
        lg = pt2.tile([P, E], F32, name="lg", tag="lg")
        nc.vector.scalar_tensor_tensor(lg[:], g_ps[:], ir_col[:], biasg_sb[:],
                                       op0=ALU.mult, op1=ALU.add)
        m1 = pt2.tile([P, 1], F32, name="m1", tag="m1")
        nc.vector.reduce_max(m1[:], lg[:], axis=mybir.AxisListType.X)
        eq1 = pt2.tile([P, E], F32, name="eq1", tag="eq1")
        nc.vector.tensor_scalar(eq1[:], lg[:], m1[:], None, op0=ALU.is_equal)
        lm = pt2.tile([P, E], F32, name="lm", tag="lm")
        nc.vector.scalar_tensor_tensor(lm[:], eq1[:], -1e9, lg[:],
                                       op0=ALU.mult, op1=ALU.add)
        m2 = pt2.tile([P, 1], F32, name="m2", tag="m2")
        nc.vector.reduce_max(m2[:], lm[:], axis=mybir.AxisListType.X)
        eq2 = pt2.tile([P, E], F32, name="eq2", tag="eq2")
        nc.vector.tensor_scalar(eq2[:], lm[:], m2[:], None, op0=ALU.is_equal)
        dm = pt2.tile([P, 1], F32, name="dm", tag="dm")
        nc.vector.tensor_scalar(dm[:], m1[:], m2[:], None, op0=ALU.subtract)
        w1 = pt2.tile([P, 1], F32, name="w1", tag="w1")
        nc.scalar.activation(w1[:], dm[:], AF.Sigmoid)
        w2 = pt2.tile([P, 1], F32, name="w2", tag="w2")
        nc.vector.tensor_scalar(w2[:], w1[:], -1.0, 1.0, op0=ALU.mult,
                                op1=ALU.add)
        cmb = pt2.tile([P, E], F32, name="cmb", tag="cmb")
        nc.vector.tensor_scalar(cmb[:], eq1[:], w1[:], None, op0=ALU.mult)
        cm2 = pt2.tile([P, E], F32, name="cm2", tag="cm2")
        nc.vector.tensor_scalar(cm2[:], eq2[:], w2[:], None, op0=ALU.mult)
        cmf = pt2.tile([P, E], F32, name="cmf", tag="cmf")
        nc.vector.tensor_tensor(cmf[:], cmb[:], cm2[:], ALU.add)
        ct_ps = psh.tile([E, P], F32, name="ct_ps", tag="ctp")
        nc.tensor.transpose(ct_ps[:], cmf[:], ident_sb[:])
        nc.scalar.copy(ct_all[:, tsl], ct_ps[:])
    if DEBUG:
        nc.sync.dma_start(dcm_d.ap(), ct_all[:])
    bcomb = []
    for e in range(E):
        cte = pt2.tile([1, TLOC], BF16, name="cte", tag="cte")
        nc.sync.dma_start(cte[:], ct_all[e:e + 1, :])
        tl = pmid.tile([P, TLOC], BF16, name=f"bcomb{e}")
        nc.gpsimd.partition_broadcast(tl[:], cte[:])
        bcomb.append(tl)

    fin.close()
    attn.close()

    # ---- MoE: fp8 DoubleRow, quad-of-experts accumulation in PSUM.
    # Quads: [shared], [e1..e4], [e5..e8]. Within a quad the token-half loop
    # is outer so the down psum (4 banks) accumulates all its experts; one
    # flush to SBUF per (quad, half, group). Weights pool holds up to 4
    # experts + 1 prefetch (pw bufs=5).
    moe = contextlib.ExitStack()
    pgu = moe.enter_context(tc.tile_pool(name="pgu", bufs=1, space="PSUM"))
    pwd = moe.enter_context(tc.tile_pool(name="pwd", bufs=1, space="PSUM"))
    pmoe = moe.enter_context(tc.tile_pool(name="pmoe", bufs=2))
    pac2 = moe.enter_context(tc.tile_pool(name="pac2", bufs=1))

    accs = [pac2.tile([P, NIB, TLOC], F32, name=f"acc{g}") for g in range(2)]

    pwm = moe.enter_context(tc.tile_pool(name="pwm", bufs=1, side="right"))
    quads = [[0], [1, 2], [3, 4, 5, 6, 7, 8]]
    for qi, quad in enumerate(quads):
        for e in quad:
            if e in wq_tiles:
                continue
            pl = pw if e < 6 else pwm
            wgt = pl.tile([P, NCB, I], F8, name="wgt", tag=f"wgt{e % 3}")
            nc.sync.dma_start(wgt[:], wg_d.ap()[e])
            wut = pl.tile([P, NCB, I], F8, name="wut", tag=f"wut{e % 3}")
            nc.sync.dma_start(wut[:], wu_d.ap()[e])
            wdt = pl.tile([P, NIB, C], F8, name="wdt", tag=f"wdt{e % 3}")
            nc.sync.dma_start(wdt[:], wd_d.ap()[e])
            wq_tiles[e] = (wgt, wut, wdt)
        for th in range(2):
            hsl = slice(th * CHUNK, (th + 1) * CHUNK)
            dps = [pwd.tile([P, NIB, CHUNK], F32, name=f"dps{g}", tag=f"d{g}")
                   for g in range(2)]
            hps = []
            for ei, e in enumerate(quad):
                wgt, wut, wdt = wq_tiles[e]
                gp = pgu.tile([P, NIB, CHUNK], F32, name="gp", tag="gp")
                for ib in range(NIB):
                    isl = slice(ib * P, (ib + 1) * P)
                    for j in range(4):
                        nc.tensor.matmul(gp[:, ib, :],
                                         wgt[:, 2 * j:2 * j + 2, isl],
                                         xmf8[:, 2 * j:2 * j + 2, hsl],
                                         start=(j == 0), stop=(j == 3),
                                         perf_mode=DR)
                sg = pmoe.tile([P, NIB, CHUNK], BF16, name="sg", tag="sg")
                nc.scalar.activation(sg[:], gp[:], AF.Silu, scale=1.0 / WSC)
                if e > 0:
                    sgc = pmoe.tile([P, NIB, CHUNK], BF16, name="sgc",
                                    tag="sgc")
                    bce = bcomb[e - 1]
                    for ib in range(NIB):
                        nc.vector.tensor_tensor(sgc[:, ib, :], sg[:, ib, :],
                                                bce[:, hsl], ALU.mult)
                else:
                    sgc = sg
                up = pgu.tile([P, NIB, CHUNK], F32, name="up", tag="up")
                for ib in range(NIB):
                    isl = slice(ib * P, (ib + 1) * P)
                    for j in range(4):
                        nc.tensor.matmul(up[:, ib, :],
                                         wut[:, 2 * j:2 * j + 2, isl],
                                         xmf8[:, 2 * j:2 * j + 2, hsl],
                                         start=(j == 0), stop=(j == 3),
                                         perf_mode=DR)
                hp = pmoe.tile([P, NIB, CHUNK], F8, name="hp", tag="hp",
                               bufs=7)
                nc.vector.scalar_tensor_tensor(hp[:], up[:], 1.0 / HSC,
                                               sgc[:],
                                               op0=ALU.mult, op1=ALU.mult)
                hps.append(hp)
            # region-major down: a PSUM region's accumulation group must
            # finish before the next group in the same bank starts
            # (start_tensor_calc clears has_written bank-wide).
            for cb in range(NCB):
                dst = dps[cb // 4][:, cb % 4, :]
                for ei, e in enumerate(quad):
                    wdt = wq_tiles[e][2]
                    for j in range(2):
                        nc.tensor.matmul(dst, wdt[:, 2 * j:2 * j + 2,
                                                  cb * P:(cb + 1) * P],
                                         hps[ei][:, 2 * j:2 * j + 2, :],
                                         start=(ei == 0 and j == 0),
                                         stop=(ei == len(quad) - 1 and j == 1),
                                         perf_mode=DR)
            for g in range(2):
                if qi == 0:
                    nc.scalar.copy(accs[g][:, :, hsl], dps[g][:])
                else:
                    nc.vector.scalar_tensor_tensor(accs[g][:, :, hsl],
                                                   dps[g][:], 1.0,
                                                   accs[g][:, :, hsl],
                                                   op0=ALU.mult, op1=ALU.add)

    if DEBUG:
        for g in range(2):
            nc.sync.dma_start(dac_d.ap()[:, g], accs[g][:])
    # ---- out = acc/512 + xa (per accumulator group, so the first half of
    # the output DMA overlaps the last experts' flushes)
    fo = pac2.tile([P, NCB, TLOC], F32)
    for g in range(2):
        for k in range(NIB):
            cb = 4 * g + k
            nc.vector.scalar_tensor_tensor(fo[:, cb, :], accs[g][:, k, :],
                                           OSC, xa[cb][:],
                                           op0=ALU.mult, op1=ALU.add)
        nc.sync.dma_start(out_d.ap()[:, 4 * g:4 * g + 4, :],
                          fo[:, 4 * g:4 * g + 4, :])

    moe.close()
    whole.close()


# =============================================================== host side
def _build():
    if "nc" in _CACHE:
        return _CACHE["nc"]
    nc = bacc.Bacc("TRN2", target_bir_lowering=False, debug=False,
                   num_devices=8)
    with tile.TileContext(nc) as tc:
        _emit(nc, tc)
    nc.compile()
    _CACHE["nc"] = nc
    return nc


def _rope_tables(pos):
    # pos: (N,) positions; returns cos,ssin of shape (ROPE, N) in the
    # row-pair layout (rows 2i/2i+1 both carry angle pos*freq_i; ssin row 2i
    # is -sin, row 2i+1 is +sin).
    freqs = 1.0 / (THETA ** (np.arange(0, ROPE, 2, dtype=np.float32) / ROPE))
    ang = np.outer(freqs, pos.astype(np.float32))          # (16, N)
    cos = np.repeat(np.cos(ang), 2, axis=0).astype(np.float32)
    sin = np.sin(ang).astype(np.float32)
    ssin = np.empty((ROPE, len(pos)), np.float32)
    ssin[0::2] = -sin
    ssin[1::2] = sin
    return cos, ssin


def _blk(a):
    # (C_like, X) -> (128, nb, X): row cb*128+p -> [p, cb, :]
    nb = a.shape[0] // P
    return np.ascontiguousarray(
        a.reshape(nb, P, -1).transpose(1, 0, 2))


def _f8(a):
    return np.clip(np.asarray(a, np.float32), -240.0, 240.0).astype(
        ml_dtypes.float8_e4m3)


def _host_shared(inputs):
    bf = lambda a: np.ascontiguousarray(a).astype(ml_dtypes.bfloat16)
    f32 = lambda a: np.ascontiguousarray(a, dtype=np.float32)
    w_ln1 = np.asarray(inputs["w_ln1"], np.float32)
    w_ln2 = np.asarray(inputs["w_ln2"], np.float32)
    posk = np.arange(T, dtype=np.float32)
    ck, sk = _rope_tables(posk)
    p32 = np.zeros((ROPE, ROPE), np.float32)
    for i in range(ROPE // 2):
        p32[2 * i + 1, 2 * i] = 1.0
        p32[2 * i, 2 * i + 1] = 1.0
    p64 = np.zeros((2 * ROPE, 2 * ROPE), np.float32)
    p64[:ROPE, :ROPE] = p32
    p64[ROPE:, ROPE:] = p32
    wq = np.asarray(inputs["Wq"], np.float32) * w_ln1[:, None]
    wkva = np.asarray(inputs["Wkva"], np.float32) * w_ln1[:, None]
    wo_nope = np.asarray(inputs["Wo"], np.float32).reshape(H, D, C)[:, :NOPE]
    wgate = np.asarray(inputs["Wgate"], np.float32) * w_ln2[:, None]
    biasg = np.broadcast_to(np.asarray(inputs["expert_bias"], np.float32),
                            (P, E)).copy()
    wg = np.asarray(inputs["Wg"], np.float32) * w_ln2[None, :, None]
    wu = np.asarray(inputs["Wu"], np.float32) * w_ln2[None, :, None]
    wd = np.asarray(inputs["Wd"], np.float32)
    swg = np.asarray(inputs["sWg"], np.float32)[0] * w_ln2[:, None]
    swu = np.asarray(inputs["sWu"], np.float32)[0] * w_ln2[:, None]
    swd = np.asarray(inputs["sWd"], np.float32)[0]
    wg9 = np.concatenate([swg[None], wg], axis=0) * WSC    # (9, C, I)
    wu9 = np.concatenate([swu[None], wu], axis=0) * WSC
    wd9 = np.concatenate([swd[None], wd], axis=0) * WSC    # (9, I, C)
    wg8 = wg9.reshape(E + 1, NCB, P, I).transpose(0, 2, 1, 3).reshape(
        E + 1, P, NCB * I)
    wu8 = wu9.reshape(E + 1, NCB, P, I).transpose(0, 2, 1, 3).reshape(
        E + 1, P, NCB * I)
    wd8 = wd9.reshape(E + 1, NIB, P, C).transpose(0, 2, 1, 3).reshape(
        E + 1, P, NIB * C)
    xT = [np.asarray(inputs["x"], np.float32)[b].T for b in range(B)]
    return {
        "shared": {
            "wq": bf(_blk(wq)),
            "wkva": bf(_blk(wkva)),
            "wkvb": bf(inputs["Wkvb"]),
            "wo": bf(_blk(wo_nope.reshape(H * NOPE, C))),
            "cosk": f32(ck), "ssink": f32(sk),
            "perm64": bf(p64), "perm32": bf(p32),
            "ident": np.eye(P, dtype=np.float32),
            "wgate": f32(_blk(wgate)),
            "biasg": biasg,
            "wg8": _f8(wg8), "wu8": _f8(wu8), "wd8": _f8(wd8),
        },
        "xT": xT,
        "xt_bf": [bf(_blk(xT[b])) for b in range(B)],
    }


def _host_inputs(inputs, core, shared):
    bf = lambda a: np.ascontiguousarray(a).astype(ml_dtypes.bfloat16)
    f32 = lambda a: np.ascontiguousarray(a, dtype=np.float32)
    b, q = core // 4, core % 4
    hi, lo = 7 - q, q           # slot0 = chunk hi, slot1 = chunk lo
    xT = shared["xT"][b]                                   # (C, T)
    loc_cols = np.r_[np.arange(hi * CHUNK, (hi + 1) * CHUNK),
                     np.arange(lo * CHUNK, (lo + 1) * CHUNK)]
    xloc = xT[:, loc_cols]
    posq = loc_cols.astype(np.float32)
    cq, sq = _rope_tables(posq)                            # (32, 512)

    # causal masks: cols 0:2048 slot0 kb 8..15 ; cols 2048:4096 slot1 kb 0..7
    kmask = np.zeros((P, 16 * CHUNK), np.float32)
    ki = np.arange(P)[:, None]
    qi = np.arange(CHUNK)[None, :]
    for half, (j, kbs) in enumerate(((hi, range(8, 16)), (lo, range(0, 8)))):
        for i, kb in enumerate(kbs):
            m = np.zeros((P, CHUNK), np.float32)
            if kb < 2 * j:
                m[:] = 1.0
            elif kb == 2 * j:
                m = (ki <= qi).astype(np.float32)
            elif kb == 2 * j + 1:
                m = (ki + P <= qi).astype(np.float32)
            col = half * 8 * CHUNK + i * CHUNK
            kmask[:, col:col + CHUNK] = m

    m = dict(shared["shared"])
    m.update({
        "xt": shared["xt_bf"][b],
        "xloc": f32(_blk(xloc)),
        "xlbf": bf(_blk(xloc)),
        "cosq": f32(np.vstack([cq, cq])), "ssinq": f32(np.vstack([sq, sq])),
        "kmask": bf(kmask),
    })
    return m


LAST_RESULTS = None


def kernel(**inputs):
    global LAST_RESULTS
    nc = _build()
    shared = _host_shared(inputs)
    in_maps = [_host_inputs(inputs, core, shared) for core in range(8)]
    kw = {}
    if os.environ.get("BASSK_TRACE"):
        kw = dict(trace=True, trace_cores=[0], stitch_traces=False)
    res = bass_utils.run_bass_kernel_spmd(nc, in_maps, core_ids=list(range(8)),
                                          **kw)
    LAST_RESULTS = res
    out = np.empty((B, T, C), np.float32)
    for core in range(8):
        b, q = core // 4, core % 4
        hi, lo = 7 - q, q
        oT = res.results[core]["outT"]                 # (128, NCB, TLOC)
        full = oT.transpose(1, 0, 2).reshape(C, TLOC)  # (C, 512)
        out[b, hi * CHUNK:(hi + 1) * CHUNK] = full[:, :CHUNK].T
        out[b, lo * CHUNK:(lo + 1) * CHUNK] = full[:, CHUNK:].T
    return out


# revision 36
# speedup vs baseline: 16832.2094x; 1.0020x over previous
# DeepSeek block (MLA attention + top-2-of-8 MoE + shared expert) on 8 TRN2
# NeuronCores, zero-collective sharding.
#
# Core c in [0..8): sequence b = c//4, q = c%4; owns token chunks
# hi = 7-q (slot 0) and lo = q (slot 1), 256 tokens each (causally balanced).
# SPMD: identical program on all cores; per-core data (x columns, rope
# tables, causal masks) arrives as inputs.
#
# v2 layout/perf notes:
# - activations feature-on-partition; matmuls bf16 except MoE which runs
#   fp8e4 DoubleRow (both operands packed [128,2,*], contract 256/matmul).
#   MoE weights pre-scaled x64 on host (fp8e4 min normal 2^-6), down input
#   h carries x8; output rescaled by 1/512 at the final accumulate.
# - one DMA per logical matrix (DRAM tensors pre-arranged (128, blk, cols)).
# - wide ACT ops ([128,1024] exp/silu) to amortize the fixed ACT overhead;
#   causal masks applied only to key blocks 8..15 of slot0 and 0..7 of
#   slot1 (interior blocks are mask-free on every core).
# - expert accumulation on the gpsimd (Pool) engine into SBUF, down
#   projections transient in PSUM.
import os
import numpy as np
import ml_dtypes

import concourse.bacc as bacc
import concourse.mybir as mybir
import concourse.tile as tile
from concourse import bass_utils

F32 = mybir.dt.float32
BF16 = mybir.dt.bfloat16
F8 = mybir.dt.float8e4
AF = mybir.ActivationFunctionType
ALU = mybir.AluOpType
DR = mybir.MatmulPerfMode.DoubleRow

B, T, C, H, D = 2, 2048, 1024, 16, 64
R, ROPE, NOPE = 128, 32, 32
E, I = 8, 512
THETA, EPS = 100000.0, 1e-5
P = 128
NCB = C // P             # 8 C blocks
NIB = I // P             # 4 I blocks
TLOC, CHUNK = 512, 256
KB_SLOT = (16, 8)        # key blocks (of 128) attended per chunk slot

WSC = 64.0               # fp8 weight prescale
HSC = 8.0                # fp8 hidden prescale
OSC = 1.0 / (WSC * HSC)  # down-psum rescale

_CACHE = {}


# =============================================================== device IR
def _emit(nc, tc):
    import contextlib

    def din(name, shape, dt):
        return nc.dram_tensor(name, shape, dt, kind="ExternalInput")

    xt_d = din("xt", (P, NCB, T), BF16)
    xloc_d = din("xloc", (P, NCB, TLOC), F32)
    xlbf_d = din("xlbf", (P, NCB, TLOC), BF16)
    wq_d = din("wq", (P, NCB, H * D), BF16)
    wkva_d = din("wkva", (P, NCB, R + ROPE), BF16)
    wkvb_d = din("wkvb", (R, H * NOPE), BF16)
    wo_d = din("wo", (P, 4, C), BF16)
    cosk_d = din("cosk", (ROPE, T), F32)
    ssink_d = din("ssink", (ROPE, T), F32)
    cosq_d = din("cosq", (2 * ROPE, TLOC), F32)
    ssinq_d = din("ssinq", (2 * ROPE, TLOC), F32)
    perm64_d = din("perm64", (2 * ROPE, 2 * ROPE), BF16)
    perm32_d = din("perm32", (ROPE, ROPE), BF16)
    ident_d = din("ident", (P, P), F32)
    kmask_d = din("kmask", (P, 16 * CHUNK), BF16)
    wgate_d = din("wgate", (P, NCB, E), F32)
    biasg_d = din("biasg", (P, E), F32)
    wg_d = din("wg8", (E + 1, P, NCB * I), F8)   # index 0 = shared expert
    wu_d = din("wu8", (E + 1, P, NCB * I), F8)
    wd_d = din("wd8", (E + 1, P, NIB * C), F8)
    out_d = nc.dram_tensor("outT", (P, NCB, TLOC), F32, kind="ExternalOutput")
    DEBUG = bool(int(os.environ.get("BASSK_DEBUG", "0")))
    if DEBUG:
        dxa_d = nc.dram_tensor("d_xa", (P, NCB, TLOC), F32,
                               kind="ExternalOutput")
        dcm_d = nc.dram_tensor("d_comb", (E, TLOC), BF16,
                               kind="ExternalOutput")
        dem_d = nc.dram_tensor("d_em", (P, 16 * CHUNK), BF16,
                               kind="ExternalOutput")
        dac_d = nc.dram_tensor("d_acc", (P, 2, NIB, TLOC), F32,
                               kind="ExternalOutput")

    whole = contextlib.ExitStack()
    early = contextlib.ExitStack()
    attn = contextlib.ExitStack()

    pc = whole.enter_context(tc.tile_pool(name="pc", bufs=1))
    pw = whole.enter_context(tc.tile_pool(name="pw", bufs=1, side="right"))
    pmid = whole.enter_context(tc.tile_pool(name="pmid", bufs=1))

    # pa: tiles written during the early phase but read by attention
    pa = attn.enter_context(tc.tile_pool(name="pa", bufs=1))
    pt2 = attn.enter_context(tc.tile_pool(name="pt2", bufs=2))

    px = early.enter_context(tc.tile_pool(name="px", bufs=1))
    pt1 = early.enter_context(tc.tile_pool(name="pt1", bufs=2))
    pse = early.enter_context(tc.tile_pool(name="pse", bufs=2, space="PSUM"))

    # ---- constants / tables
    ones_128x1 = pc.tile([P, 1], BF16)
    nc.any.memset(ones_128x1[:], 1.0)
    ones1f = pc.tile([1, 1], F32)
    nc.any.memset(ones1f[:], 1.0)
    eps_sb = pc.tile([1, 1], F32)
    nc.any.memset(eps_sb[:], EPS)
    ident_sb = pc.tile([P, P], F32)
    nc.sync.dma_start(ident_sb[:], ident_d.ap())
    perm64_sb = pc.tile([2 * ROPE, 2 * ROPE], BF16)
    nc.sync.dma_start(perm64_sb[:], perm64_d.ap())
    perm32_sb = pc.tile([ROPE, ROPE], BF16)
    nc.sync.dma_start(perm32_sb[:], perm32_d.ap())
    biasg_sb = pc.tile([P, E], F32)
    nc.sync.dma_start(biasg_sb[:], biasg_d.ap())
    wgate_sb = pc.tile([P, NCB, E], F32)
    nc.sync.dma_start(wgate_sb[:], wgate_d.ap())

    cosk_t = px.tile([ROPE, T], F32)
    nc.sync.dma_start(cosk_t[:], cosk_d.ap())
    ssink_t = px.tile([ROPE, T], F32)
    nc.sync.dma_start(ssink_t[:], ssink_d.ap())
    cosq_t = px.tile([2 * ROPE, TLOC], F32)
    nc.sync.dma_start(cosq_t[:], cosq_d.ap())
    ssinq_t = px.tile([2 * ROPE, TLOC], F32)
    nc.sync.dma_start(ssinq_t[:], ssinq_d.ap())

    # ---- bulk loads
    xt = px.tile([P, NCB, T], BF16)
    for cb in range(NCB):
        nc.sync.dma_start(xt[:, cb, :], xt_d.ap()[:, cb, :])
    xlbf = px.tile([P, NCB, TLOC], BF16)
    nc.sync.dma_start(xlbf[:], xlbf_d.ap())
    wkva_sb = px.tile([P, NCB, R + ROPE], BF16)
    nc.sync.dma_start(wkva_sb[:], wkva_d.ap())
    wq_sb = px.tile([P, NCB, H * D], BF16)
    for cb in range(0, NCB, 2):
        nc.sync.dma_start(wq_sb[:, cb:cb + 2, :], wq_d.ap()[:, cb:cb + 2, :])

    # ---- prefetch first MoE expert weights (slots 0..2) before the
    # attention-section DMAs claim the SP queue
    wq_tiles = {}
    for e in (0, 1, 2):
        wgt = pw.tile([P, NCB, I], F8, name="wgt", tag=f"wgt{e % 3}")
        nc.sync.dma_start(wgt[:], wg_d.ap()[e])
        wut = pw.tile([P, NCB, I], F8, name="wut", tag=f"wut{e % 3}")
        nc.sync.dma_start(wut[:], wu_d.ap()[e])
        wdt = pw.tile([P, NIB, C], F8, name="wdt", tag=f"wdt{e % 3}")
        nc.sync.dma_start(wdt[:], wd_d.ap()[e])
        wq_tiles[e] = (wgt, wut, wdt)

    # ---- rmsnorm1 stats: global (keys) then local (queries)
    bc1 = px.tile([P, T], F32)
    for nt in range(T // 512):
        sl = slice(nt * 512, (nt + 1) * 512)
        sps = pse.tile([1, 512], F32, name="sps", tag="accA")
        for cb in range(NCB):
            xq = pt1.tile([P, 512], BF16, name="xq", tag="xq")
            nc.scalar.square(xq[:], xt[:, cb, sl])
            nc.tensor.matmul(sps[:], ones_128x1[:], xq[:],
                             start=(cb == 0), stop=(cb == NCB - 1))
        rr = pt1.tile([1, 512], F32, name="rr", tag="rr", bufs=1)
        nc.scalar.activation(rr[:], sps[:], AF.Sqrt, bias=eps_sb[:],
                             scale=1.0 / C)
        iv = pt1.tile([1, 512], F32, name="iv", tag="iv", bufs=1)
        nc.vector.reciprocal(iv[:], rr[:])
        nc.gpsimd.partition_broadcast(bc1[:, sl], iv[:])

    bc1l = px.tile([P, TLOC], BF16)
    spsl = pse.tile([1, TLOC], F32, name="spsl", tag="accA")
    for cb in range(NCB):
        xql = pt1.tile([P, TLOC], BF16, name="xql", tag="xq")
        nc.scalar.square(xql[:], xlbf[:, cb, :])
        nc.tensor.matmul(spsl[:], ones_128x1[:], xql[:],
                         start=(cb == 0), stop=(cb == NCB - 1))
    rrl = pt1.tile([1, TLOC], F32, name="rrl", tag="rr", bufs=1)
    nc.scalar.activation(rrl[:], spsl[:], AF.Sqrt, bias=eps_sb[:],
                         scale=1.0 / C)
    ivl = pt1.tile([1, TLOC], BF16, name="ivl", tag="ivb", bufs=1)
    with nc.allow_low_precision(reason="rms scale in bf16 (0.4% on q norm)"):
        nc.vector.reciprocal(ivl[:], rrl[:])
    nc.gpsimd.partition_broadcast(bc1l[:], ivl[:])
    # normalize local x in place (bf16 2x): Q projections then need no
    # per-column rescale, so their psum extracts become ACT copies
    for cb in range(NCB):
        nc.vector.tensor_tensor(xlbf[:, cb, :], xlbf[:, cb, :], bc1l[:],
                                ALU.mult)

    # ---- ckv: kv latent (scaled) + scaled k_rope
    kvlat = pa.tile([R, T], BF16)
    krsc = pa.tile([ROPE, T], BF16)     # scaled raw k_rope
    for nt in range(T // 512):
        sl = slice(nt * 512, (nt + 1) * 512)
        lat_ps = pse.tile([P, 512], F32, name="lat_ps", tag="pA")
        for cb in range(NCB):
            nc.tensor.matmul(lat_ps[:], wkva_sb[:, cb, 0:R], xt[:, cb, sl],
                             start=(cb == 0), stop=(cb == NCB - 1))
        rop_ps = pse.tile([ROPE, 512], F32, name="rop_ps", tag="par")
        for cb in range(NCB):
            nc.tensor.matmul(rop_ps[:], wkva_sb[:, cb, R:R + ROPE],
                             xt[:, cb, sl],
                             start=(cb == 0), stop=(cb == NCB - 1))
        nc.vector.tensor_tensor(kvlat[:, sl], lat_ps[:], bc1[:, sl], ALU.mult)
        nc.vector.tensor_tensor(krsc[:, sl], rop_ps[:], bc1[0:ROPE, sl],
                                ALU.mult)

    # ---- rope K -> kropebf [32, T]
    kropebf = pa.tile([ROPE, T], BF16)
    for nt in range(T // 512):
        sl = slice(nt * 512, (nt + 1) * 512)
        park = pse.tile([ROPE, 512], F32, name="park", tag="par")
        nc.tensor.matmul(park[:], perm32_sb[:], krsc[:, sl])
        t1k = pt1.tile([ROPE, 512], F32, name="t1k", tag="t1q")
        nc.gpsimd.tensor_tensor(t1k[:], krsc[:, sl], cosk_t[:, sl], ALU.mult)
        t2k = pt1.tile([ROPE, 512], F32, name="t2k", tag="t2q")
        nc.vector.tensor_tensor(t2k[:], park[:], ssink_t[:, sl], ALU.mult)
        nc.vector.tensor_tensor(kropebf[:, sl], t1k[:], t2k[:], ALU.add)

    # ---- Q projection + rope (whole TLOC per head-pair)
    qbf = []
    for mb in range(8):
        tl = pa.tile([P, TLOC], BF16, name=f"qbf{mb}")
        qps = pse.tile([P, TLOC], F32, name="qps", tag="pA")
        for cb in range(NCB):
            nc.tensor.matmul(qps[:], wq_sb[:, cb, mb * P:(mb + 1) * P],
                             xlbf[:, cb, :],
                             start=(cb == 0), stop=(cb == NCB - 1))
        nc.scalar.copy(tl[:], qps[:])
        qr = pt1.tile([2 * ROPE, TLOC], BF16, name="qr", tag="qr")
        nc.scalar.copy(qr[0:ROPE, :], qps[32:64, :])
        nc.scalar.copy(qr[ROPE:2 * ROPE, :], qps[96:128, :])
        parq = pse.tile([2 * ROPE, TLOC], F32, name="parq", tag="par")
        nc.tensor.matmul(parq[:], perm64_sb[:], qr[:])
        t1q = pt1.tile([2 * ROPE, TLOC], F32, name="t1q", tag="t1q")
        nc.gpsimd.tensor_tensor(t1q[:], qr[:], cosq_t[:], ALU.mult)
        t2q = pt1.tile([2 * ROPE, TLOC], F32, name="t2q", tag="t2q")
        nc.vector.tensor_tensor(t2q[:], parq[:], ssinq_t[:], ALU.mult)
        eng1 = nc.gpsimd if mb % 2 == 0 else nc.vector
        eng2 = nc.vector if mb % 2 == 0 else nc.gpsimd
        eng1.tensor_tensor(tl[32:64, :], t1q[0:ROPE, :], t2q[0:ROPE, :],
                           ALU.add)
        eng2.tensor_tensor(tl[96:128, :], t1q[ROPE:2 * ROPE, :],
                           t2q[ROPE:2 * ROPE, :], ALU.add)
        qbf.append(tl)

    early.close()

    # ---- deferred loads (space freed by the early pools)
    pk = attn.enter_context(tc.tile_pool(name="pk", bufs=1))
    xloc = pk.tile([P, NCB, TLOC], F32)
    nc.sync.dma_start(xloc[:], xloc_d.ap())
    wkvb_sb = pk.tile([R, H * NOPE], BF16)
    nc.sync.dma_start(wkvb_sb[:], wkvb_d.ap())
    kmask_sb = pk.tile([P, 16 * CHUNK], BF16)
    nc.sync.dma_start(kmask_sb[:], kmask_d.ap())
    wo_sb = pk.tile([P, 4, C], BF16)
    nc.sync.dma_start(wo_sb[:], wo_d.ap())

    # ---- k_nope -> kfull assembly; V extended with ones row.
    # rope rows depend only on kropebf: DMA them first; nope rows stream in
    # right after each knope block so scores can start early.
    kfull = []
    for mb in range(H // 2):
        kfull.append(pk.tile([P, T], BF16, name=f"kfull{mb}"))
    for mb in range(H // 2):
        nc.sync.dma_start(kfull[mb][32:64, :], kropebf[:])
        nc.sync.dma_start(kfull[mb][96:128, :], kropebf[:])
    sub = contextlib.ExitStack()
    pkx = sub.enter_context(tc.tile_pool(name="pkx", bufs=1))
    psk = sub.enter_context(tc.tile_pool(name="psk", bufs=2, space="PSUM"))
    for j in range(4):
        tl = pkx.tile([P, T], BF16, name=f"knope{j}")
        for nt in range(T // 512):
            sl = slice(nt * 512, (nt + 1) * 512)
            kps = psk.tile([P, 512], F32, name="kps", tag="pA")
            nc.tensor.matmul(kps[:], wkvb_sb[:, j * P:(j + 1) * P],
                             kvlat[:, sl])
            nc.vector.tensor_scalar(tl[:, sl], kps[:], 1.0, None,
                                    op0=ALU.mult)
        for h in (4 * j, 4 * j + 1, 4 * j + 2, 4 * j + 3):
            mb, po = h // 2, (h % 2) * 64
            nc.sync.dma_start(kfull[mb][po:po + 32, :],
                              tl[(h % 4) * 32:(h % 4) * 32 + 32, :])
    vext = []
    for tb in range(16):
        tl = pk.tile([P, H, 34], BF16, name=f"vext{tb}")
        vps = psk.tile([P, H * NOPE], F32, name="vps", tag="pA")
        nc.tensor.matmul(vps[:], kvlat[:, tb * P:(tb + 1) * P], wkvb_sb[:])
        nc.vector.tensor_scalar(tl[:, :, 0:NOPE],
                                vps[:].rearrange("p (h d) -> p h d", h=H),
                                1.0, None, op0=ALU.mult)
        nc.any.memset(tl[:, :, NOPE:NOPE + 1], 1.0)
        vext.append(tl)
    sub.close()

    # ---- attention core
    core = contextlib.ExitStack()
    psc = core.enter_context(tc.tile_pool(name="psc", bufs=2, space="PSUM"))
    pE = core.enter_context(tc.tile_pool(name="pE", bufs=2))
    yall = []
    for yb in range(4):
        yall.append(pk.tile([P, TLOC], BF16, name=f"yall{yb}"))
    for ch in range(2):
        csl = slice(ch * CHUNK, (ch + 1) * CHUNK)
        nkb = KB_SLOT[ch]
        for h in range(H):
            mb, po = h // 2, (h % 2) * 64
            em = pE.tile([P, nkb * CHUNK], BF16, name="em", tag="em")
            for g in range(nkb // 4):
                sp = psc.tile([P, 4 * CHUNK], F32, name="sp", tag="sc")
                for k4 in range(4):
                    kb = 4 * g + k4
                    nc.tensor.matmul(
                        sp[:, k4 * CHUNK:(k4 + 1) * CHUNK],
                        kfull[mb][po:po + 64, kb * P:(kb + 1) * P],
                        qbf[mb][po:po + 64, csl])
                nc.scalar.activation(em[:, g * 4 * CHUNK:(g + 1) * 4 * CHUNK],
                                     sp[:], AF.Exp, scale=0.125)
            if ch == 0:
                nc.vector.tensor_tensor(em[:, 8 * CHUNK:16 * CHUNK],
                                        em[:, 8 * CHUNK:16 * CHUNK],
                                        kmask_sb[:, 0:8 * CHUNK], ALU.mult)
            else:
                nc.vector.tensor_tensor(em[:], em[:],
                                        kmask_sb[:, 8 * CHUNK:16 * CHUNK],
                                        ALU.mult)
            if DEBUG and ch == 0 and h == 0:
                nc.sync.dma_start(dem_d.ap(), em[:])
            y_ps = psc.tile([NOPE + 1, CHUNK], F32, name="y_ps", tag="yv")
            for kb in range(nkb):
                nc.tensor.matmul(y_ps[:], vext[kb][:, h, 0:NOPE + 1],
                                 em[:, kb * CHUNK:(kb + 1) * CHUNK],
                                 start=(kb == 0), stop=(kb == nkb - 1))
            rr2 = pt2.tile([1, CHUNK], F32, name="rr2", tag="rrA")
            nc.vector.reciprocal(rr2[:], y_ps[NOPE:NOPE + 1, :])
            rb = pt2.tile([NOPE, CHUNK], F32, name="rb", tag="rb")
            nc.gpsimd.partition_broadcast(rb[:], rr2[:])
            yt = yall[h // 4]
            ro = (h % 4) * NOPE
            nc.vector.tensor_tensor(yt[ro:ro + NOPE, csl], y_ps[0:NOPE, :],
                                    rb[:], ALU.mult)
    core.close()

    # ---- Wo + residual -> xa (f32) ; rmsnorm2 ; gate ; comb broadcast
    fin = contextlib.ExitStack()
    psg = fin.enter_context(tc.tile_pool(name="psg", bufs=2, space="PSUM"))
    psh = fin.enter_context(tc.tile_pool(name="psh", bufs=1, space="PSUM"))

    xa = []
    for cb in range(NCB):
        xa.append(pmid.tile([P, TLOC], F32, name=f"xa{cb}"))
    for cb in range(NCB):
        ops = psg.tile([P, TLOC], F32, name="ops", tag="wo")
        for kb in range(4):
            nc.tensor.matmul(ops[:], wo_sb[:, kb, cb * P:(cb + 1) * P],
                             yall[kb][:], start=(kb == 0), stop=(kb == 3))
        nc.vector.scalar_tensor_tensor(xa[cb][:], ops[:], 1.0,
                                       xloc[:, cb, :],
                                       op0=ALU.mult, op1=ALU.add)

    invr2 = pmid.tile([1, TLOC], F32)
    sps2 = psh.tile([1, TLOC], F32, name="sps2", tag="acc2")
    for cb in range(NCB):
        xq2 = pt2.tile([P, TLOC], BF16, name="xq2", tag="xq2")
        nc.vector.tensor_tensor(xq2[:], xa[cb][:], xa[cb][:], ALU.mult)
        nc.tensor.matmul(sps2[:], ones_128x1[:], xq2[:],
                         start=(cb == 0), stop=(cb == NCB - 1))
    rr3 = pt2.tile([1, TLOC], F32, name="rr3", tag="rrA")
    nc.scalar.activation(rr3[:], sps2[:], AF.Sqrt, bias=eps_sb[:],
                         scale=1.0 / C)
    nc.vector.reciprocal(invr2[:], rr3[:])
    bc2 = pt2.tile([P, TLOC], F32, name="bc2", tag="bc2")
    nc.gpsimd.partition_broadcast(bc2[:], invr2[:])

    xmf8 = pmid.tile([P, NCB, TLOC], F8)
    for cb in range(NCB):
        nc.vector.tensor_tensor(xmf8[:, cb, :], xa[cb][:], bc2[:], ALU.mult)
    if DEBUG:
        for cb in range(NCB):
            nc.sync.dma_start(dxa_d.ap()[:, cb, :], xa[cb][:])

    # gate (fp32, from xa scaled by invr2 via transposed column)
    ct_all = pmid.tile([E, TLOC], BF16)
    for tb in range(4):
        tsl = slice(tb * P, (tb + 1) * P)
        g_ps = psh.tile([P, E], F32, name="g_ps", tag="gps")
        for cb in range(NCB):
            nc.tensor.matmul(g_ps[:], xa[cb][:, tsl], wgate_sb[:, cb, :],
                             start=(cb == 0), stop=(cb == NCB - 1))
        ir_ps = psh.tile([P, 1], F32, name="ir_ps", tag="irp")
        nc.tensor.transpose(ir_ps[:], invr2[:, tsl], ones1f[:])
        ir_col = pt2.tile([P, 1], F32, name="ir_col", tag="irc")
        nc.scalar.copy(ir_col[:], ir_ps[:])
        lg = pt2.tile([P, E], F32, name="lg", tag="lg")
        nc.vector.scalar_tensor_tensor(lg[:], g_ps[:], ir_col[:], biasg_sb[:],
                                       op0=ALU.mult, op1=ALU.add)
        m1 = pt2.tile([P, 1], F32, name="m1", tag="m1")
        nc.vector.reduce_max(m1[:], lg[:], axis=mybir.AxisListType.X)
        eq1 = pt2.tile([P, E], F32, name="eq1", tag="eq1")
        nc.vector.tensor_scalar(eq1[:], lg[:], m1[:], None, op0=ALU.is_equal)
        lm = pt2.tile([P, E], F32, name="lm", tag="lm")
        nc.vector.scalar_tensor_tensor(lm[:], eq1[:], -1e9, lg[:],
                                       op0=ALU.mult, op1=ALU.add)
        m2 = pt2.tile([P, 1], F32, name="m2", tag="m2")
        nc.vector.reduce_max(m2[:], lm[:], axis=mybir.AxisListType.X)
        eq2 = pt2.tile([P, E], F32, name="eq2", tag="eq2")
        nc.vector.tensor_scalar(eq2[:], lm[:], m2[:], None, op0=ALU.is_equal)
        dm = pt2.tile([P, 1], F32, name="dm", tag="dm")
        nc.vector.tensor_scalar(dm[:], m1[:], m2[:], None, op0=ALU.subtract)
        w1 = pt2.tile([P, 1], F32, name="w1", tag="w1")
        nc.scalar.activation(w1[:], dm[:], AF.Sigmoid)
        w2 = pt2.tile([P, 1], F32, name="w2", tag="w2")
        nc.vector.tensor_scalar(w2[:], w1[:], -1.0, 1.0, op0=ALU.mult,
                                op1=ALU.add)
        cmb = pt2.tile([P, E], F32, name="cmb", tag="cmb")
        nc.vector.tensor_scalar(cmb[:], eq1[:], w1[:], None, op0=ALU.mult)
        cm2 = pt2.tile([P, E], F32, name="cm2", tag="cm2")
        nc.vector.tensor_scalar(cm2[:], eq2[:], w2[:], None, op0=ALU.mult)
        cmf = pt2.tile([P, E], F32, name="cmf", tag="cmf")
        nc.vector.tensor_tensor(cmf[:], cmb[:], cm2[:], ALU.add)
        ct_ps = psh.tile([E, P], F32, name="ct_ps", tag="ctp")
        nc.tensor.transpose(ct_ps[:], cmf[:], ident_sb[:])
        nc.scalar.copy(ct_all[:, tsl], ct_ps[:])
    if DEBUG:
        nc.sync.dma_start(dcm_d.ap(), ct_all[:])
    bcomb = []
    for e in range(E):
        cte = pt2.tile([1, TLOC], BF16, name="cte", tag="cte")
        nc.sync.dma_start(cte[:], ct_all[e:e + 1, :])
        tl = pmid.tile([P, TLOC], BF16, name=f"bcomb{e}")
        nc.gpsimd.partition_broadcast(tl[:], cte[:])
        bcomb.append(tl)

    fin.close()
    attn.close()

    # ---- MoE: fp8 DoubleRow, quad-of-experts accumulation in PSUM.
    # Quads: [shared], [e1..e4], [e5..e8]. Within a quad the token-half loop
    # is outer so the down psum (4 banks) accumulates all its experts; one
    # flush to SBUF per (quad, half, group). Weights pool holds up to 4
    # experts + 1 prefetch (pw bufs=5).
    moe = contextlib.ExitStack()
    pgu = moe.enter_context(tc.tile_pool(name="pgu", bufs=1, space="PSUM"))
    pwd = moe.enter_context(tc.tile_pool(name="pwd", bufs=1, space="PSUM"))
    pmoe = moe.enter_context(tc.tile_pool(name="pmoe", bufs=2))
    pac2 = moe.enter_context(tc.tile_pool(name="pac2", bufs=1))

    accs = [pac2.tile([P, NIB, TLOC], F32, name=f"acc{g}") for g in range(2)]

    pwm = moe.enter_context(tc.tile_pool(name="pwm", bufs=1, side="right"))
    quads = [[0], [1, 2], [3, 4, 5, 6, 7, 8]]
    for qi, quad in enumerate(quads):
        for e in quad:
            if e in wq_tiles:
                continue
            pl = pw if e < 6 else pwm
            wgt = pl.tile([P, NCB, I], F8, name="wgt", tag=f"wgt{e % 3}")
            nc.sync.dma_start(wgt[:], wg_d.ap()[e])
            wut = pl.tile([P, NCB, I], F8, name="wut", tag=f"wut{e % 3}")
            nc.sync.dma_start(wut[:], wu_d.ap()[e])
            wdt = pl.tile([P, NIB, C], F8, name="wdt", tag=f"wdt{e % 3}")
            nc.sync.dma_start(wdt[:], wd_d.ap()[e])
            wq_tiles[e] = (wgt, wut, wdt)
        for th in range(2):
            hsl = slice(th * CHUNK, (th + 1) * CHUNK)
            dps = [pwd.tile([P, NIB, CHUNK], F32, name=f"dps{g}", tag=f"d{g}")
                   for g in range(2)]
            hps = []
            for ei, e in enumerate(quad):
                wgt, wut, wdt = wq_tiles[e]
                gp = pgu.tile([P, NIB, CHUNK], F32, name="gp", tag="gp")
                for ib in range(NIB):
                    isl = slice(ib * P, (ib + 1) * P)
                    for j in range(4):
                        nc.tensor.matmul(gp[:, ib, :],
                                         wgt[:, 2 * j:2 * j + 2, isl],
                                         xmf8[:, 2 * j:2 * j + 2, hsl],
                                         start=(j == 0), stop=(j == 3),
                                         perf_mode=DR)
                sg = pmoe.tile([P, NIB, CHUNK], BF16, name="sg", tag="sg")
                nc.scalar.activation(sg[:], gp[:], AF.Silu, scale=1.0 / WSC)
                if e > 0:
                    sgc = pmoe.tile([P, NIB, CHUNK], BF16, name="sgc",
                                    tag="sgc")
                    bce = bcomb[e - 1]
                    for ib in range(NIB):
                        nc.vector.tensor_tensor(sgc[:, ib, :], sg[:, ib, :],
                                                bce[:, hsl], ALU.mult)
                else:
                    sgc = sg
                up = pgu.tile([P, NIB, CHUNK], F32, name="up", tag="up")
                for ib in range(NIB):
                    isl = slice(ib * P, (ib + 1) * P)
                    for j in range(4):
                        nc.tensor.matmul(up[:, ib, :],
                                         wut[:, 2 * j:2 * j + 2, isl],
                                         xmf8[:, 2 * j:2 * j + 2, hsl],
                                         start=(j == 0), stop=(j == 3),
                                         perf_mode=DR)
                hp = pmoe.tile([P, NIB, CHUNK], F8, name="hp", tag="hp",
                               bufs=7)
                nc.vector.scalar_tensor_tensor(hp[:], up[:], 1.0 / HSC,
                                               sgc[:],
                                               op0=ALU.mult, op1=ALU.mult)
                hps.append(hp)
            # region-major down: a PSUM region's accumulation group must
            # finish before the next group in the same bank starts
            # (start_tensor_calc clears has_written bank-wide).
            for cb in range(NCB):
                dst = dps[cb // 4][:, cb % 4, :]
                for ei, e in enumerate(quad):
                    wdt = wq_tiles[e][2]
                    for j in range(2):
                        nc.tensor.matmul(dst, wdt[:, 2 * j:2 * j + 2,
                                                  cb * P:(cb + 1) * P],
                                         hps[ei][:, 2 * j:2 * j + 2, :],
                                         start=(ei == 0 and j == 0),
                                         stop=(ei == len(quad) - 1 and j == 1),
                                         perf_mode=DR)
            for g in range(2):
                if qi == 0:
                    nc.scalar.copy(accs[g][:, :, hsl], dps[g][:])
                else:
                    nc.vector.scalar_tensor_tensor(accs[g][:, :, hsl],
                                                   dps[g][:], 1.0,
                                                   accs[g][:, :, hsl],
                                                   op0=ALU.mult, op1=ALU.add)

    if DEBUG:
        for g in range(2):
            nc.sync.dma_start(dac_d.ap()[:, g], accs[g][:])
    # ---- out = acc/512 + xa (per accumulator group, so the first half of
    # the output DMA overlaps the last experts' flushes)
    fo = pac2.tile([P, NCB, TLOC], F32)
    for g in range(2):
        for k in range(NIB):
            cb = 4 * g + k
            nc.vector.scalar_tensor_tensor(fo[:, cb, :], accs[g][:, k, :],
                                           OSC, xa[cb][:],
                                           op0=ALU.mult, op1=ALU.add)
        nc.sync.dma_start(out_d.ap()[:, 4 * g:4 * g + 4, :],
                          fo[:, 4 * g:4 * g + 4, :])

    moe.close()
    whole.close()


# =============================================================== host side
def _build():
    if "nc" in _CACHE:
        return _CACHE["nc"]
    nc = bacc.Bacc("TRN2", target_bir_lowering=False, debug=False,
                   num_devices=8)
    with tile.TileContext(nc) as tc:
        _emit(nc, tc)
    nc.compile()
    _CACHE["nc"] = nc
    return nc


def _rope_tables(pos):
    # pos: (N,) positions; returns cos,ssin of shape (ROPE, N) in the
    # row-pair layout (rows 2i/2i+1 both carry angle pos*freq_i; ssin row 2i
    # is -sin, row 2i+1 is +sin).
    freqs = 1.0 / (THETA ** (np.arange(0, ROPE, 2, dtype=np.float32) / ROPE))
    ang = np.outer(freqs, pos.astype(np.float32))          # (16, N)
    cos = np.repeat(np.cos(ang), 2, axis=0).astype(np.float32)
    sin = np.sin(ang).astype(np.float32)
    ssin = np.empty((ROPE, len(pos)), np.float32)
    ssin[0::2] = -sin
    ssin[1::2] = sin
    return cos, ssin


def _blk(a):
    # (C_like, X) -> (128, nb, X): row cb*128+p -> [p, cb, :]
    nb = a.shape[0] // P
    return np.ascontiguousarray(
        a.reshape(nb, P, -1).transpose(1, 0, 2))


def _f8(a):
    return np.clip(np.asarray(a, np.float32), -240.0, 240.0).astype(
        ml_dtypes.float8_e4m3)


def _host_shared(inputs):
    bf = lambda a: np.ascontiguousarray(a).astype(ml_dtypes.bfloat16)
    f32 = lambda a: np.ascontiguousarray(a, dtype=np.float32)
    w_ln1 = np.asarray(inputs["w_ln1"], np.float32)
    w_ln2 = np.asarray(inputs["w_ln2"], np.float32)
    posk = np.arange(T, dtype=np.float32)
    ck, sk = _rope_tables(posk)
    p32 = np.zeros((ROPE, ROPE), np.float32)
    for i in range(ROPE // 2):
        p32[2 * i + 1, 2 * i] = 1.0
        p32[2 * i, 2 * i + 1] = 1.0
    p64 = np.zeros((2 * ROPE, 2 * ROPE), np.float32)
    p64[:ROPE, :ROPE] = p32
    p64[ROPE:, ROPE:] = p32
    wq = np.asarray(inputs["Wq"], np.float32) * w_ln1[:, None]
    wkva = np.asarray(inputs["Wkva"], np.float32) * w_ln1[:, None]
    wo_nope = np.asarray(inputs["Wo"], np.float32).reshape(H, D, C)[:, :NOPE]
    wgate = np.asarray(inputs["Wgate"], np.float32) * w_ln2[:, None]
    biasg = np.broadcast_to(np.asarray(inputs["expert_bias"], np.float32),
                            (P, E)).copy()
    wg = np.asarray(inputs["Wg"], np.float32) * w_ln2[None, :, None]
    wu = np.asarray(inputs["Wu"], np.float32) * w_ln2[None, :, None]
    wd = np.asarray(inputs["Wd"], np.float32)
    swg = np.asarray(inputs["sWg"], np.float32)[0] * w_ln2[:, None]
    swu = np.asarray(inputs["sWu"], np.float32)[0] * w_ln2[:, None]
    swd = np.asarray(inputs["sWd"], np.float32)[0]
    wg9 = np.concatenate([swg[None], wg], axis=0) * WSC    # (9, C, I)
    wu9 = np.concatenate([swu[None], wu], axis=0) * WSC
    wd9 = np.concatenate([swd[None], wd], axis=0) * WSC    # (9, I, C)
    wg8 = wg9.reshape(E + 1, NCB, P, I).transpose(0, 2, 1, 3).reshape(
        E + 1, P, NCB * I)
    wu8 = wu9.reshape(E + 1, NCB, P, I).transpose(0, 2, 1, 3).reshape(
        E + 1, P, NCB * I)
    wd8 = wd9.reshape(E + 1, NIB, P, C).transpose(0, 2, 1, 3).reshape(
        E + 1, P, NIB * C)
    xT = [np.asarray(inputs["x"], np.float32)[b].T for b in range(B)]
    return {
        "shared": {
            "wq": bf(_blk(wq)),
            "wkva": bf(_blk(wkva)),
            "wkvb": bf(inputs["Wkvb"]),
            "wo": bf(_blk(wo_nope.reshape(H * NOPE, C))),
            "cosk": f32(ck), "ssink": f32(sk),
            "perm64": bf(p64), "perm32": bf(p32),
            "ident": np.eye(P, dtype=np.float32),
            "wgate": f32(_blk(wgate)),
            "biasg": biasg,
            "wg8": _f8(wg8), "wu8": _f8(wu8), "wd8": _f8(wd8),
        },
        "xT": xT,
        "xt_bf": [bf(_blk(xT[b])) for b in range(B)],
    }


def _host_inputs(inputs, core, shared):
    bf = lambda a: np.ascontiguousarray(a).astype(ml_dtypes.bfloat16)
    f32 = lambda a: np.ascontiguousarray(a, dtype=np.float32)
    b, q = core // 4, core % 4
    hi, lo = 7 - q, q           # slot0 = chunk hi, slot1 = chunk lo
    xT = shared["xT"][b]                                   # (C, T)
    loc_cols = np.r_[np.arange(hi * CHUNK, (hi + 1) * CHUNK),
                     np.arange(lo * CHUNK, (lo + 1) * CHUNK)]
    xloc = xT[:, loc_cols]
    posq = loc_cols.astype(np.float32)
    cq, sq = _rope_tables(posq)                            # (32, 512)

    # causal masks: cols 0:2048 slot0 kb 8..15 ; cols 2048:4096 slot1 kb 0..7
    kmask = np.zeros((P, 16 * CHUNK), np.float32)
    ki = np.arange(P)[:, None]
    qi = np.arange(CHUNK)[None, :]
    for half, (j, kbs) in enumerate(((hi, range(8, 16)), (lo, range(0, 8)))):
        for i, kb in enumerate(kbs):
            m = np.zeros((P, CHUNK), np.float32)
            if kb < 2 * j:
                m[:] = 1.0
            elif kb == 2 * j:
                m = (ki <= qi).astype(np.float32)
            elif kb == 2 * j + 1:
                m = (ki + P <= qi).astype(np.float32)
            col = half * 8 * CHUNK + i * CHUNK
            kmask[:, col:col + CHUNK] = m

    m = dict(shared["shared"])
    m.update({
        "xt": shared["xt_bf"][b],
        "xloc": f32(_blk(xloc)),
        "xlbf": bf(_blk(xloc)),
        "cosq": f32(np.vstack([cq, cq])), "ssinq": f32(np.vstack([sq, sq])),
        "kmask": bf(kmask),
    })
    return m


LAST_RESULTS = None


def kernel(**inputs):
    global LAST_RESULTS
    nc = _build()
    shared = _host_shared(inputs)
    in_maps = [_host_inputs(inputs, core, shared) for core in range(8)]
    kw = {}
    if os.environ.get("BASSK_TRACE"):
        kw = dict(trace=True, trace_cores=[0], stitch_traces=False)
    res = bass_utils.run_bass_kernel_spmd(nc, in_maps, core_ids=list(range(8)),
                                          **kw)
    LAST_RESULTS = res
    out = np.empty((B, T, C), np.float32)
    for core in range(8):
        b, q = core // 4, core % 4
        hi, lo = 7 - q, q
        oT = res.results[core]["outT"]                 # (128, NCB, TLOC)
        full = oT.transpose(1, 0, 2).reshape(C, TLOC)  # (C, 512)
        out[b, hi * CHUNK:(hi + 1) * CHUNK] = full[:, :CHUNK].T
        out[b, lo * CHUNK:(lo + 1) * CHUNK] = full[:, CHUNK:].T
    return out
